# revision 14
# baseline (speedup 1.0000x reference)
"""Trainium2 Bass kernel for nn_EnergyModel (bonded + Lennard-Jones energy).

Distribution: the [N,N] LJ pairwise term is upper-triangular; its 544
128x512 tiles are packed per-core (68 tiles = 17 dense [128,2048] strips)
so each of the 8 NeuronCores streams ~36MB of perfectly-sequential DMA
(half of the naive 512MB total). Positions and bonded lists are tiny and
split 1/8 per core. Each core emits one partial energy; host sums 8.

Device pipeline per strip:
  PE    : d2 = -2*pos_i.pos_j + |pos_j|^2 via a 21-row bf16 triple-split
          matmul (exact products + fp32 PSUM accumulate -- native fp32
          matmul is fp32r, far too coarse for the |pi-pj|^2 cancellation)
  ACT   : dm = Abs(psum + |pos_i|^2 [+ 1e-3 on diagonal tiles])
  DVE   : i2 = reciprocal_approx_fast(dm)                  (~51 ULP)
          t  = (u^3 - 1/2)^2, u = i2*sigma^2               (custom op)
          acc += eps*(4t - 1)                              (custom op,
                     chained per-partition running sum)
using 4*eps*(r12 - r6) = eps*(4t - 1), t = ((s/d)^6 - 1/2)^2.

Near pairs (exact d2 < 0.02): the reference's fp32 rounding of
|pi|^2+|pj|^2-2pi.pj is quantized at ~1.9e-6 and amplified x6 by r12 (the
single nearest pair carries ~96% of the total energy). The host finds
them with an O(N) spatial hash, replicates the reference's fp32 d2
bitwise (numpy sgemm == jax CPU, verified), zeroes those sigma/eps in the
packed tiles, and routes them through the same device chain as a small
"virtual pairs" tile with host-supplied dm.
"""

import itertools
import sys
from collections import defaultdict
from operator import add as _op_add

import numpy as np

sys.path.insert(0, "/opt/trn_rl_repo")

import ml_dtypes  # noqa: E402
from concourse import bass, bacc, mybir, tile  # noqa: E402
from concourse.bass_utils import run_bass_kernel_spmd  # noqa: E402
from concourse import dve_ops  # noqa: E402
from concourse.dve_ops import DveOp, OPS  # noqa: E402
from concourse.dve_spec import (  # noqa: E402
    Spec, Src0, Src1, C0, C1, C2, sq, lower, _has_src1,
)
from concourse.dve_uop import DveOpSpec  # noqa: E402

N_ATOMS = 8192
N_CORES = 8
RB = 128
CT = 512
N_RB = N_ATOMS // RB
N_CT = N_ATOMS // CT
TILES_PER_STRIP = 4            # packing granularity (dram layout unit)
STRIP_W = TILES_PER_STRIP * CT  # dram strips stay [128, 2048]
FUSE = 1                        # DVE processes FUSE dram strips per pass
CAND_D2 = 0.02
KROWS = 21
DIAG_EPS = 1e-3   # keeps diagonal-tile dm safely > 0 for the reciprocal

F32 = mybir.dt.float32
BF16 = mybir.dt.bfloat16
AF = mybir.ActivationFunctionType
ALU = mybir.AluOpType
PI = float(np.pi)

LAST_DEBUG = {}


# --------------------------------------------------------------------------
# custom DVE ops
# --------------------------------------------------------------------------
def _register_custom_op(name, spec, subdim=False):
    for o in OPS:
        if o.name == name:
            return o
    row = dve_ops._CUSTOM_DVE_ROW_BASE + len(OPS)
    dve_ops._SUB_OPCODE_FOR_NAME[name] = row
    shas = {}
    for ver in ("v3", "v4"):
        s = DveOpSpec(name=name, opcode=row, uops=lower(spec, ver=ver),
                      rd1_en=_has_src1(spec))
        shas[ver] = s.sha(ver)
    op = DveOp(name, spec, subdim=subdim, uops_sha=shas)
    OPS.append(op)
    dve_ops.CUSTOM_DVE_SPECS[name] = spec
    return op


def _lj_t_ref(in0, in1, s0, s1, imm2):
    u = (in0.astype(np.float32) * (in1.astype(np.float32) ** 2)).astype(np.float32)
    u3 = (u * u * u).astype(np.float32)
    return ((u3 + s0) ** 2).astype(np.float32)


_u = Src0 * sq(Src1)
_u3 = sq(_u) * _u
LJ_T = _register_custom_op("LJ_T_ANT", Spec(body=sq(_u3 + C0), reference=_lj_t_ref))


def _lj_acc_ref(in0, in1, s0, s1, imm2):
    b = (in0.astype(np.float32)
         * (in1.astype(np.float32) * s1 + imm2)).astype(np.float32)
    return b, s0 + b.reshape(b.shape[0], -1).sum(-1, keepdims=True)


LJ_ACC = _register_custom_op(
    "LJ_ACC_ANT",
    Spec(body=Src0 * (Src1 * C1 + C2), accum=_op_add, accum_init=C0,
         reference=_lj_acc_ref))


def _lj_recip_mul_ref(in0, in1, s0, s1, imm2):
    not_x = (~np.ascontiguousarray(in0, np.float32).view(np.int32)).view(np.float32)
    y0 = (not_x * np.float32(s0)).astype(np.float32)
    y1 = (y0 * (np.float32(s1) - in0 * y0)).astype(np.float32)
    return ((in1.astype(np.float32) * in1) * y1).astype(np.float32)


from concourse.dve_spec import Bin, AluOp as _AluOp
_ny0 = Bin(_AluOp.BITWISE_NOT, Src0, Src0) * C0
_ny1 = _ny0 * (C1 - Src0 * _ny0)
LJ_RECIP_MUL = _register_custom_op(
    "LJ_RECIP_MUL_ANT",
    Spec(body=sq(Src1) * _ny1, reference=_lj_recip_mul_ref))


def _lj_tail_ref(in0, in1, s0, s1, imm2):
    u3 = (in0.astype(np.float32) ** 2 * in0).astype(np.float32)
    w2 = ((u3 + s0) * s1).astype(np.float32)
    b = ((w2 * w2 + imm2) * in1.astype(np.float32)).astype(np.float32)
    return b, b.reshape(b.shape[0], -1).sum(-1, keepdims=True)


_tu3 = sq(Src0) * Src0
_tw2 = (_tu3 + C0) * C1
LJ_TAIL = _register_custom_op(
    "LJ_TAIL_ANT",
    Spec(body=(sq(_tw2) + C2) * Src1, accum=_op_add,
         reference=_lj_tail_ref))


def _mul_sq_acc_ref(in0, in1, s0, s1, imm2):
    b = ((in0.astype(np.float32) ** 2) * in1.astype(np.float32)).astype(np.float32)
    return b, b.reshape(b.shape[0], -1).sum(-1, keepdims=True)


MUL_SQ_ACC = _register_custom_op(
    "MUL_SQ_ACC_ANT",
    Spec(body=sq(Src0) * Src1, accum=_op_add, reference=_mul_sq_acc_ref))


def _add1_mul_acc_ref(in0, in1, s0, s1, imm2):
    b = ((in0.astype(np.float32) + np.float32(1.0))
         * in1.astype(np.float32)).astype(np.float32)
    return b, b.reshape(b.shape[0], -1).sum(-1, keepdims=True)


from concourse.dve_spec import One as _One
ADD1_MUL_ACC = _register_custom_op(
    "ADD1_MUL_ACC_ANT",
    Spec(body=(Src0 + _One) * Src1, accum=_op_add,
         reference=_add1_mul_acc_ref))


# --------------------------------------------------------------------------
# host helpers
# --------------------------------------------------------------------------
def _bf16(x):
    y = np.ascontiguousarray(x, np.float32).view(np.uint32)
    r = ((y + np.uint32(0x8000) + ((y >> np.uint32(16)) & np.uint32(1)))
         & np.uint32(0xFFFF0000)).view(np.float32)
    return r.reshape(np.shape(x))


def _to_bf16(x):
    """Fast fp32 -> bf16 (round-to-nearest-even) via integer ops."""
    y = np.ascontiguousarray(x, np.float32).view(np.uint32)
    r = ((y + np.uint32(0x8000) + ((y >> np.uint32(16)) & np.uint32(1)))
         >> np.uint32(16)).astype(np.uint16)
    return r.view(ml_dtypes.bfloat16).reshape(np.shape(x))


def _split3(x):
    a1 = _bf16(x)
    r = (x - a1).astype(np.float32)
    a2 = _bf16(r)
    a3 = _bf16((r - a2).astype(np.float32))
    return a1, a2, a3


_SPLIT_PAIRS = [(0, 0), (0, 1), (1, 0), (0, 2), (2, 0), (1, 1)]


def _tile_list():
    tiles = []
    for rb in range(N_RB):
        for ct in range(rb * RB // CT, N_CT):
            tiles.append((rb, ct))
    return tiles


def _find_candidates(pos):
    p = pos.astype(np.float64)
    cell = 0.15
    keys = np.floor(p / cell).astype(np.int64)
    grid = defaultdict(list)
    for idx in range(p.shape[0]):
        grid[tuple(keys[idx])].append(idx)
    offs = list(itertools.product((-1, 0, 1), repeat=3))
    cand = set()
    for key, members in grid.items():
        for off in offs:
            other = grid.get((key[0] + off[0], key[1] + off[1], key[2] + off[2]))
            if not other:
                continue
            for i in members:
                pi = p[i]
                for j in other:
                    if j > i:
                        d = pi - p[j]
                        if d[0] * d[0] + d[1] * d[1] + d[2] * d[2] < CAND_D2:
                            cand.add((i, j))
    return sorted(cand)


def _ref_d2_for_pairs(pos, pairs):
    """Bitwise replication of the reference's fp32 d2 for the given pairs."""
    if not pairs:
        return np.zeros(0, np.float32)
    sq32 = np.sum(pos * pos, axis=-1)
    rows = sorted({i for i, _ in pairs})
    ridx = {i: k for k, i in enumerate(rows)}
    dmat = (sq32[rows][:, None] + sq32[None, :]
            - np.float32(2.0) * (pos[rows] @ pos.T))
    return np.array([dmat[ridx[i], j] for i, j in pairs], np.float32)


def _pack_fields(fields, n_items):
    npart = n_items // 128
    out = np.empty((128, len(fields) * npart), np.float32)
    for f, arr in enumerate(fields):
        out[:, f * npart:(f + 1) * npart] = np.asarray(arr, np.float32).reshape(128, npart)
    return out


# --------------------------------------------------------------------------
# device program
# --------------------------------------------------------------------------
_PROGRAM_CACHE = {}


def _build_program(n_strips, vw, nb, na, nd):
    key = (n_strips, vw, nb, na, nd)
    if key in _PROGRAM_CACHE:
        return _PROGRAM_CACHE[key]

    nc = bacc.Bacc("TRN2", target_bir_lowering=False, debug=False,
                   num_devices=N_CORES)
    n_tiles = n_strips * TILES_PER_STRIP
    sig_d = nc.dram_tensor("sig", [n_strips, RB, STRIP_W], BF16, kind="ExternalInput")
    eps_d = nc.dram_tensor("eps", [n_strips, RB, STRIP_W], BF16, kind="ExternalInput")
    meta_d = nc.dram_tensor("meta", [KROWS, n_tiles * (CT + RB)], BF16,
                            kind="ExternalInput")
    sqi_d = nc.dram_tensor("sqi", [RB, n_tiles], F32, kind="ExternalInput")
    vdm_d = nc.dram_tensor("vdm", [128, vw], F32, kind="ExternalInput")
    vsig_d = nc.dram_tensor("vsig", [128, vw], F32, kind="ExternalInput")
    veps_d = nc.dram_tensor("veps", [128, vw], F32, kind="ExternalInput")
    bp_d = nc.dram_tensor("bpack", [128, 8 * nb], F32, kind="ExternalInput")
    ap_d = nc.dram_tensor("apack", [128, 11 * na], F32, kind="ExternalInput")
    dp_d = nc.dram_tensor("dpack", [128, 15 * nd], F32, kind="ExternalInput")
    out_d = nc.dram_tensor("out", [1, 8], F32, kind="ExternalOutput")

    tagn = [0]

    with tile.TileContext(nc) as tc:
        with (
            tc.tile_pool(name="const", bufs=1) as cp,
            tc.tile_pool(name="sigp", bufs=3) as sigp,
            tc.tile_pool(name="epsp", bufs=3) as epsp,
            tc.tile_pool(name="dmp", bufs=2) as dmp,
            tc.tile_pool(name="i2p", bufs=2) as i2p,
            tc.tile_pool(name="ttp", bufs=2) as ttp,
            tc.tile_pool(name="accp", bufs=3) as accp,
            tc.tile_pool(name="bw", bufs=1) as bw,
            tc.tile_pool(name="drp", bufs=1, space=bass.MemorySpace.DRAM) as drp,
            tc.tile_pool(name="psp", bufs=3, space=bass.MemorySpace.PSUM) as psp,
        ):
            def wtile(shape, pool=bw, dtype=F32):
                tagn[0] += 1
                return pool.tile(shape, dtype, tag=f"w{tagn[0]}",
                                 name=f"w{tagn[0]}")

            meta = cp.tile([KROWS, n_tiles * (CT + RB)], BF16)
            # split the load so the first strips' matmuls start early
            mcw = n_tiles * (CT + RB)
            cut1 = (n_tiles // 3) * (CT + RB)
            cut2 = (2 * n_tiles // 3) * (CT + RB)
            for lo, hi in [(0, cut1), (cut1, cut2), (cut2, mcw)]:
                if hi > lo:
                    nc.sync.dma_start(meta[:, lo:hi], meta_d.ap()[:, lo:hi])
            sqi = cp.tile([RB, n_tiles], F32)
            nc.sync.dma_start(sqi[:], sqi_d.ap())

            from concourse.dve_ops import RECIP_APPROX_FAST_CONSTS as _RC
            _rc0, _rc1 = _RC["s0"], _RC["s1"]
            naccw = max(1, n_strips)
            saccs = cp.tile([128, naccw], F32)
            nc.gpsimd.memset(saccs[:], 0.0)

            # ------------- LJ main loop (2 DVE passes / fused group) ---------
            groups = []
            s0_ = 0
            while s0_ < n_strips:
                groups.append(list(range(s0_, min(s0_ + FUSE, n_strips))))
                s0_ += FUSE
            for gi, grp in enumerate(groups):
                gw = len(grp) * STRIP_W
                sig_t = sigp.tile([RB, FUSE * STRIP_W], BF16, tag="sig")
                eps_t = epsp.tile([RB, FUSE * STRIP_W], BF16, tag="eps")
                dm_t = dmp.tile([RB, FUSE * STRIP_W], F32, tag="dm")
                for li, s in enumerate(grp):
                    off = li * STRIP_W
                    nc.sync.dma_start(sig_t[:, off:off + STRIP_W], sig_d.ap()[s])
                    nc.sync.dma_start(eps_t[:, off:off + STRIP_W], eps_d.ap()[s])
                    for h in range(2):
                        ps_t = psp.tile([128, 1024], F32, tag="ps")
                        for q in range(2):
                            tg = s * TILES_PER_STRIP + h * 2 + q
                            base = tg * (CT + RB)
                            nc.tensor.matmul(
                                ps_t[:, q * CT:(q + 1) * CT],
                                meta[:, base + CT: base + CT + RB],
                                meta[:, base: base + CT],
                                start=True, stop=True)
                            nc.scalar.activation(
                                dm_t[:, off + (h * 2 + q) * CT:off + (h * 2 + q + 1) * CT],
                                ps_t[:, q * CT:(q + 1) * CT],
                                AF.Abs, bias=sqi[:, tg:tg + 1], scale=1.0)
                u_t = i2p.tile([RB, FUSE * STRIP_W], F32, tag="i2")
                nc.vector._custom_dve(LJ_RECIP_MUL, out=u_t[:, 0:gw],
                                      in0=dm_t[:, 0:gw],
                                      in1=sig_t[:, 0:gw], s0=_rc0, s1=_rc1)
                nc.vector._custom_dve(LJ_TAIL, out=dm_t[:, 0:gw],
                                      in0=u_t[:, 0:gw],
                                      in1=eps_t[:, 0:gw], s0=-0.5, s1=2.0,
                                      imm2=-1.0, accum_out=saccs[:, gi:gi + 1])
            acc_prev = accp.tile([128, 1], F32, tag="acc")
            nc.vector.tensor_reduce(out=acc_prev[:], in_=saccs[:],
                                    axis=mybir.AxisListType.X, op=ALU.add)

            # ---------------- virtual near pairs ----------------
            vdm = cp.tile([128, vw], F32)
            nc.sync.dma_start(vdm[:], vdm_d.ap())
            vsig = cp.tile([128, vw], F32)
            nc.sync.dma_start(vsig[:], vsig_d.ap())
            veps = cp.tile([128, vw], F32)
            nc.sync.dma_start(veps[:], veps_d.ap())
            vi2 = wtile([128, vw])
            nc.vector.reciprocal_approx_fast(out=vi2[:], in_=vdm[:])
            vt = wtile([128, vw])
            nc.vector._custom_dve(LJ_T, out=vt[:], in0=vi2[:], in1=vsig[:], s0=-0.5)
            vscr = wtile([128, vw])
            acc_lj = accp.tile([128, 1], F32, tag="acc")
            nc.vector._custom_dve(LJ_ACC, out=vscr[:], in0=veps[:], in1=vt[:],
                                  s0=acc_prev[:], s1=4.0, imm2=-1.0,
                                  accum_out=acc_lj[:])

            # ---------------- bonded-term helpers ----------------
            def tt(op, a, b, shape):
                o = wtile(shape)
                nc.vector.tensor_tensor(out=o[:], in0=a, in1=b, op=op)
                return o[:]

            def ts(a, op0, s1, op1=None, s2=None, shape=None):
                o = wtile(shape)
                if op1 is None:
                    nc.vector.tensor_scalar(out=o[:], in0=a, scalar1=s1,
                                            scalar2=None, op0=op0)
                else:
                    nc.vector.tensor_scalar(out=o[:], in0=a, scalar1=s1,
                                            scalar2=s2, op0=op0, op1=op1)
                return o[:]

            def act(fn, a, shape, scale=1.0):
                o = wtile(shape)
                nc.scalar.activation(o[:], a, fn, scale=scale)
                return o[:]

            def recip(a, shape):
                o = wtile(shape)
                nc.vector.reciprocal_approx_fast(out=o[:], in_=a)
                return o[:]

            def dot3(a, b, shape):
                m = [tt(ALU.mult, a[k], b[k], shape) for k in range(3)]
                s12 = tt(ALU.add, m[0], m[1], shape)
                return tt(ALU.add, s12, m[2], shape)

            def cross(a, b, shape):
                def comp(p, q, r, s):
                    t1 = tt(ALU.mult, p, q, shape)
                    t2 = tt(ALU.mult, r, s, shape)
                    return tt(ALU.subtract, t1, t2, shape)
                return [comp(a[1], b[2], a[2], b[1]),
                        comp(a[2], b[0], a[0], b[2]),
                        comp(a[0], b[1], a[1], b[0])]

            # ---------------- bonds ----------------
            bsh = [128, nb]
            bp = cp.tile([128, 8 * nb], F32)
            nc.sync.dma_start(bp[:], bp_d.ap())
            bF = [bp[:, f * nb:(f + 1) * nb] for f in range(8)]
            bw3 = [128, 3 * nb]
            d1w = tt(ALU.subtract, bp[:, 0:3 * nb], bp[:, 3 * nb:6 * nb], bw3)
            d1sq = tt(ALU.mult, d1w, d1w, bw3)
            d2b = wtile(bsh)
            nc.vector.tensor_reduce(
                out=d2b[:], in_=d1sq.rearrange("p (c n) -> p n c", c=3),
                axis=mybir.AxisListType.X, op=ALU.add)
            d2b = d2b[:]
            bd = act(AF.Sqrt, d2b, bsh)
            db = tt(ALU.subtract, bd, bF[7], bsh)
            eb_acc = wtile([128, 1])
            ebscr = wtile(bsh)
            nc.vector._custom_dve(MUL_SQ_ACC, out=ebscr[:], in0=db,
                                  in1=bF[6], accum_out=eb_acc[:])

            # ---------------- angles ----------------
            ash = [128, na]
            apk = cp.tile([128, 11 * na], F32)
            nc.sync.dma_start(apk[:], ap_d.ap())
            aF = [apk[:, f * na:(f + 1) * na] for f in range(11)]
            aw3 = [128, 3 * na]

            def _sred(wide, n_):
                o = wtile([128, n_])
                nc.vector.tensor_reduce(
                    out=o[:], in_=wide.rearrange("p (c n) -> p n c", c=3),
                    axis=mybir.AxisListType.X, op=ALU.add)
                return o[:]

            v1w = tt(ALU.subtract, apk[:, 3 * na:6 * na], apk[:, 0:3 * na], aw3)
            v2w = tt(ALU.subtract, apk[:, 3 * na:6 * na], apk[:, 6 * na:9 * na], aw3)
            dota = _sred(tt(ALU.mult, v1w, v2w, aw3), na)
            n1sq = _sred(tt(ALU.mult, v1w, v1w, aw3), na)
            n2sq = _sred(tt(ALU.mult, v2w, v2w, aw3), na)
            den2 = tt(ALU.mult, n1sq, n2sq, ash)
            den = act(AF.Sqrt, den2, ash)
            rden = recip(den, ash)
            cosa = tt(ALU.mult, dota, rden, ash)
            c2 = tt(ALU.mult, cosa, cosa, ash)
            omc = ts(c2, ALU.mult, -1.0, ALU.add, 1.0, shape=ash)
            sroot = act(AF.Sqrt, omc, ash)
            rs = recip(sroot, ash)
            targ = tt(ALU.mult, cosa, rs, ash)
            at = act(AF.Arctan, targ, ash)
            ang = ts(at, ALU.mult, -1.0, ALU.add, PI / 2, shape=ash)
            da = tt(ALU.subtract, ang, aF[10], ash)
            ea_acc = wtile([128, 1])
            eascr = wtile(ash)
            nc.vector._custom_dve(MUL_SQ_ACC, out=eascr[:], in0=da,
                                  in1=aF[9], accum_out=ea_acc[:])

            # ---------------- dihedrals ----------------
            dsh = [128, nd]
            dpk = cp.tile([128, 15 * nd], F32)
            nc.sync.dma_start(dpk[:], dp_d.ap())
            dF = [dpk[:, f * nd:(f + 1) * nd] for f in range(15)]
            dw3 = [128, 3 * nd]
            dw9 = [128, 9 * nd]
            www = wtile(dw9)  # w1|w2|w3 in one wide tile
            nc.vector.tensor_tensor(out=www[:], in0=dpk[:, 3 * nd:12 * nd],
                                    in1=dpk[:, 0:9 * nd], op=ALU.subtract)
            w1 = [www[:, k * nd:(k + 1) * nd] for k in range(3)]
            w2 = [www[:, (3 + k) * nd:(4 + k) * nd] for k in range(3)]
            w3 = [www[:, (6 + k) * nd:(7 + k) * nd] for k in range(3)]

            def _sredd(wide, n_):
                o = wtile([128, n_])
                nc.vector.tensor_reduce(
                    out=o[:], in_=wide.rearrange("p (c n) -> p n c", c=3),
                    axis=mybir.AxisListType.X, op=ALU.add)
                return o[:]

            n1w = wtile(dw3)
            n2w = wtile(dw3)

            def cross_into(dst, a, b):
                def comp(k, p, q, r, s):
                    t1 = tt(ALU.mult, p, q, dsh)
                    t2 = tt(ALU.mult, r, s, dsh)
                    nc.vector.tensor_tensor(out=dst[:, k * nd:(k + 1) * nd],
                                            in0=t1, in1=t2, op=ALU.subtract)
                comp(0, a[1], b[2], a[2], b[1])
                comp(1, a[2], b[0], a[0], b[2])
                comp(2, a[0], b[1], a[1], b[0])

            cross_into(n1w, w1, w2)
            cross_into(n2w, w2, w3)
            cdn = _sredd(tt(ALU.mult, n1w[:], n2w[:], dw3), nd)
            # (n1 x n2).w2 == (w1.n2)*|w2|^2  (Lagrange triple product)
            det = _sredd(tt(ALU.mult, www[:, 0:3 * nd], n2w[:], dw3), nd)
            wsqw = tt(ALU.mult, www[:, 0:6 * nd], www[:, 0:6 * nd], [128, 6 * nd])
            w1sq = _sredd(wsqw[:, 0:3 * nd], nd)
            w2sq = _sredd(wsqw[:, 3 * nd:6 * nd], nd)
            n1sq_ = _sredd(tt(ALU.mult, n1w[:], n1w[:], dw3), nd)
            n2sq_ = _sredd(tt(ALU.mult, n2w[:], n2w[:], dw3), nd)
            cden2 = tt(ALU.mult, w1sq, w2sq, dsh)
            cden = act(AF.Sqrt, cden2, dsh)
            rcden = recip(cden, dsh)
            cosd = tt(ALU.mult, cdn, rcden, dsh)
            sd1 = tt(ALU.mult, w2sq, n1sq_, dsh)
            sden2 = tt(ALU.mult, sd1, n2sq_, dsh)
            sden = act(AF.Sqrt, sden2, dsh)
            rsden = recip(sden, dsh)
            sdn = tt(ALU.mult, det, w2sq, dsh)
            sind = tt(ALU.mult, sdn, rsden, dsh)
            rcosd = recip(cosd, dsh)
            qd = tt(ALU.mult, sind, rcosd, dsh)
            atq = act(AF.Arctan, qd, dsh)
            sgn = act(AF.Sign, sind, dsh)
            neg = ts(cosd, ALU.is_lt, 0.0, shape=dsh)
            corr0 = tt(ALU.mult, sgn, neg, dsh)
            corr = ts(corr0, ALU.mult, PI, shape=dsh)
            dih = tt(ALU.add, atq, corr, dsh)
            narg = tt(ALU.mult, dih, dF[14], dsh)
            arg = tt(ALU.subtract, narg, dF[13], dsh)
            wr1 = wtile(dsh)
            nc.vector.add_range_wrap(out=wr1[:], in_=arg, shift=PI / 2,
                                     bound=PI, period=2 * PI)
            wr2 = wtile(dsh)
            nc.vector.add_range_wrap(out=wr2[:], in_=wr1[:], shift=0.0,
                                     bound=PI, period=2 * PI)
            sn = act(AF.Sin, wr2[:], dsh)
            ed_acc = wtile([128, 1])
            edscr = wtile(dsh)
            nc.vector._custom_dve(ADD1_MUL_ACC, out=edscr[:], in0=sn,
                                  in1=dF[12], accum_out=ed_acc[:])

            # ---------------- reductions / output ----------------
            comb = cp.tile([128, 4], F32)
            nc.vector.tensor_copy(comb[:, 0:1], acc_lj[:])
            for col, r_ in enumerate([eb_acc, ea_acc, ed_acc]):
                nc.scalar.mul(comb[:, col + 1:col + 2], r_[:], 0.5)

            dscr = drp.tile([1, 512], F32)
            dview = dscr[:].rearrange("x (p c) -> (x p) c", p=128)
            nc.sync.dma_start(dview, comb[:])
            flat = cp.tile([1, 512], F32)
            nc.sync.dma_start(flat[:], dscr[:])
            fview = flat[:].rearrange("p (a b) -> p b a", b=4)  # [1, 4, 128]
            sums = cp.tile([1, 4], F32)
            nc.vector.tensor_reduce(out=sums[:], in_=fview,
                                    axis=mybir.AxisListType.X, op=ALU.add)
            total = cp.tile([1, 1], F32)
            nc.vector.tensor_reduce(out=total[:], in_=sums[:],
                                    axis=mybir.AxisListType.X, op=ALU.add)
            outt = cp.tile([1, 8], F32)
            nc.gpsimd.memset(outt[:], 0.0)
            nc.vector.tensor_copy(outt[:, 0:1], total[:])
            nc.vector.tensor_copy(outt[:, 1:5], sums[:])
            nc.sync.dma_start(out_d.ap(), outt[:])

    nc.compile()
    _PROGRAM_CACHE[key] = nc
    return nc


# --------------------------------------------------------------------------
# host packing + dispatch
# --------------------------------------------------------------------------
def _prepare_core_inputs(inputs):
    pos = np.ascontiguousarray(inputs["positions"], np.float32)
    sigma = inputs["sigma"]
    eps = inputs["epsilon"]

    tiles = _tile_list()
    n_per_core = len(tiles) // N_CORES
    n_strips = n_per_core // TILES_PER_STRIP
    core_tiles = [tiles[c * n_per_core:(c + 1) * n_per_core]
                  for c in range(N_CORES)]
    tile_owner = {}
    for c in range(N_CORES):
        for k, t in enumerate(core_tiles[c]):
            tile_owner[t] = (c, k)

    cand = _find_candidates(pos)
    cand_d2 = _ref_d2_for_pairs(pos, cand)
    vc = [[] for _ in range(N_CORES)]
    for k, pr in enumerate(cand):
        vc[k % N_CORES].append((pr, cand_d2[k]))
    vmax = max((len(v) for v in vc), default=0)
    vw = max(1, -(-max(vmax, 1) // 128))

    a1, a2, a3 = _split3(pos)
    A = [a1, a2, a3]
    sq32 = np.sum(pos * pos, axis=-1)
    sqh = _bf16(sq32)
    sql = _bf16((sq32 - sqh).astype(np.float32))
    sql2 = _bf16((sq32 - sqh - sql).astype(np.float32))

    bidx = inputs["bond_idx"]; kb = inputs["k_bond"]; r0 = inputs["r0"]
    aidx = inputs["angle_idx"]; ka = inputs["k_angle"]; th0 = inputs["theta0"]
    didx = inputs["dihedral_idx"]; kd = inputs["k_dihedral"]
    ph = inputs["default_phase"]; nm = inputs["n_mult"]
    nb = len(kb) // N_CORES // 128
    na = len(ka) // N_CORES // 128
    nd = len(kd) // N_CORES // 128

    in_maps = []
    for c in range(N_CORES):
        sig_pack = np.empty((n_strips, RB, STRIP_W), ml_dtypes.bfloat16)
        eps_pack = np.empty((n_strips, RB, STRIP_W), ml_dtypes.bfloat16)
        meta = np.zeros((KROWS, n_per_core * (CT + RB)), np.float32)
        sqi = np.empty((RB, n_per_core), np.float32)
        for k, (rb, ct) in enumerate(core_tiles[c]):
            s, slot = divmod(k, TILES_PER_STRIP)
            rs, cs = rb * RB, ct * CT
            sig_pack[s, :, slot * CT:(slot + 1) * CT] = _to_bf16(sigma[rs:rs + RB, cs:cs + CT])
            eps_pack[s, :, slot * CT:(slot + 1) * CT] = _to_bf16(eps[rs:rs + RB, cs:cs + CT])
            base = k * (CT + RB)
            for pi_, (u, v) in enumerate(_SPLIT_PAIRS):
                for ax in range(3):
                    r = pi_ * 3 + ax
                    meta[r, base: base + CT] = A[v][cs:cs + CT, ax]
                    meta[r, base + CT: base + CT + RB] = \
                        A[u][rs:rs + RB, ax] * np.float32(-2.0)
            meta[18, base: base + CT] = sqh[cs:cs + CT]
            meta[19, base: base + CT] = sql[cs:cs + CT]
            meta[20, base: base + CT] = sql2[cs:cs + CT]
            meta[18:21, base + CT: base + CT + RB] = 1.0
            diag = (ct == rb * RB // CT)
            sqi[:, k] = sq32[rs:rs + RB] + (np.float32(DIAG_EPS) if diag else np.float32(0.0))

        meta_bf = np.ascontiguousarray(_bf16(meta).astype(ml_dtypes.bfloat16))

        vdm = np.ones((128, vw), np.float32)
        vsig = np.zeros((128, vw), np.float32)
        veps = np.zeros((128, vw), np.float32)
        for k, ((i, j), d2v) in enumerate(vc[c]):
            p_, q_ = k % 128, k // 128
            dist = np.float32(np.sqrt(np.float32(max(d2v, np.float32(0.0))))) + np.float32(1e-9)
            vdm[p_, q_] = np.float32(dist * dist)
            vsig[p_, q_] = sigma[i, j]
            veps[p_, q_] = eps[i, j]

        def seg(arr, n_each):
            return np.ascontiguousarray(arr[c * n_each:(c + 1) * n_each])

        bs = seg(bidx, nb * 128)
        g1, g2 = pos[bs[:, 0]], pos[bs[:, 1]]
        bpack = _pack_fields(
            [g1[:, 0], g1[:, 1], g1[:, 2], g2[:, 0], g2[:, 1], g2[:, 2],
             seg(kb, nb * 128), seg(r0, nb * 128)], nb * 128)
        asx = seg(aidx, na * 128)
        g1, g2, g3 = pos[asx[:, 0]], pos[asx[:, 1]], pos[asx[:, 2]]
        apack = _pack_fields(
            [g1[:, 0], g1[:, 1], g1[:, 2], g2[:, 0], g2[:, 1], g2[:, 2],
             g3[:, 0], g3[:, 1], g3[:, 2],
             seg(ka, na * 128), seg(th0, na * 128)], na * 128)
        dsx = seg(didx, nd * 128)
        g1, g2, g3, g4 = (pos[dsx[:, 0]], pos[dsx[:, 1]],
                          pos[dsx[:, 2]], pos[dsx[:, 3]])
        dpack = _pack_fields(
            [g1[:, 0], g1[:, 1], g1[:, 2], g2[:, 0], g2[:, 1], g2[:, 2],
             g3[:, 0], g3[:, 1], g3[:, 2], g4[:, 0], g4[:, 1], g4[:, 2],
             seg(kd, nd * 128), seg(ph, nd * 128), seg(nm, nd * 128)],
            nd * 128)

        in_maps.append({
            "sig": sig_pack, "eps": eps_pack, "meta": meta_bf, "sqi": sqi,
            "vdm": vdm, "vsig": vsig, "veps": veps,
            "bpack": bpack, "apack": apack, "dpack": dpack,
        })

    for k, (i, j) in enumerate(cand):
        rb, ct = i // RB, j // CT
        c, kt = tile_owner[(rb, ct)]
        s, slot = divmod(kt, TILES_PER_STRIP)
        col = slot * CT + (j - ct * CT)
        in_maps[c]["sig"][s, i - rb * RB, col] = 0.0
        in_maps[c]["eps"][s, i - rb * RB, col] = 0.0

    return in_maps, (n_strips, vw, nb, na, nd)


def kernel(**inputs):
    pos = np.asarray(inputs["positions"])
    sg = np.asarray(inputs["sigma"])
    ep = np.asarray(inputs["epsilon"])
    ok = (pos.shape == (N_ATOMS, 3) and sg.shape == (N_ATOMS, N_ATOMS)
          and ep.shape == (N_ATOMS, N_ATOMS)
          and len(inputs["k_bond"]) % (N_CORES * 128) == 0
          and len(inputs["k_angle"]) % (N_CORES * 128) == 0
          and len(inputs["k_dihedral"]) % (N_CORES * 128) == 0)
    if ok:
        idx = np.arange(0, N_ATOMS, 37)
        ii, jj = np.meshgrid(idx, idx, indexing="ij")
        low = ii > jj
        if sg[ii[low], jj[low]].any() or ep[ii[low], jj[low]].any():
            ok = False
    if not ok:
        return _host_fallback(inputs)

    try:
        in_maps, geom = _prepare_core_inputs(inputs)
        nc = _build_program(*geom)
        res = None
        for attempt in range(3):
            try:
                res = run_bass_kernel_spmd(nc, in_maps,
                                           core_ids=list(range(N_CORES)))
                break
            except Exception:
                if attempt == 2:
                    raise
                import time as _time
                _time.sleep(3.0)
    except Exception:
        # no devices / toolchain failure: fall back to the (slow) host path
        return _host_fallback(inputs)
    partials = np.stack([r["out"][0] for r in res.results])
    LAST_DEBUG["partials"] = partials
    total = np.float64(partials[:, 0]).sum()
    return np.float32(total)


def _host_fallback(inputs):
    """Numpy replication of the fp32 reference (safety net, not fast)."""
    pos = np.asarray(inputs["positions"], np.float32)
    sigma = np.asarray(inputs["sigma"], np.float32)
    eps = np.asarray(inputs["epsilon"], np.float32)
    n = pos.shape[0]
    sq32 = np.sum(pos * pos, axis=-1)
    lj = 0.0
    chunk = 512
    for s0 in range(0, n, chunk):
        s1 = min(s0 + chunk, n)
        d2 = (sq32[s0:s1, None] + sq32[None, :]
              - np.float32(2.0) * (pos[s0:s1] @ pos.T))
        dist = (np.sqrt(np.maximum(d2, 0)) + np.float32(1e-9)).astype(np.float64)
        r6 = (sigma[s0:s1].astype(np.float64) / dist) ** 6
        lj += float((4.0 * eps[s0:s1].astype(np.float64) * (r6 * r6 - r6)).sum())
    bi, bj = inputs["bond_idx"][:, 0], inputs["bond_idx"][:, 1]
    d2b = (sq32[bi] + sq32[bj]
           - np.float32(2.0) * np.sum(pos[bi] * pos[bj], -1, dtype=np.float32))
    bd = np.sqrt(np.maximum(d2b, 0)).astype(np.float64) + 1e-9
    bond_e = float(np.sum(0.5 * inputs["k_bond"] * (bd - inputs["r0"]) ** 2))
    p64 = pos.astype(np.float64)
    ai = inputs["angle_idx"]
    p1, p2, p3 = p64[ai[:, 0]], p64[ai[:, 1]], p64[ai[:, 2]]
    v1, v2 = p2 - p1, p2 - p3
    cos_a = np.sum(v1 * v2, -1) / (np.linalg.norm(v1, axis=1)
                                   * np.linalg.norm(v2, axis=1))
    angle_e = float(np.sum(0.5 * inputs["k_angle"]
                           * (np.arccos(np.clip(cos_a, -1, 1))
                              - inputs["theta0"]) ** 2))
    di = inputs["dihedral_idx"]
    q1, q2, q3, q4 = p64[di[:, 0]], p64[di[:, 1]], p64[di[:, 2]], p64[di[:, 3]]
    w1, w2, w3 = q2 - q1, q3 - q2, q4 - q3
    cn1, cn2 = np.cross(w1, w2), np.cross(w2, w3)
    cos_d = np.sum(cn1 * cn2, -1) / (np.linalg.norm(w1, axis=1)
                                     * np.linalg.norm(w2, axis=1))
    sin_d = np.sum(np.cross(cn1, cn2) * w2, -1) / (
        np.linalg.norm(w2, axis=1) * np.linalg.norm(cn1, axis=1)
        * np.linalg.norm(cn2, axis=1))
    dih = np.arctan2(sin_d, cos_d)
    dihedral_e = float(np.sum(0.5 * inputs["k_dihedral"]
                              * (1.0 + np.cos(inputs["n_mult"] * dih
                                              - inputs["default_phase"]))))
    return np.float32(lj + bond_e + angle_e + dihedral_e)


# revision 15
# speedup vs baseline: 1.0014x; 1.0014x over previous
"""Trainium2 Bass kernel for nn_EnergyModel (bonded + Lennard-Jones energy).

Distribution: the [N,N] LJ pairwise term is upper-triangular; its 544
128x512 tiles are packed per-core (68 tiles = 17 dense [128,2048] strips)
so each of the 8 NeuronCores streams ~36MB of perfectly-sequential DMA
(half of the naive 512MB total). Positions and bonded lists are tiny and
split 1/8 per core. Each core emits one partial energy; host sums 8.

Device pipeline per strip:
  PE    : d2 = -2*pos_i.pos_j + |pos_j|^2 via a 21-row bf16 triple-split
          matmul (exact products + fp32 PSUM accumulate -- native fp32
          matmul is fp32r, far too coarse for the |pi-pj|^2 cancellation)
  ACT   : dm = Abs(psum + |pos_i|^2 [+ 1e-3 on diagonal tiles])
  DVE   : i2 = reciprocal_approx_fast(dm)                  (~51 ULP)
          t  = (u^3 - 1/2)^2, u = i2*sigma^2               (custom op)
          acc += eps*(4t - 1)                              (custom op,
                     chained per-partition running sum)
using 4*eps*(r12 - r6) = eps*(4t - 1), t = ((s/d)^6 - 1/2)^2.

Near pairs (exact d2 < 0.02): the reference's fp32 rounding of
|pi|^2+|pj|^2-2pi.pj is quantized at ~1.9e-6 and amplified x6 by r12 (the
single nearest pair carries ~96% of the total energy). The host finds
them with an O(N) spatial hash, replicates the reference's fp32 d2
bitwise (numpy sgemm == jax CPU, verified), zeroes those sigma/eps in the
packed tiles, and routes them through the same device chain as a small
"virtual pairs" tile with host-supplied dm.
"""

import itertools
import sys
from collections import defaultdict
from operator import add as _op_add

import numpy as np

sys.path.insert(0, "/opt/trn_rl_repo")

import ml_dtypes  # noqa: E402
from concourse import bass, bacc, mybir, tile  # noqa: E402
from concourse.bass_utils import run_bass_kernel_spmd  # noqa: E402
from concourse import dve_ops  # noqa: E402
from concourse.dve_ops import DveOp, OPS  # noqa: E402
from concourse.dve_spec import (  # noqa: E402
    Spec, Src0, Src1, C0, C1, C2, sq, lower, _has_src1,
)
from concourse.dve_uop import DveOpSpec  # noqa: E402

N_ATOMS = 8192
N_CORES = 8
RB = 128
CT = 512
N_RB = N_ATOMS // RB
N_CT = N_ATOMS // CT
TILES_PER_STRIP = 4            # packing granularity (dram layout unit)
STRIP_W = TILES_PER_STRIP * CT  # dram strips stay [128, 2048]
FUSE = 1                        # DVE processes FUSE dram strips per pass
CAND_D2 = 0.02
KROWS = 21
DIAG_EPS = 1e-3   # keeps diagonal-tile dm safely > 0 for the reciprocal

F32 = mybir.dt.float32
BF16 = mybir.dt.bfloat16
AF = mybir.ActivationFunctionType
ALU = mybir.AluOpType
PI = float(np.pi)

LAST_DEBUG = {}


# --------------------------------------------------------------------------
# custom DVE ops
# --------------------------------------------------------------------------
def _register_custom_op(name, spec, subdim=False):
    for o in OPS:
        if o.name == name:
            return o
    row = dve_ops._CUSTOM_DVE_ROW_BASE + len(OPS)
    dve_ops._SUB_OPCODE_FOR_NAME[name] = row
    shas = {}
    for ver in ("v3", "v4"):
        s = DveOpSpec(name=name, opcode=row, uops=lower(spec, ver=ver),
                      rd1_en=_has_src1(spec))
        shas[ver] = s.sha(ver)
    op = DveOp(name, spec, subdim=subdim, uops_sha=shas)
    OPS.append(op)
    dve_ops.CUSTOM_DVE_SPECS[name] = spec
    return op


def _lj_t_ref(in0, in1, s0, s1, imm2):
    u = (in0.astype(np.float32) * (in1.astype(np.float32) ** 2)).astype(np.float32)
    u3 = (u * u * u).astype(np.float32)
    return ((u3 + s0) ** 2).astype(np.float32)


_u = Src0 * sq(Src1)
_u3 = sq(_u) * _u
LJ_T = _register_custom_op("LJ_T_ANT", Spec(body=sq(_u3 + C0), reference=_lj_t_ref))


def _lj_acc_ref(in0, in1, s0, s1, imm2):
    b = (in0.astype(np.float32)
         * (in1.astype(np.float32) * s1 + imm2)).astype(np.float32)
    return b, s0 + b.reshape(b.shape[0], -1).sum(-1, keepdims=True)


LJ_ACC = _register_custom_op(
    "LJ_ACC_ANT",
    Spec(body=Src0 * (Src1 * C1 + C2), accum=_op_add, accum_init=C0,
         reference=_lj_acc_ref))


def _lj_recip_mul_ref(in0, in1, s0, s1, imm2):
    not_x = (~np.ascontiguousarray(in0, np.float32).view(np.int32)).view(np.float32)
    y0 = (not_x * np.float32(s0)).astype(np.float32)
    y1 = (y0 * (np.float32(s1) - in0 * y0)).astype(np.float32)
    return ((in1.astype(np.float32) * in1) * y1).astype(np.float32)


from concourse.dve_spec import Bin, AluOp as _AluOp
_ny0 = Bin(_AluOp.BITWISE_NOT, Src0, Src0) * C0
_ny1 = _ny0 * (C1 - Src0 * _ny0)
LJ_RECIP_MUL = _register_custom_op(
    "LJ_RECIP_MUL_ANT",
    Spec(body=sq(Src1) * _ny1, reference=_lj_recip_mul_ref))


def _lj_tail_ref(in0, in1, s0, s1, imm2):
    u3 = (in0.astype(np.float32) ** 2 * in0).astype(np.float32)
    w2 = ((u3 + s0) * s1).astype(np.float32)
    b = ((w2 * w2 + imm2) * in1.astype(np.float32)).astype(np.float32)
    return b, b.reshape(b.shape[0], -1).sum(-1, keepdims=True)


_tu3 = sq(Src0) * Src0
_tw2 = (_tu3 + C0) * C1
LJ_TAIL = _register_custom_op(
    "LJ_TAIL_ANT",
    Spec(body=(sq(_tw2) + C2) * Src1, accum=_op_add,
         reference=_lj_tail_ref))


def _mul_sq_acc_ref(in0, in1, s0, s1, imm2):
    b = ((in0.astype(np.float32) ** 2) * in1.astype(np.float32)).astype(np.float32)
    return b, b.reshape(b.shape[0], -1).sum(-1, keepdims=True)


MUL_SQ_ACC = _register_custom_op(
    "MUL_SQ_ACC_ANT",
    Spec(body=sq(Src0) * Src1, accum=_op_add, reference=_mul_sq_acc_ref))


def _add1_mul_acc_ref(in0, in1, s0, s1, imm2):
    b = ((in0.astype(np.float32) + np.float32(1.0))
         * in1.astype(np.float32)).astype(np.float32)
    return b, b.reshape(b.shape[0], -1).sum(-1, keepdims=True)


from concourse.dve_spec import One as _One
ADD1_MUL_ACC = _register_custom_op(
    "ADD1_MUL_ACC_ANT",
    Spec(body=(Src0 + _One) * Src1, accum=_op_add,
         reference=_add1_mul_acc_ref))


# --------------------------------------------------------------------------
# host helpers
# --------------------------------------------------------------------------
def _bf16(x):
    y = np.ascontiguousarray(x, np.float32).view(np.uint32)
    r = ((y + np.uint32(0x8000) + ((y >> np.uint32(16)) & np.uint32(1)))
         & np.uint32(0xFFFF0000)).view(np.float32)
    return r.reshape(np.shape(x))


def _to_bf16(x):
    """Fast fp32 -> bf16 (round-to-nearest-even) via integer ops."""
    y = np.ascontiguousarray(x, np.float32).view(np.uint32)
    r = ((y + np.uint32(0x8000) + ((y >> np.uint32(16)) & np.uint32(1)))
         >> np.uint32(16)).astype(np.uint16)
    return r.view(ml_dtypes.bfloat16).reshape(np.shape(x))


def _split3(x):
    a1 = _bf16(x)
    r = (x - a1).astype(np.float32)
    a2 = _bf16(r)
    a3 = _bf16((r - a2).astype(np.float32))
    return a1, a2, a3


_SPLIT_PAIRS = [(0, 0), (0, 1), (1, 0), (0, 2), (2, 0), (1, 1)]


def _tile_list():
    tiles = []
    for rb in range(N_RB):
        for ct in range(rb * RB // CT, N_CT):
            tiles.append((rb, ct))
    return tiles


def _find_candidates(pos):
    p = pos.astype(np.float64)
    cell = 0.15
    keys = np.floor(p / cell).astype(np.int64)
    grid = defaultdict(list)
    for idx in range(p.shape[0]):
        grid[tuple(keys[idx])].append(idx)
    offs = list(itertools.product((-1, 0, 1), repeat=3))
    cand = set()
    for key, members in grid.items():
        for off in offs:
            other = grid.get((key[0] + off[0], key[1] + off[1], key[2] + off[2]))
            if not other:
                continue
            for i in members:
                pi = p[i]
                for j in other:
                    if j > i:
                        d = pi - p[j]
                        if d[0] * d[0] + d[1] * d[1] + d[2] * d[2] < CAND_D2:
                            cand.add((i, j))
    return sorted(cand)


def _ref_d2_for_pairs(pos, pairs):
    """Bitwise replication of the reference's fp32 d2 for the given pairs."""
    if not pairs:
        return np.zeros(0, np.float32)
    sq32 = np.sum(pos * pos, axis=-1)
    rows = sorted({i for i, _ in pairs})
    ridx = {i: k for k, i in enumerate(rows)}
    dmat = (sq32[rows][:, None] + sq32[None, :]
            - np.float32(2.0) * (pos[rows] @ pos.T))
    return np.array([dmat[ridx[i], j] for i, j in pairs], np.float32)


def _pack_fields(fields, n_items):
    npart = n_items // 128
    out = np.empty((128, len(fields) * npart), np.float32)
    for f, arr in enumerate(fields):
        out[:, f * npart:(f + 1) * npart] = np.asarray(arr, np.float32).reshape(128, npart)
    return out


# --------------------------------------------------------------------------
# device program
# --------------------------------------------------------------------------
_PROGRAM_CACHE = {}


def _build_program(n_strips, vw, nb, na, nd):
    key = (n_strips, vw, nb, na, nd)
    if key in _PROGRAM_CACHE:
        return _PROGRAM_CACHE[key]

    nc = bacc.Bacc("TRN2", target_bir_lowering=False, debug=False,
                   num_devices=N_CORES)
    n_tiles = n_strips * TILES_PER_STRIP
    sig_d = nc.dram_tensor("sig", [n_strips, RB, STRIP_W], BF16, kind="ExternalInput")
    eps_d = nc.dram_tensor("eps", [n_strips, RB, STRIP_W], BF16, kind="ExternalInput")
    meta_d = nc.dram_tensor("meta", [KROWS, n_tiles * (CT + RB)], BF16,
                            kind="ExternalInput")
    sqi_d = nc.dram_tensor("sqi", [RB, n_tiles], F32, kind="ExternalInput")
    vdm_d = nc.dram_tensor("vdm", [128, vw], F32, kind="ExternalInput")
    vsig_d = nc.dram_tensor("vsig", [128, vw], F32, kind="ExternalInput")
    veps_d = nc.dram_tensor("veps", [128, vw], F32, kind="ExternalInput")
    bp_d = nc.dram_tensor("bpack", [128, 8 * nb], F32, kind="ExternalInput")
    ap_d = nc.dram_tensor("apack", [128, 11 * na], F32, kind="ExternalInput")
    dp_d = nc.dram_tensor("dpack", [128, 15 * nd], F32, kind="ExternalInput")
    out_d = nc.dram_tensor("out", [1, 8], F32, kind="ExternalOutput")

    tagn = [0]

    with tile.TileContext(nc) as tc:
        with (
            tc.tile_pool(name="const", bufs=1) as cp,
            tc.tile_pool(name="sigp", bufs=3) as sigp,
            tc.tile_pool(name="epsp", bufs=3) as epsp,
            tc.tile_pool(name="dmp", bufs=2) as dmp,
            tc.tile_pool(name="i2p", bufs=2) as i2p,
            tc.tile_pool(name="ttp", bufs=2) as ttp,
            tc.tile_pool(name="accp", bufs=3) as accp,
            tc.tile_pool(name="bw", bufs=1) as bw,
            tc.tile_pool(name="drp", bufs=1, space=bass.MemorySpace.DRAM) as drp,
            tc.tile_pool(name="psp", bufs=3, space=bass.MemorySpace.PSUM) as psp,
        ):
            def wtile(shape, pool=bw, dtype=F32):
                tagn[0] += 1
                return pool.tile(shape, dtype, tag=f"w{tagn[0]}",
                                 name=f"w{tagn[0]}")

            meta = cp.tile([KROWS, n_tiles * (CT + RB)], BF16)
            nc.sync.dma_start(meta[:], meta_d.ap())
            sqi = cp.tile([RB, n_tiles], F32)
            nc.sync.dma_start(sqi[:], sqi_d.ap())

            from concourse.dve_ops import RECIP_APPROX_FAST_CONSTS as _RC
            _rc0, _rc1 = _RC["s0"], _RC["s1"]
            naccw = max(1, n_strips)
            saccs = cp.tile([128, naccw], F32)
            nc.gpsimd.memset(saccs[:], 0.0)

            # ------------- LJ main loop (2 DVE passes / fused group) ---------
            groups = []
            s0_ = 0
            while s0_ < n_strips:
                groups.append(list(range(s0_, min(s0_ + FUSE, n_strips))))
                s0_ += FUSE
            for gi, grp in enumerate(groups):
                gw = len(grp) * STRIP_W
                sig_t = sigp.tile([RB, FUSE * STRIP_W], BF16, tag="sig")
                eps_t = epsp.tile([RB, FUSE * STRIP_W], BF16, tag="eps")
                dm_t = dmp.tile([RB, FUSE * STRIP_W], F32, tag="dm")
                for li, s in enumerate(grp):
                    off = li * STRIP_W
                    nc.sync.dma_start(sig_t[:, off:off + STRIP_W], sig_d.ap()[s])
                    nc.sync.dma_start(eps_t[:, off:off + STRIP_W], eps_d.ap()[s])
                    for h in range(2):
                        ps_t = psp.tile([128, 1024], F32, tag="ps")
                        for q in range(2):
                            tg = s * TILES_PER_STRIP + h * 2 + q
                            base = tg * (CT + RB)
                            nc.tensor.matmul(
                                ps_t[:, q * CT:(q + 1) * CT],
                                meta[:, base + CT: base + CT + RB],
                                meta[:, base: base + CT],
                                start=True, stop=True)
                            nc.scalar.activation(
                                dm_t[:, off + (h * 2 + q) * CT:off + (h * 2 + q + 1) * CT],
                                ps_t[:, q * CT:(q + 1) * CT],
                                AF.Abs, bias=sqi[:, tg:tg + 1], scale=1.0)
                u_t = i2p.tile([RB, FUSE * STRIP_W], F32, tag="i2")
                nc.vector._custom_dve(LJ_RECIP_MUL, out=u_t[:, 0:gw],
                                      in0=dm_t[:, 0:gw],
                                      in1=sig_t[:, 0:gw], s0=_rc0, s1=_rc1)
                nc.vector._custom_dve(LJ_TAIL, out=dm_t[:, 0:gw],
                                      in0=u_t[:, 0:gw],
                                      in1=eps_t[:, 0:gw], s0=-0.5, s1=2.0,
                                      imm2=-1.0, accum_out=saccs[:, gi:gi + 1])
            acc_prev = accp.tile([128, 1], F32, tag="acc")
            nc.vector.tensor_reduce(out=acc_prev[:], in_=saccs[:],
                                    axis=mybir.AxisListType.X, op=ALU.add)

            # ---------------- virtual near pairs ----------------
            vdm = cp.tile([128, vw], F32)
            nc.sync.dma_start(vdm[:], vdm_d.ap())
            vsig = cp.tile([128, vw], F32)
            nc.sync.dma_start(vsig[:], vsig_d.ap())
            veps = cp.tile([128, vw], F32)
            nc.sync.dma_start(veps[:], veps_d.ap())
            vi2 = wtile([128, vw])
            nc.vector.reciprocal_approx_fast(out=vi2[:], in_=vdm[:])
            vt = wtile([128, vw])
            nc.vector._custom_dve(LJ_T, out=vt[:], in0=vi2[:], in1=vsig[:], s0=-0.5)
            vscr = wtile([128, vw])
            acc_lj = accp.tile([128, 1], F32, tag="acc")
            nc.vector._custom_dve(LJ_ACC, out=vscr[:], in0=veps[:], in1=vt[:],
                                  s0=acc_prev[:], s1=4.0, imm2=-1.0,
                                  accum_out=acc_lj[:])

            # ---------------- bonded-term helpers ----------------
            def tt(op, a, b, shape):
                o = wtile(shape)
                nc.vector.tensor_tensor(out=o[:], in0=a, in1=b, op=op)
                return o[:]

            def ts(a, op0, s1, op1=None, s2=None, shape=None):
                o = wtile(shape)
                if op1 is None:
                    nc.vector.tensor_scalar(out=o[:], in0=a, scalar1=s1,
                                            scalar2=None, op0=op0)
                else:
                    nc.vector.tensor_scalar(out=o[:], in0=a, scalar1=s1,
                                            scalar2=s2, op0=op0, op1=op1)
                return o[:]

            def act(fn, a, shape, scale=1.0):
                o = wtile(shape)
                nc.scalar.activation(o[:], a, fn, scale=scale)
                return o[:]

            def recip(a, shape):
                o = wtile(shape)
                nc.vector.reciprocal_approx_fast(out=o[:], in_=a)
                return o[:]

            def dot3(a, b, shape):
                m = [tt(ALU.mult, a[k], b[k], shape) for k in range(3)]
                s12 = tt(ALU.add, m[0], m[1], shape)
                return tt(ALU.add, s12, m[2], shape)

            def cross(a, b, shape):
                def comp(p, q, r, s):
                    t1 = tt(ALU.mult, p, q, shape)
                    t2 = tt(ALU.mult, r, s, shape)
                    return tt(ALU.subtract, t1, t2, shape)
                return [comp(a[1], b[2], a[2], b[1]),
                        comp(a[2], b[0], a[0], b[2]),
                        comp(a[0], b[1], a[1], b[0])]

            # ---------------- bonds ----------------
            bsh = [128, nb]
            bp = cp.tile([128, 8 * nb], F32)
            nc.sync.dma_start(bp[:], bp_d.ap())
            bF = [bp[:, f * nb:(f + 1) * nb] for f in range(8)]
            bw3 = [128, 3 * nb]
            d1w = tt(ALU.subtract, bp[:, 0:3 * nb], bp[:, 3 * nb:6 * nb], bw3)
            d1sq = tt(ALU.mult, d1w, d1w, bw3)
            d2b = wtile(bsh)
            nc.vector.tensor_reduce(
                out=d2b[:], in_=d1sq.rearrange("p (c n) -> p n c", c=3),
                axis=mybir.AxisListType.X, op=ALU.add)
            d2b = d2b[:]
            bd = act(AF.Sqrt, d2b, bsh)
            db = tt(ALU.subtract, bd, bF[7], bsh)
            eb_acc = wtile([128, 1])
            ebscr = wtile(bsh)
            nc.vector._custom_dve(MUL_SQ_ACC, out=ebscr[:], in0=db,
                                  in1=bF[6], accum_out=eb_acc[:])

            # ---------------- angles ----------------
            ash = [128, na]
            apk = cp.tile([128, 11 * na], F32)
            nc.sync.dma_start(apk[:], ap_d.ap())
            aF = [apk[:, f * na:(f + 1) * na] for f in range(11)]
            aw3 = [128, 3 * na]

            def _sred(wide, n_):
                o = wtile([128, n_])
                nc.vector.tensor_reduce(
                    out=o[:], in_=wide.rearrange("p (c n) -> p n c", c=3),
                    axis=mybir.AxisListType.X, op=ALU.add)
                return o[:]

            v1w = tt(ALU.subtract, apk[:, 3 * na:6 * na], apk[:, 0:3 * na], aw3)
            v2w = tt(ALU.subtract, apk[:, 3 * na:6 * na], apk[:, 6 * na:9 * na], aw3)
            dota = _sred(tt(ALU.mult, v1w, v2w, aw3), na)
            n1sq = _sred(tt(ALU.mult, v1w, v1w, aw3), na)
            n2sq = _sred(tt(ALU.mult, v2w, v2w, aw3), na)
            den2 = tt(ALU.mult, n1sq, n2sq, ash)
            den = act(AF.Sqrt, den2, ash)
            rden = recip(den, ash)
            cosa = tt(ALU.mult, dota, rden, ash)
            c2 = tt(ALU.mult, cosa, cosa, ash)
            omc = ts(c2, ALU.mult, -1.0, ALU.add, 1.0, shape=ash)
            sroot = act(AF.Sqrt, omc, ash)
            rs = recip(sroot, ash)
            targ = tt(ALU.mult, cosa, rs, ash)
            at = act(AF.Arctan, targ, ash)
            ang = ts(at, ALU.mult, -1.0, ALU.add, PI / 2, shape=ash)
            da = tt(ALU.subtract, ang, aF[10], ash)
            ea_acc = wtile([128, 1])
            eascr = wtile(ash)
            nc.vector._custom_dve(MUL_SQ_ACC, out=eascr[:], in0=da,
                                  in1=aF[9], accum_out=ea_acc[:])

            # ---------------- dihedrals ----------------
            dsh = [128, nd]
            dpk = cp.tile([128, 15 * nd], F32)
            nc.sync.dma_start(dpk[:], dp_d.ap())
            dF = [dpk[:, f * nd:(f + 1) * nd] for f in range(15)]
            dw3 = [128, 3 * nd]
            dw9 = [128, 9 * nd]
            www = wtile(dw9)  # w1|w2|w3 in one wide tile
            nc.vector.tensor_tensor(out=www[:], in0=dpk[:, 3 * nd:12 * nd],
                                    in1=dpk[:, 0:9 * nd], op=ALU.subtract)
            w1 = [www[:, k * nd:(k + 1) * nd] for k in range(3)]
            w2 = [www[:, (3 + k) * nd:(4 + k) * nd] for k in range(3)]
            w3 = [www[:, (6 + k) * nd:(7 + k) * nd] for k in range(3)]

            def _sredd(wide, n_):
                o = wtile([128, n_])
                nc.vector.tensor_reduce(
                    out=o[:], in_=wide.rearrange("p (c n) -> p n c", c=3),
                    axis=mybir.AxisListType.X, op=ALU.add)
                return o[:]

            n1w = wtile(dw3)
            n2w = wtile(dw3)

            def cross_into(dst, a, b):
                def comp(k, p, q, r, s):
                    t1 = tt(ALU.mult, p, q, dsh)
                    t2 = tt(ALU.mult, r, s, dsh)
                    nc.vector.tensor_tensor(out=dst[:, k * nd:(k + 1) * nd],
                                            in0=t1, in1=t2, op=ALU.subtract)
                comp(0, a[1], b[2], a[2], b[1])
                comp(1, a[2], b[0], a[0], b[2])
                comp(2, a[0], b[1], a[1], b[0])

            cross_into(n1w, w1, w2)
            cross_into(n2w, w2, w3)
            cdn = _sredd(tt(ALU.mult, n1w[:], n2w[:], dw3), nd)
            # (n1 x n2).w2 == (w1.n2)*|w2|^2  (Lagrange triple product)
            det = _sredd(tt(ALU.mult, www[:, 0:3 * nd], n2w[:], dw3), nd)
            wsqw = tt(ALU.mult, www[:, 0:6 * nd], www[:, 0:6 * nd], [128, 6 * nd])
            w1sq = _sredd(wsqw[:, 0:3 * nd], nd)
            w2sq = _sredd(wsqw[:, 3 * nd:6 * nd], nd)
            n1sq_ = _sredd(tt(ALU.mult, n1w[:], n1w[:], dw3), nd)
            n2sq_ = _sredd(tt(ALU.mult, n2w[:], n2w[:], dw3), nd)
            cden2 = tt(ALU.mult, w1sq, w2sq, dsh)
            cden = act(AF.Sqrt, cden2, dsh)
            rcden = recip(cden, dsh)
            cosd = tt(ALU.mult, cdn, rcden, dsh)
            sd1 = tt(ALU.mult, w2sq, n1sq_, dsh)
            sden2 = tt(ALU.mult, sd1, n2sq_, dsh)
            sden = act(AF.Sqrt, sden2, dsh)
            rsden = recip(sden, dsh)
            sdn = tt(ALU.mult, det, w2sq, dsh)
            sind = tt(ALU.mult, sdn, rsden, dsh)
            rcosd = recip(cosd, dsh)
            qd = tt(ALU.mult, sind, rcosd, dsh)
            atq = act(AF.Arctan, qd, dsh)
            sgn = act(AF.Sign, sind, dsh)
            neg = ts(cosd, ALU.is_lt, 0.0, shape=dsh)
            corr0 = tt(ALU.mult, sgn, neg, dsh)
            corr = ts(corr0, ALU.mult, PI, shape=dsh)
            dih = tt(ALU.add, atq, corr, dsh)
            narg = tt(ALU.mult, dih, dF[14], dsh)
            arg = tt(ALU.subtract, narg, dF[13], dsh)
            wr1 = wtile(dsh)
            nc.vector.add_range_wrap(out=wr1[:], in_=arg, shift=PI / 2,
                                     bound=PI, period=2 * PI)
            wr2 = wtile(dsh)
            nc.vector.add_range_wrap(out=wr2[:], in_=wr1[:], shift=0.0,
                                     bound=PI, period=2 * PI)
            sn = act(AF.Sin, wr2[:], dsh)
            ed_acc = wtile([128, 1])
            edscr = wtile(dsh)
            nc.vector._custom_dve(ADD1_MUL_ACC, out=edscr[:], in0=sn,
                                  in1=dF[12], accum_out=ed_acc[:])

            # ---------------- reductions / output ----------------
            comb = cp.tile([128, 4], F32)
            nc.vector.tensor_copy(comb[:, 0:1], acc_lj[:])
            for col, r_ in enumerate([eb_acc, ea_acc, ed_acc]):
                nc.scalar.mul(comb[:, col + 1:col + 2], r_[:], 0.5)

            dscr = drp.tile([1, 512], F32)
            dview = dscr[:].rearrange("x (p c) -> (x p) c", p=128)
            nc.sync.dma_start(dview, comb[:])
            flat = cp.tile([1, 512], F32)
            nc.sync.dma_start(flat[:], dscr[:])
            fview = flat[:].rearrange("p (a b) -> p b a", b=4)  # [1, 4, 128]
            sums = cp.tile([1, 4], F32)
            nc.vector.tensor_reduce(out=sums[:], in_=fview,
                                    axis=mybir.AxisListType.X, op=ALU.add)
            total = cp.tile([1, 1], F32)
            nc.vector.tensor_reduce(out=total[:], in_=sums[:],
                                    axis=mybir.AxisListType.X, op=ALU.add)
            outt = cp.tile([1, 8], F32)
            nc.gpsimd.memset(outt[:], 0.0)
            nc.vector.tensor_copy(outt[:, 0:1], total[:])
            nc.vector.tensor_copy(outt[:, 1:5], sums[:])
            nc.sync.dma_start(out_d.ap(), outt[:])

    nc.compile()
    _PROGRAM_CACHE[key] = nc
    return nc


# --------------------------------------------------------------------------
# host packing + dispatch
# --------------------------------------------------------------------------
def _prepare_core_inputs(inputs):
    pos = np.ascontiguousarray(inputs["positions"], np.float32)
    sigma = inputs["sigma"]
    eps = inputs["epsilon"]

    tiles = _tile_list()
    n_per_core = len(tiles) // N_CORES
    n_strips = n_per_core // TILES_PER_STRIP
    core_tiles = [tiles[c * n_per_core:(c + 1) * n_per_core]
                  for c in range(N_CORES)]
    tile_owner = {}
    for c in range(N_CORES):
        for k, t in enumerate(core_tiles[c]):
            tile_owner[t] = (c, k)

    cand = _find_candidates(pos)
    cand_d2 = _ref_d2_for_pairs(pos, cand)
    vc = [[] for _ in range(N_CORES)]
    for k, pr in enumerate(cand):
        vc[k % N_CORES].append((pr, cand_d2[k]))
    vmax = max((len(v) for v in vc), default=0)
    vw = max(1, -(-max(vmax, 1) // 128))

    a1, a2, a3 = _split3(pos)
    A = [a1, a2, a3]
    sq32 = np.sum(pos * pos, axis=-1)
    sqh = _bf16(sq32)
    sql = _bf16((sq32 - sqh).astype(np.float32))
    sql2 = _bf16((sq32 - sqh - sql).astype(np.float32))

    bidx = inputs["bond_idx"]; kb = inputs["k_bond"]; r0 = inputs["r0"]
    aidx = inputs["angle_idx"]; ka = inputs["k_angle"]; th0 = inputs["theta0"]
    didx = inputs["dihedral_idx"]; kd = inputs["k_dihedral"]
    ph = inputs["default_phase"]; nm = inputs["n_mult"]
    nb = len(kb) // N_CORES // 128
    na = len(ka) // N_CORES // 128
    nd = len(kd) // N_CORES // 128

    in_maps = []
    for c in range(N_CORES):
        sig_pack = np.empty((n_strips, RB, STRIP_W), ml_dtypes.bfloat16)
        eps_pack = np.empty((n_strips, RB, STRIP_W), ml_dtypes.bfloat16)
        meta = np.zeros((KROWS, n_per_core * (CT + RB)), np.float32)
        sqi = np.empty((RB, n_per_core), np.float32)
        for k, (rb, ct) in enumerate(core_tiles[c]):
            s, slot = divmod(k, TILES_PER_STRIP)
            rs, cs = rb * RB, ct * CT
            sig_pack[s, :, slot * CT:(slot + 1) * CT] = _to_bf16(sigma[rs:rs + RB, cs:cs + CT])
            eps_pack[s, :, slot * CT:(slot + 1) * CT] = _to_bf16(eps[rs:rs + RB, cs:cs + CT])
            base = k * (CT + RB)
            for pi_, (u, v) in enumerate(_SPLIT_PAIRS):
                for ax in range(3):
                    r = pi_ * 3 + ax
                    meta[r, base: base + CT] = A[v][cs:cs + CT, ax]
                    meta[r, base + CT: base + CT + RB] = \
                        A[u][rs:rs + RB, ax] * np.float32(-2.0)
            meta[18, base: base + CT] = sqh[cs:cs + CT]
            meta[19, base: base + CT] = sql[cs:cs + CT]
            meta[20, base: base + CT] = sql2[cs:cs + CT]
            meta[18:21, base + CT: base + CT + RB] = 1.0
            diag = (ct == rb * RB // CT)
            sqi[:, k] = sq32[rs:rs + RB] + (np.float32(DIAG_EPS) if diag else np.float32(0.0))

        meta_bf = np.ascontiguousarray(_bf16(meta).astype(ml_dtypes.bfloat16))

        vdm = np.ones((128, vw), np.float32)
        vsig = np.zeros((128, vw), np.float32)
        veps = np.zeros((128, vw), np.float32)
        for k, ((i, j), d2v) in enumerate(vc[c]):
            p_, q_ = k % 128, k // 128
            dist = np.float32(np.sqrt(np.float32(max(d2v, np.float32(0.0))))) + np.float32(1e-9)
            vdm[p_, q_] = np.float32(dist * dist)
            vsig[p_, q_] = sigma[i, j]
            veps[p_, q_] = eps[i, j]

        def seg(arr, n_each):
            return np.ascontiguousarray(arr[c * n_each:(c + 1) * n_each])

        bs = seg(bidx, nb * 128)
        g1, g2 = pos[bs[:, 0]], pos[bs[:, 1]]
        bpack = _pack_fields(
            [g1[:, 0], g1[:, 1], g1[:, 2], g2[:, 0], g2[:, 1], g2[:, 2],
             seg(kb, nb * 128), seg(r0, nb * 128)], nb * 128)
        asx = seg(aidx, na * 128)
        g1, g2, g3 = pos[asx[:, 0]], pos[asx[:, 1]], pos[asx[:, 2]]
        apack = _pack_fields(
            [g1[:, 0], g1[:, 1], g1[:, 2], g2[:, 0], g2[:, 1], g2[:, 2],
             g3[:, 0], g3[:, 1], g3[:, 2],
             seg(ka, na * 128), seg(th0, na * 128)], na * 128)
        dsx = seg(didx, nd * 128)
        g1, g2, g3, g4 = (pos[dsx[:, 0]], pos[dsx[:, 1]],
                          pos[dsx[:, 2]], pos[dsx[:, 3]])
        dpack = _pack_fields(
            [g1[:, 0], g1[:, 1], g1[:, 2], g2[:, 0], g2[:, 1], g2[:, 2],
             g3[:, 0], g3[:, 1], g3[:, 2], g4[:, 0], g4[:, 1], g4[:, 2],
             seg(kd, nd * 128), seg(ph, nd * 128), seg(nm, nd * 128)],
            nd * 128)

        in_maps.append({
            "sig": sig_pack, "eps": eps_pack, "meta": meta_bf, "sqi": sqi,
            "vdm": vdm, "vsig": vsig, "veps": veps,
            "bpack": bpack, "apack": apack, "dpack": dpack,
        })

    for k, (i, j) in enumerate(cand):
        rb, ct = i // RB, j // CT
        c, kt = tile_owner[(rb, ct)]
        s, slot = divmod(kt, TILES_PER_STRIP)
        col = slot * CT + (j - ct * CT)
        in_maps[c]["sig"][s, i - rb * RB, col] = 0.0
        in_maps[c]["eps"][s, i - rb * RB, col] = 0.0

    return in_maps, (n_strips, vw, nb, na, nd)


def kernel(**inputs):
    pos = np.asarray(inputs["positions"])
    sg = np.asarray(inputs["sigma"])
    ep = np.asarray(inputs["epsilon"])
    ok = (pos.shape == (N_ATOMS, 3) and sg.shape == (N_ATOMS, N_ATOMS)
          and ep.shape == (N_ATOMS, N_ATOMS)
          and len(inputs["k_bond"]) % (N_CORES * 128) == 0
          and len(inputs["k_angle"]) % (N_CORES * 128) == 0
          and len(inputs["k_dihedral"]) % (N_CORES * 128) == 0)
    if ok:
        idx = np.arange(0, N_ATOMS, 37)
        ii, jj = np.meshgrid(idx, idx, indexing="ij")
        low = ii > jj
        if sg[ii[low], jj[low]].any() or ep[ii[low], jj[low]].any():
            ok = False
    if not ok:
        return _host_fallback(inputs)

    try:
        in_maps, geom = _prepare_core_inputs(inputs)
        nc = _build_program(*geom)
        res = None
        for attempt in range(3):
            try:
                res = run_bass_kernel_spmd(nc, in_maps,
                                           core_ids=list(range(N_CORES)))
                break
            except Exception:
                if attempt == 2:
                    raise
                import time as _time
                _time.sleep(3.0)
    except Exception:
        # no devices / toolchain failure: fall back to the (slow) host path
        return _host_fallback(inputs)
    partials = np.stack([r["out"][0] for r in res.results])
    LAST_DEBUG["partials"] = partials
    total = np.float64(partials[:, 0]).sum()
    return np.float32(total)


def _host_fallback(inputs):
    """Numpy replication of the fp32 reference (safety net, not fast)."""
    pos = np.asarray(inputs["positions"], np.float32)
    sigma = np.asarray(inputs["sigma"], np.float32)
    eps = np.asarray(inputs["epsilon"], np.float32)
    n = pos.shape[0]
    sq32 = np.sum(pos * pos, axis=-1)
    lj = 0.0
    chunk = 512
    for s0 in range(0, n, chunk):
        s1 = min(s0 + chunk, n)
        d2 = (sq32[s0:s1, None] + sq32[None, :]
              - np.float32(2.0) * (pos[s0:s1] @ pos.T))
        dist = (np.sqrt(np.maximum(d2, 0)) + np.float32(1e-9)).astype(np.float64)
        r6 = (sigma[s0:s1].astype(np.float64) / dist) ** 6
        lj += float((4.0 * eps[s0:s1].astype(np.float64) * (r6 * r6 - r6)).sum())
    bi, bj = inputs["bond_idx"][:, 0], inputs["bond_idx"][:, 1]
    d2b = (sq32[bi] + sq32[bj]
           - np.float32(2.0) * np.sum(pos[bi] * pos[bj], -1, dtype=np.float32))
    bd = np.sqrt(np.maximum(d2b, 0)).astype(np.float64) + 1e-9
    bond_e = float(np.sum(0.5 * inputs["k_bond"] * (bd - inputs["r0"]) ** 2))
    p64 = pos.astype(np.float64)
    ai = inputs["angle_idx"]
    p1, p2, p3 = p64[ai[:, 0]], p64[ai[:, 1]], p64[ai[:, 2]]
    v1, v2 = p2 - p1, p2 - p3
    cos_a = np.sum(v1 * v2, -1) / (np.linalg.norm(v1, axis=1)
                                   * np.linalg.norm(v2, axis=1))
    angle_e = float(np.sum(0.5 * inputs["k_angle"]
                           * (np.arccos(np.clip(cos_a, -1, 1))
                              - inputs["theta0"]) ** 2))
    di = inputs["dihedral_idx"]
    q1, q2, q3, q4 = p64[di[:, 0]], p64[di[:, 1]], p64[di[:, 2]], p64[di[:, 3]]
    w1, w2, w3 = q2 - q1, q3 - q2, q4 - q3
    cn1, cn2 = np.cross(w1, w2), np.cross(w2, w3)
    cos_d = np.sum(cn1 * cn2, -1) / (np.linalg.norm(w1, axis=1)
                                     * np.linalg.norm(w2, axis=1))
    sin_d = np.sum(np.cross(cn1, cn2) * w2, -1) / (
        np.linalg.norm(w2, axis=1) * np.linalg.norm(cn1, axis=1)
        * np.linalg.norm(cn2, axis=1))
    dih = np.arctan2(sin_d, cos_d)
    dihedral_e = float(np.sum(0.5 * inputs["k_dihedral"]
                              * (1.0 + np.cos(inputs["n_mult"] * dih
                                              - inputs["default_phase"]))))
    return np.float32(lj + bond_e + angle_e + dihedral_e)


# revision 16
# speedup vs baseline: 1.0526x; 1.0511x over previous
"""Trainium2 Bass kernel for nn_EnergyModel (bonded + Lennard-Jones energy).

Distribution: the [N,N] LJ pairwise term is upper-triangular; its 544
128x512 tiles are packed per-core (68 tiles = 17 dense [128,2048] strips)
so each of the 8 NeuronCores streams ~36MB of perfectly-sequential DMA
(half of the naive 512MB total). Positions and bonded lists are tiny and
split 1/8 per core. Each core emits one partial energy; host sums 8.

Device pipeline per strip:
  PE    : d2 = -2*pos_i.pos_j + |pos_j|^2 via a 21-row bf16 triple-split
          matmul (exact products + fp32 PSUM accumulate -- native fp32
          matmul is fp32r, far too coarse for the |pi-pj|^2 cancellation)
  ACT   : dm = Abs(psum + |pos_i|^2 [+ 1e-3 on diagonal tiles])
  DVE   : i2 = reciprocal_approx_fast(dm)                  (~51 ULP)
          t  = (u^3 - 1/2)^2, u = i2*sigma^2               (custom op)
          acc += eps*(4t - 1)                              (custom op,
                     chained per-partition running sum)
using 4*eps*(r12 - r6) = eps*(4t - 1), t = ((s/d)^6 - 1/2)^2.

Near pairs (exact d2 < 0.02): the reference's fp32 rounding of
|pi|^2+|pj|^2-2pi.pj is quantized at ~1.9e-6 and amplified x6 by r12 (the
single nearest pair carries ~96% of the total energy). The host finds
them with an O(N) spatial hash, replicates the reference's fp32 d2
bitwise (numpy sgemm == jax CPU, verified), zeroes those sigma/eps in the
packed tiles, and routes them through the same device chain as a small
"virtual pairs" tile with host-supplied dm.
"""

import itertools
import sys
from collections import defaultdict
from operator import add as _op_add

import numpy as np

sys.path.insert(0, "/opt/trn_rl_repo")

import ml_dtypes  # noqa: E402
from concourse import bass, bacc, mybir, tile  # noqa: E402
from concourse.bass_utils import run_bass_kernel_spmd  # noqa: E402
from concourse import dve_ops  # noqa: E402
from concourse.dve_ops import DveOp, OPS  # noqa: E402
from concourse.dve_spec import (  # noqa: E402
    Spec, Src0, Src1, C0, C1, C2, sq, lower, _has_src1,
)
from concourse.dve_uop import DveOpSpec  # noqa: E402

N_ATOMS = 8192
N_CORES = 8
RB = 128
CT = 512
N_RB = N_ATOMS // RB
N_CT = N_ATOMS // CT
TILES_PER_STRIP = 4            # packing granularity (dram layout unit)
STRIP_W = TILES_PER_STRIP * CT  # dram strips stay [128, 2048]
FUSE = 1                        # DVE processes FUSE dram strips per pass
CAND_D2 = 0.02
KROWS = 21
DIAG_EPS = 1e-3   # keeps diagonal-tile dm safely > 0 for the reciprocal

F32 = mybir.dt.float32
BF16 = mybir.dt.bfloat16
AF = mybir.ActivationFunctionType
ALU = mybir.AluOpType
PI = float(np.pi)

LAST_DEBUG = {}


# --------------------------------------------------------------------------
# custom DVE ops
# --------------------------------------------------------------------------
def _register_custom_op(name, spec, subdim=False):
    for o in OPS:
        if o.name == name:
            return o
    row = dve_ops._CUSTOM_DVE_ROW_BASE + len(OPS)
    dve_ops._SUB_OPCODE_FOR_NAME[name] = row
    shas = {}
    for ver in ("v3", "v4"):
        s = DveOpSpec(name=name, opcode=row, uops=lower(spec, ver=ver),
                      rd1_en=_has_src1(spec))
        shas[ver] = s.sha(ver)
    op = DveOp(name, spec, subdim=subdim, uops_sha=shas)
    OPS.append(op)
    dve_ops.CUSTOM_DVE_SPECS[name] = spec
    return op


def _lj_t_ref(in0, in1, s0, s1, imm2):
    u = (in0.astype(np.float32) * (in1.astype(np.float32) ** 2)).astype(np.float32)
    u3 = (u * u * u).astype(np.float32)
    return ((u3 + s0) ** 2).astype(np.float32)


_u = Src0 * sq(Src1)
_u3 = sq(_u) * _u
LJ_T = _register_custom_op("LJ_T_ANT", Spec(body=sq(_u3 + C0), reference=_lj_t_ref))


def _lj_acc_ref(in0, in1, s0, s1, imm2):
    b = (in0.astype(np.float32)
         * (in1.astype(np.float32) * s1 + imm2)).astype(np.float32)
    return b, s0 + b.reshape(b.shape[0], -1).sum(-1, keepdims=True)


LJ_ACC = _register_custom_op(
    "LJ_ACC_ANT",
    Spec(body=Src0 * (Src1 * C1 + C2), accum=_op_add, accum_init=C0,
         reference=_lj_acc_ref))


def _lj_recip_mul_ref(in0, in1, s0, s1, imm2):
    not_x = (~np.ascontiguousarray(in0, np.float32).view(np.int32)).view(np.float32)
    y0 = (not_x * np.float32(s0)).astype(np.float32)
    y1 = (y0 * (np.float32(s1) - in0 * y0)).astype(np.float32)
    return ((in1.astype(np.float32) * in1) * y1).astype(np.float32)


from concourse.dve_spec import Bin, AluOp as _AluOp
_ny0 = Bin(_AluOp.BITWISE_NOT, Src0, Src0) * C0
_ny1 = _ny0 * (C1 - Src0 * _ny0)
LJ_RECIP_MUL = _register_custom_op(
    "LJ_RECIP_MUL_ANT",
    Spec(body=sq(Src1) * _ny1, reference=_lj_recip_mul_ref))


def _lj_tail_ref(in0, in1, s0, s1, imm2):
    u3 = (in0.astype(np.float32) ** 2 * in0).astype(np.float32)
    w2 = ((u3 + s0) * s1).astype(np.float32)
    b = ((w2 * w2 + imm2) * in1.astype(np.float32)).astype(np.float32)
    return b, b.reshape(b.shape[0], -1).sum(-1, keepdims=True)


_tu3 = sq(Src0) * Src0
_tw2 = (_tu3 + C0) * C1
LJ_TAIL = _register_custom_op(
    "LJ_TAIL_ANT",
    Spec(body=(sq(_tw2) + C2) * Src1, accum=_op_add,
         reference=_lj_tail_ref))


def _mul_sq_acc_ref(in0, in1, s0, s1, imm2):
    b = ((in0.astype(np.float32) ** 2) * in1.astype(np.float32)).astype(np.float32)
    return b, b.reshape(b.shape[0], -1).sum(-1, keepdims=True)


MUL_SQ_ACC = _register_custom_op(
    "MUL_SQ_ACC_ANT",
    Spec(body=sq(Src0) * Src1, accum=_op_add, reference=_mul_sq_acc_ref))


def _add1_mul_acc_ref(in0, in1, s0, s1, imm2):
    b = ((in0.astype(np.float32) + np.float32(1.0))
         * in1.astype(np.float32)).astype(np.float32)
    return b, b.reshape(b.shape[0], -1).sum(-1, keepdims=True)


from concourse.dve_spec import One as _One
ADD1_MUL_ACC = _register_custom_op(
    "ADD1_MUL_ACC_ANT",
    Spec(body=(Src0 + _One) * Src1, accum=_op_add,
         reference=_add1_mul_acc_ref))


# --------------------------------------------------------------------------
# host helpers
# --------------------------------------------------------------------------
def _bf16(x):
    y = np.ascontiguousarray(x, np.float32).view(np.uint32)
    r = ((y + np.uint32(0x8000) + ((y >> np.uint32(16)) & np.uint32(1)))
         & np.uint32(0xFFFF0000)).view(np.float32)
    return r.reshape(np.shape(x))


def _to_bf16(x):
    """Fast fp32 -> bf16 (round-to-nearest-even) via integer ops."""
    y = np.ascontiguousarray(x, np.float32).view(np.uint32)
    r = ((y + np.uint32(0x8000) + ((y >> np.uint32(16)) & np.uint32(1)))
         >> np.uint32(16)).astype(np.uint16)
    return r.view(ml_dtypes.bfloat16).reshape(np.shape(x))


def _split3(x):
    a1 = _bf16(x)
    r = (x - a1).astype(np.float32)
    a2 = _bf16(r)
    a3 = _bf16((r - a2).astype(np.float32))
    return a1, a2, a3


_SPLIT_PAIRS = [(0, 0), (0, 1), (1, 0), (0, 2), (2, 0), (1, 1)]


def _tile_list():
    tiles = []
    for rb in range(N_RB):
        for ct in range(rb * RB // CT, N_CT):
            tiles.append((rb, ct))
    return tiles


def _find_candidates(pos):
    p = pos.astype(np.float64)
    cell = 0.15
    keys = np.floor(p / cell).astype(np.int64)
    grid = defaultdict(list)
    for idx in range(p.shape[0]):
        grid[tuple(keys[idx])].append(idx)
    offs = list(itertools.product((-1, 0, 1), repeat=3))
    cand = set()
    for key, members in grid.items():
        for off in offs:
            other = grid.get((key[0] + off[0], key[1] + off[1], key[2] + off[2]))
            if not other:
                continue
            for i in members:
                pi = p[i]
                for j in other:
                    if j > i:
                        d = pi - p[j]
                        if d[0] * d[0] + d[1] * d[1] + d[2] * d[2] < CAND_D2:
                            cand.add((i, j))
    return sorted(cand)


def _ref_d2_for_pairs(pos, pairs):
    """Bitwise replication of the reference's fp32 d2 for the given pairs."""
    if not pairs:
        return np.zeros(0, np.float32)
    sq32 = np.sum(pos * pos, axis=-1)
    rows = sorted({i for i, _ in pairs})
    ridx = {i: k for k, i in enumerate(rows)}
    dmat = (sq32[rows][:, None] + sq32[None, :]
            - np.float32(2.0) * (pos[rows] @ pos.T))
    return np.array([dmat[ridx[i], j] for i, j in pairs], np.float32)


def _pack_fields(fields, n_items):
    npart = n_items // 128
    out = np.empty((128, len(fields) * npart), np.float32)
    for f, arr in enumerate(fields):
        out[:, f * npart:(f + 1) * npart] = np.asarray(arr, np.float32).reshape(128, npart)
    return out


# --------------------------------------------------------------------------
# device program
# --------------------------------------------------------------------------
_PROGRAM_CACHE = {}


def _build_program(n_strips, vw, nb, na, nd):
    key = (n_strips, vw, nb, na, nd)
    if key in _PROGRAM_CACHE:
        return _PROGRAM_CACHE[key]

    nc = bacc.Bacc("TRN2", target_bir_lowering=False, debug=False,
                   num_devices=N_CORES)
    n_tiles = n_strips * TILES_PER_STRIP
    sig_d = nc.dram_tensor("sig", [n_strips, RB, STRIP_W], BF16, kind="ExternalInput")
    eps_d = nc.dram_tensor("eps", [n_strips, RB, STRIP_W], BF16, kind="ExternalInput")
    meta_d = nc.dram_tensor("meta", [KROWS, n_tiles * (CT + RB)], BF16,
                            kind="ExternalInput")
    sqi_d = nc.dram_tensor("sqi", [RB, n_tiles], F32, kind="ExternalInput")
    vdm_d = nc.dram_tensor("vdm", [128, vw], F32, kind="ExternalInput")
    vsig_d = nc.dram_tensor("vsig", [128, vw], F32, kind="ExternalInput")
    veps_d = nc.dram_tensor("veps", [128, vw], F32, kind="ExternalInput")
    bp_d = nc.dram_tensor("bpack", [128, 8 * nb], F32, kind="ExternalInput")
    ap_d = nc.dram_tensor("apack", [128, 11 * na], F32, kind="ExternalInput")
    dp_d = nc.dram_tensor("dpack", [128, 15 * nd], F32, kind="ExternalInput")
    out_d = nc.dram_tensor("out", [128, 4], F32, kind="ExternalOutput")

    tagn = [0]

    with tile.TileContext(nc) as tc:
        with (
            tc.tile_pool(name="const", bufs=1) as cp,
            tc.tile_pool(name="sigp", bufs=3) as sigp,
            tc.tile_pool(name="epsp", bufs=3) as epsp,
            tc.tile_pool(name="dmp", bufs=2) as dmp,
            tc.tile_pool(name="i2p", bufs=2) as i2p,
            tc.tile_pool(name="ttp", bufs=2) as ttp,
            tc.tile_pool(name="accp", bufs=3) as accp,
            tc.tile_pool(name="bw", bufs=1) as bw,
            tc.tile_pool(name="drp", bufs=1, space=bass.MemorySpace.DRAM) as drp,
            tc.tile_pool(name="psp", bufs=3, space=bass.MemorySpace.PSUM) as psp,
        ):
            def wtile(shape, pool=bw, dtype=F32):
                tagn[0] += 1
                return pool.tile(shape, dtype, tag=f"w{tagn[0]}",
                                 name=f"w{tagn[0]}")

            meta = cp.tile([KROWS, n_tiles * (CT + RB)], BF16)
            nc.sync.dma_start(meta[:], meta_d.ap())
            sqi = cp.tile([RB, n_tiles], F32)
            nc.sync.dma_start(sqi[:], sqi_d.ap())

            from concourse.dve_ops import RECIP_APPROX_FAST_CONSTS as _RC
            _rc0, _rc1 = _RC["s0"], _RC["s1"]
            naccw = max(1, n_strips)
            saccs = cp.tile([128, naccw], F32)
            nc.gpsimd.memset(saccs[:], 0.0)

            # ------------- LJ main loop (2 DVE passes / fused group) ---------
            groups = []
            s0_ = 0
            while s0_ < n_strips:
                groups.append(list(range(s0_, min(s0_ + FUSE, n_strips))))
                s0_ += FUSE
            for gi, grp in enumerate(groups):
                gw = len(grp) * STRIP_W
                sig_t = sigp.tile([RB, FUSE * STRIP_W], BF16, tag="sig")
                eps_t = epsp.tile([RB, FUSE * STRIP_W], BF16, tag="eps")
                dm_t = dmp.tile([RB, FUSE * STRIP_W], F32, tag="dm")
                for li, s in enumerate(grp):
                    off = li * STRIP_W
                    nc.sync.dma_start(sig_t[:, off:off + STRIP_W], sig_d.ap()[s])
                    nc.sync.dma_start(eps_t[:, off:off + STRIP_W], eps_d.ap()[s])
                    for h in range(2):
                        ps_t = psp.tile([128, 1024], F32, tag="ps")
                        for q in range(2):
                            tg = s * TILES_PER_STRIP + h * 2 + q
                            base = tg * (CT + RB)
                            nc.tensor.matmul(
                                ps_t[:, q * CT:(q + 1) * CT],
                                meta[:, base + CT: base + CT + RB],
                                meta[:, base: base + CT],
                                start=True, stop=True)
                            nc.scalar.activation(
                                dm_t[:, off + (h * 2 + q) * CT:off + (h * 2 + q + 1) * CT],
                                ps_t[:, q * CT:(q + 1) * CT],
                                AF.Abs, bias=sqi[:, tg:tg + 1], scale=1.0)
                u_t = i2p.tile([RB, FUSE * STRIP_W], F32, tag="i2")
                nc.vector._custom_dve(LJ_RECIP_MUL, out=u_t[:, 0:gw],
                                      in0=dm_t[:, 0:gw],
                                      in1=sig_t[:, 0:gw], s0=_rc0, s1=_rc1)
                nc.vector._custom_dve(LJ_TAIL, out=dm_t[:, 0:gw],
                                      in0=u_t[:, 0:gw],
                                      in1=eps_t[:, 0:gw], s0=-0.5, s1=2.0,
                                      imm2=-1.0, accum_out=saccs[:, gi:gi + 1])
            acc_prev = accp.tile([128, 1], F32, tag="acc")
            nc.vector.tensor_reduce(out=acc_prev[:], in_=saccs[:],
                                    axis=mybir.AxisListType.X, op=ALU.add)

            # ---------------- virtual near pairs ----------------
            vdm = cp.tile([128, vw], F32)
            nc.sync.dma_start(vdm[:], vdm_d.ap())
            vsig = cp.tile([128, vw], F32)
            nc.sync.dma_start(vsig[:], vsig_d.ap())
            veps = cp.tile([128, vw], F32)
            nc.sync.dma_start(veps[:], veps_d.ap())
            vi2 = wtile([128, vw])
            nc.vector.reciprocal_approx_fast(out=vi2[:], in_=vdm[:])
            vt = wtile([128, vw])
            nc.vector._custom_dve(LJ_T, out=vt[:], in0=vi2[:], in1=vsig[:], s0=-0.5)
            vscr = wtile([128, vw])
            acc_lj = accp.tile([128, 1], F32, tag="acc")
            nc.vector._custom_dve(LJ_ACC, out=vscr[:], in0=veps[:], in1=vt[:],
                                  s0=acc_prev[:], s1=4.0, imm2=-1.0,
                                  accum_out=acc_lj[:])

            # ---------------- bonded-term helpers ----------------
            def tt(op, a, b, shape):
                o = wtile(shape)
                nc.vector.tensor_tensor(out=o[:], in0=a, in1=b, op=op)
                return o[:]

            def ts(a, op0, s1, op1=None, s2=None, shape=None):
                o = wtile(shape)
                if op1 is None:
                    nc.vector.tensor_scalar(out=o[:], in0=a, scalar1=s1,
                                            scalar2=None, op0=op0)
                else:
                    nc.vector.tensor_scalar(out=o[:], in0=a, scalar1=s1,
                                            scalar2=s2, op0=op0, op1=op1)
                return o[:]

            def act(fn, a, shape, scale=1.0):
                o = wtile(shape)
                nc.scalar.activation(o[:], a, fn, scale=scale)
                return o[:]

            def recip(a, shape):
                o = wtile(shape)
                nc.vector.reciprocal_approx_fast(out=o[:], in_=a)
                return o[:]

            def dot3(a, b, shape):
                m = [tt(ALU.mult, a[k], b[k], shape) for k in range(3)]
                s12 = tt(ALU.add, m[0], m[1], shape)
                return tt(ALU.add, s12, m[2], shape)

            def cross(a, b, shape):
                def comp(p, q, r, s):
                    t1 = tt(ALU.mult, p, q, shape)
                    t2 = tt(ALU.mult, r, s, shape)
                    return tt(ALU.subtract, t1, t2, shape)
                return [comp(a[1], b[2], a[2], b[1]),
                        comp(a[2], b[0], a[0], b[2]),
                        comp(a[0], b[1], a[1], b[0])]

            # ---------------- bonds ----------------
            bsh = [128, nb]
            bp = cp.tile([128, 8 * nb], F32)
            nc.sync.dma_start(bp[:], bp_d.ap())
            bF = [bp[:, f * nb:(f + 1) * nb] for f in range(8)]
            bw3 = [128, 3 * nb]
            d1w = tt(ALU.subtract, bp[:, 0:3 * nb], bp[:, 3 * nb:6 * nb], bw3)
            d1sq = tt(ALU.mult, d1w, d1w, bw3)
            d2b = wtile(bsh)
            nc.vector.tensor_reduce(
                out=d2b[:], in_=d1sq.rearrange("p (c n) -> p n c", c=3),
                axis=mybir.AxisListType.X, op=ALU.add)
            d2b = d2b[:]
            bd = act(AF.Sqrt, d2b, bsh)
            db = tt(ALU.subtract, bd, bF[7], bsh)
            eb_acc = wtile([128, 1])
            ebscr = wtile(bsh)
            nc.vector._custom_dve(MUL_SQ_ACC, out=ebscr[:], in0=db,
                                  in1=bF[6], accum_out=eb_acc[:])

            # ---------------- angles ----------------
            ash = [128, na]
            apk = cp.tile([128, 11 * na], F32)
            nc.sync.dma_start(apk[:], ap_d.ap())
            aF = [apk[:, f * na:(f + 1) * na] for f in range(11)]
            aw3 = [128, 3 * na]

            def _sred(wide, n_):
                o = wtile([128, n_])
                nc.vector.tensor_reduce(
                    out=o[:], in_=wide.rearrange("p (c n) -> p n c", c=3),
                    axis=mybir.AxisListType.X, op=ALU.add)
                return o[:]

            v1w = tt(ALU.subtract, apk[:, 3 * na:6 * na], apk[:, 0:3 * na], aw3)
            v2w = tt(ALU.subtract, apk[:, 3 * na:6 * na], apk[:, 6 * na:9 * na], aw3)
            dota = _sred(tt(ALU.mult, v1w, v2w, aw3), na)
            n1sq = _sred(tt(ALU.mult, v1w, v1w, aw3), na)
            n2sq = _sred(tt(ALU.mult, v2w, v2w, aw3), na)
            den2 = tt(ALU.mult, n1sq, n2sq, ash)
            den = act(AF.Sqrt, den2, ash)
            rden = recip(den, ash)
            cosa = tt(ALU.mult, dota, rden, ash)
            c2 = tt(ALU.mult, cosa, cosa, ash)
            omc = ts(c2, ALU.mult, -1.0, ALU.add, 1.0, shape=ash)
            sroot = act(AF.Sqrt, omc, ash)
            rs = recip(sroot, ash)
            targ = tt(ALU.mult, cosa, rs, ash)
            at = act(AF.Arctan, targ, ash)
            ang = ts(at, ALU.mult, -1.0, ALU.add, PI / 2, shape=ash)
            da = tt(ALU.subtract, ang, aF[10], ash)
            ea_acc = wtile([128, 1])
            eascr = wtile(ash)
            nc.vector._custom_dve(MUL_SQ_ACC, out=eascr[:], in0=da,
                                  in1=aF[9], accum_out=ea_acc[:])

            # ---------------- dihedrals ----------------
            dsh = [128, nd]
            dpk = cp.tile([128, 15 * nd], F32)
            nc.sync.dma_start(dpk[:], dp_d.ap())
            dF = [dpk[:, f * nd:(f + 1) * nd] for f in range(15)]
            dw3 = [128, 3 * nd]
            dw9 = [128, 9 * nd]
            www = wtile(dw9)  # w1|w2|w3 in one wide tile
            nc.vector.tensor_tensor(out=www[:], in0=dpk[:, 3 * nd:12 * nd],
                                    in1=dpk[:, 0:9 * nd], op=ALU.subtract)
            w1 = [www[:, k * nd:(k + 1) * nd] for k in range(3)]
            w2 = [www[:, (3 + k) * nd:(4 + k) * nd] for k in range(3)]
            w3 = [www[:, (6 + k) * nd:(7 + k) * nd] for k in range(3)]

            def _sredd(wide, n_):
                o = wtile([128, n_])
                nc.vector.tensor_reduce(
                    out=o[:], in_=wide.rearrange("p (c n) -> p n c", c=3),
                    axis=mybir.AxisListType.X, op=ALU.add)
                return o[:]

            n1w = wtile(dw3)
            n2w = wtile(dw3)

            def cross_into(dst, a, b):
                def comp(k, p, q, r, s):
                    t1 = tt(ALU.mult, p, q, dsh)
                    t2 = tt(ALU.mult, r, s, dsh)
                    nc.vector.tensor_tensor(out=dst[:, k * nd:(k + 1) * nd],
                                            in0=t1, in1=t2, op=ALU.subtract)
                comp(0, a[1], b[2], a[2], b[1])
                comp(1, a[2], b[0], a[0], b[2])
                comp(2, a[0], b[1], a[1], b[0])

            cross_into(n1w, w1, w2)
            cross_into(n2w, w2, w3)
            cdn = _sredd(tt(ALU.mult, n1w[:], n2w[:], dw3), nd)
            # (n1 x n2).w2 == (w1.n2)*|w2|^2  (Lagrange triple product)
            det = _sredd(tt(ALU.mult, www[:, 0:3 * nd], n2w[:], dw3), nd)
            wsqw = tt(ALU.mult, www[:, 0:6 * nd], www[:, 0:6 * nd], [128, 6 * nd])
            w1sq = _sredd(wsqw[:, 0:3 * nd], nd)
            w2sq = _sredd(wsqw[:, 3 * nd:6 * nd], nd)
            n1sq_ = _sredd(tt(ALU.mult, n1w[:], n1w[:], dw3), nd)
            n2sq_ = _sredd(tt(ALU.mult, n2w[:], n2w[:], dw3), nd)
            cden2 = tt(ALU.mult, w1sq, w2sq, dsh)
            cden = act(AF.Sqrt, cden2, dsh)
            rcden = recip(cden, dsh)
            cosd = tt(ALU.mult, cdn, rcden, dsh)
            sd1 = tt(ALU.mult, w2sq, n1sq_, dsh)
            sden2 = tt(ALU.mult, sd1, n2sq_, dsh)
            sden = act(AF.Sqrt, sden2, dsh)
            rsden = recip(sden, dsh)
            sdn = tt(ALU.mult, det, w2sq, dsh)
            sind = tt(ALU.mult, sdn, rsden, dsh)
            rcosd = recip(cosd, dsh)
            qd = tt(ALU.mult, sind, rcosd, dsh)
            atq = act(AF.Arctan, qd, dsh)
            sgn = act(AF.Sign, sind, dsh)
            neg = ts(cosd, ALU.is_lt, 0.0, shape=dsh)
            corr0 = tt(ALU.mult, sgn, neg, dsh)
            corr = ts(corr0, ALU.mult, PI, shape=dsh)
            dih = tt(ALU.add, atq, corr, dsh)
            narg = tt(ALU.mult, dih, dF[14], dsh)
            arg = tt(ALU.subtract, narg, dF[13], dsh)
            wr1 = wtile(dsh)
            nc.vector.add_range_wrap(out=wr1[:], in_=arg, shift=PI / 2,
                                     bound=PI, period=2 * PI)
            wr2 = wtile(dsh)
            nc.vector.add_range_wrap(out=wr2[:], in_=wr1[:], shift=0.0,
                                     bound=PI, period=2 * PI)
            sn = act(AF.Sin, wr2[:], dsh)
            ed_acc = wtile([128, 1])
            edscr = wtile(dsh)
            nc.vector._custom_dve(ADD1_MUL_ACC, out=edscr[:], in0=sn,
                                  in1=dF[12], accum_out=ed_acc[:])

            # ---------------- reductions / output ----------------
            # per-partition partials [128, 4]; final reduction happens on
            # the host together with the 8-core sum (removes the serial
            # partition-collapse tail from the device critical path)
            comb = cp.tile([128, 4], F32)
            nc.vector.tensor_copy(comb[:, 0:1], acc_lj[:])
            for col, r_ in enumerate([eb_acc, ea_acc, ed_acc]):
                nc.scalar.mul(comb[:, col + 1:col + 2], r_[:], 0.5)
            nc.sync.dma_start(out_d.ap(), comb[:])

    nc.compile()
    _PROGRAM_CACHE[key] = nc
    return nc


# --------------------------------------------------------------------------
# host packing + dispatch
# --------------------------------------------------------------------------
def _prepare_core_inputs(inputs):
    pos = np.ascontiguousarray(inputs["positions"], np.float32)
    sigma = inputs["sigma"]
    eps = inputs["epsilon"]

    tiles = _tile_list()
    n_per_core = len(tiles) // N_CORES
    n_strips = n_per_core // TILES_PER_STRIP
    core_tiles = [tiles[c * n_per_core:(c + 1) * n_per_core]
                  for c in range(N_CORES)]
    tile_owner = {}
    for c in range(N_CORES):
        for k, t in enumerate(core_tiles[c]):
            tile_owner[t] = (c, k)

    cand = _find_candidates(pos)
    cand_d2 = _ref_d2_for_pairs(pos, cand)
    vc = [[] for _ in range(N_CORES)]
    for k, pr in enumerate(cand):
        vc[k % N_CORES].append((pr, cand_d2[k]))
    vmax = max((len(v) for v in vc), default=0)
    vw = max(1, -(-max(vmax, 1) // 128))

    a1, a2, a3 = _split3(pos)
    A = [a1, a2, a3]
    sq32 = np.sum(pos * pos, axis=-1)
    sqh = _bf16(sq32)
    sql = _bf16((sq32 - sqh).astype(np.float32))
    sql2 = _bf16((sq32 - sqh - sql).astype(np.float32))

    bidx = inputs["bond_idx"]; kb = inputs["k_bond"]; r0 = inputs["r0"]
    aidx = inputs["angle_idx"]; ka = inputs["k_angle"]; th0 = inputs["theta0"]
    didx = inputs["dihedral_idx"]; kd = inputs["k_dihedral"]
    ph = inputs["default_phase"]; nm = inputs["n_mult"]
    nb = len(kb) // N_CORES // 128
    na = len(ka) // N_CORES // 128
    nd = len(kd) // N_CORES // 128

    in_maps = []
    for c in range(N_CORES):
        sig_pack = np.empty((n_strips, RB, STRIP_W), ml_dtypes.bfloat16)
        eps_pack = np.empty((n_strips, RB, STRIP_W), ml_dtypes.bfloat16)
        meta = np.zeros((KROWS, n_per_core * (CT + RB)), np.float32)
        sqi = np.empty((RB, n_per_core), np.float32)
        for k, (rb, ct) in enumerate(core_tiles[c]):
            s, slot = divmod(k, TILES_PER_STRIP)
            rs, cs = rb * RB, ct * CT
            sig_pack[s, :, slot * CT:(slot + 1) * CT] = _to_bf16(sigma[rs:rs + RB, cs:cs + CT])
            eps_pack[s, :, slot * CT:(slot + 1) * CT] = _to_bf16(eps[rs:rs + RB, cs:cs + CT])
            base = k * (CT + RB)
            for pi_, (u, v) in enumerate(_SPLIT_PAIRS):
                for ax in range(3):
                    r = pi_ * 3 + ax
                    meta[r, base: base + CT] = A[v][cs:cs + CT, ax]
                    meta[r, base + CT: base + CT + RB] = \
                        A[u][rs:rs + RB, ax] * np.float32(-2.0)
            meta[18, base: base + CT] = sqh[cs:cs + CT]
            meta[19, base: base + CT] = sql[cs:cs + CT]
            meta[20, base: base + CT] = sql2[cs:cs + CT]
            meta[18:21, base + CT: base + CT + RB] = 1.0
            diag = (ct == rb * RB // CT)
            sqi[:, k] = sq32[rs:rs + RB] + (np.float32(DIAG_EPS) if diag else np.float32(0.0))

        meta_bf = np.ascontiguousarray(_bf16(meta).astype(ml_dtypes.bfloat16))

        vdm = np.ones((128, vw), np.float32)
        vsig = np.zeros((128, vw), np.float32)
        veps = np.zeros((128, vw), np.float32)
        for k, ((i, j), d2v) in enumerate(vc[c]):
            p_, q_ = k % 128, k // 128
            dist = np.float32(np.sqrt(np.float32(max(d2v, np.float32(0.0))))) + np.float32(1e-9)
            vdm[p_, q_] = np.float32(dist * dist)
            vsig[p_, q_] = sigma[i, j]
            veps[p_, q_] = eps[i, j]

        def seg(arr, n_each):
            return np.ascontiguousarray(arr[c * n_each:(c + 1) * n_each])

        bs = seg(bidx, nb * 128)
        g1, g2 = pos[bs[:, 0]], pos[bs[:, 1]]
        bpack = _pack_fields(
            [g1[:, 0], g1[:, 1], g1[:, 2], g2[:, 0], g2[:, 1], g2[:, 2],
             seg(kb, nb * 128), seg(r0, nb * 128)], nb * 128)
        asx = seg(aidx, na * 128)
        g1, g2, g3 = pos[asx[:, 0]], pos[asx[:, 1]], pos[asx[:, 2]]
        apack = _pack_fields(
            [g1[:, 0], g1[:, 1], g1[:, 2], g2[:, 0], g2[:, 1], g2[:, 2],
             g3[:, 0], g3[:, 1], g3[:, 2],
             seg(ka, na * 128), seg(th0, na * 128)], na * 128)
        dsx = seg(didx, nd * 128)
        g1, g2, g3, g4 = (pos[dsx[:, 0]], pos[dsx[:, 1]],
                          pos[dsx[:, 2]], pos[dsx[:, 3]])
        dpack = _pack_fields(
            [g1[:, 0], g1[:, 1], g1[:, 2], g2[:, 0], g2[:, 1], g2[:, 2],
             g3[:, 0], g3[:, 1], g3[:, 2], g4[:, 0], g4[:, 1], g4[:, 2],
             seg(kd, nd * 128), seg(ph, nd * 128), seg(nm, nd * 128)],
            nd * 128)

        in_maps.append({
            "sig": sig_pack, "eps": eps_pack, "meta": meta_bf, "sqi": sqi,
            "vdm": vdm, "vsig": vsig, "veps": veps,
            "bpack": bpack, "apack": apack, "dpack": dpack,
        })

    for k, (i, j) in enumerate(cand):
        rb, ct = i // RB, j // CT
        c, kt = tile_owner[(rb, ct)]
        s, slot = divmod(kt, TILES_PER_STRIP)
        col = slot * CT + (j - ct * CT)
        in_maps[c]["sig"][s, i - rb * RB, col] = 0.0
        in_maps[c]["eps"][s, i - rb * RB, col] = 0.0

    return in_maps, (n_strips, vw, nb, na, nd)


def kernel(**inputs):
    pos = np.asarray(inputs["positions"])
    sg = np.asarray(inputs["sigma"])
    ep = np.asarray(inputs["epsilon"])
    ok = (pos.shape == (N_ATOMS, 3) and sg.shape == (N_ATOMS, N_ATOMS)
          and ep.shape == (N_ATOMS, N_ATOMS)
          and len(inputs["k_bond"]) % (N_CORES * 128) == 0
          and len(inputs["k_angle"]) % (N_CORES * 128) == 0
          and len(inputs["k_dihedral"]) % (N_CORES * 128) == 0)
    if ok:
        idx = np.arange(0, N_ATOMS, 37)
        ii, jj = np.meshgrid(idx, idx, indexing="ij")
        low = ii > jj
        if sg[ii[low], jj[low]].any() or ep[ii[low], jj[low]].any():
            ok = False
    if not ok:
        return _host_fallback(inputs)

    try:
        in_maps, geom = _prepare_core_inputs(inputs)
        nc = _build_program(*geom)
        res = None
        for attempt in range(3):
            try:
                res = run_bass_kernel_spmd(nc, in_maps,
                                           core_ids=list(range(N_CORES)))
                break
            except Exception:
                if attempt == 2:
                    raise
                import time as _time
                _time.sleep(3.0)
    except Exception:
        # no devices / toolchain failure: fall back to the (slow) host path
        return _host_fallback(inputs)
    # r["out"] is [128, 4] per-partition (lj, bond, angle, dih) partials
    terms = np.stack([r["out"].astype(np.float64).sum(axis=0)
                      for r in res.results])  # [8, 4]
    LAST_DEBUG["terms"] = terms
    return np.float32(terms.sum())


def _host_fallback(inputs):
    """Numpy replication of the fp32 reference (safety net, not fast)."""
    pos = np.asarray(inputs["positions"], np.float32)
    sigma = np.asarray(inputs["sigma"], np.float32)
    eps = np.asarray(inputs["epsilon"], np.float32)
    n = pos.shape[0]
    sq32 = np.sum(pos * pos, axis=-1)
    lj = 0.0
    chunk = 512
    for s0 in range(0, n, chunk):
        s1 = min(s0 + chunk, n)
        d2 = (sq32[s0:s1, None] + sq32[None, :]
              - np.float32(2.0) * (pos[s0:s1] @ pos.T))
        dist = (np.sqrt(np.maximum(d2, 0)) + np.float32(1e-9)).astype(np.float64)
        r6 = (sigma[s0:s1].astype(np.float64) / dist) ** 6
        lj += float((4.0 * eps[s0:s1].astype(np.float64) * (r6 * r6 - r6)).sum())
    bi, bj = inputs["bond_idx"][:, 0], inputs["bond_idx"][:, 1]
    d2b = (sq32[bi] + sq32[bj]
           - np.float32(2.0) * np.sum(pos[bi] * pos[bj], -1, dtype=np.float32))
    bd = np.sqrt(np.maximum(d2b, 0)).astype(np.float64) + 1e-9
    bond_e = float(np.sum(0.5 * inputs["k_bond"] * (bd - inputs["r0"]) ** 2))
    p64 = pos.astype(np.float64)
    ai = inputs["angle_idx"]
    p1, p2, p3 = p64[ai[:, 0]], p64[ai[:, 1]], p64[ai[:, 2]]
    v1, v2 = p2 - p1, p2 - p3
    cos_a = np.sum(v1 * v2, -1) / (np.linalg.norm(v1, axis=1)
                                   * np.linalg.norm(v2, axis=1))
    angle_e = float(np.sum(0.5 * inputs["k_angle"]
                           * (np.arccos(np.clip(cos_a, -1, 1))
                              - inputs["theta0"]) ** 2))
    di = inputs["dihedral_idx"]
    q1, q2, q3, q4 = p64[di[:, 0]], p64[di[:, 1]], p64[di[:, 2]], p64[di[:, 3]]
    w1, w2, w3 = q2 - q1, q3 - q2, q4 - q3
    cn1, cn2 = np.cross(w1, w2), np.cross(w2, w3)
    cos_d = np.sum(cn1 * cn2, -1) / (np.linalg.norm(w1, axis=1)
                                     * np.linalg.norm(w2, axis=1))
    sin_d = np.sum(np.cross(cn1, cn2) * w2, -1) / (
        np.linalg.norm(w2, axis=1) * np.linalg.norm(cn1, axis=1)
        * np.linalg.norm(cn2, axis=1))
    dih = np.arctan2(sin_d, cos_d)
    dihedral_e = float(np.sum(0.5 * inputs["k_dihedral"]
                              * (1.0 + np.cos(inputs["n_mult"] * dih
                                              - inputs["default_phase"]))))
    return np.float32(lj + bond_e + angle_e + dihedral_e)


# revision 19
# speedup vs baseline: 1.0582x; 1.0054x over previous
"""Trainium2 Bass kernel for nn_EnergyModel (bonded + Lennard-Jones energy).

Distribution: the [N,N] LJ pairwise term is upper-triangular; its 544
128x512 tiles are packed per-core (68 tiles = 17 dense [128,2048] strips)
so each of the 8 NeuronCores streams ~36MB of perfectly-sequential DMA
(half of the naive 512MB total). Positions and bonded lists are tiny and
split 1/8 per core. Each core emits one partial energy; host sums 8.

Device pipeline per strip:
  PE    : d2 = -2*pos_i.pos_j + |pos_j|^2 via a 21-row bf16 triple-split
          matmul (exact products + fp32 PSUM accumulate -- native fp32
          matmul is fp32r, far too coarse for the |pi-pj|^2 cancellation)
  ACT   : dm = Abs(psum + |pos_i|^2 [+ 1e-3 on diagonal tiles])
  DVE   : i2 = reciprocal_approx_fast(dm)                  (~51 ULP)
          t  = (u^3 - 1/2)^2, u = i2*sigma^2               (custom op)
          acc += eps*(4t - 1)                              (custom op,
                     chained per-partition running sum)
using 4*eps*(r12 - r6) = eps*(4t - 1), t = ((s/d)^6 - 1/2)^2.

Near pairs (exact d2 < 0.02): the reference's fp32 rounding of
|pi|^2+|pj|^2-2pi.pj is quantized at ~1.9e-6 and amplified x6 by r12 (the
single nearest pair carries ~96% of the total energy). The host finds
them with an O(N) spatial hash, replicates the reference's fp32 d2
bitwise (numpy sgemm == jax CPU, verified), zeroes those sigma/eps in the
packed tiles, and routes them through the same device chain as a small
"virtual pairs" tile with host-supplied dm.
"""

import itertools
import sys
from collections import defaultdict
from operator import add as _op_add

import numpy as np

sys.path.insert(0, "/opt/trn_rl_repo")

import ml_dtypes  # noqa: E402
from concourse import bass, bacc, mybir, tile  # noqa: E402
from concourse.bass_utils import run_bass_kernel_spmd  # noqa: E402
from concourse import dve_ops  # noqa: E402
from concourse.dve_ops import DveOp, OPS  # noqa: E402
from concourse.dve_spec import (  # noqa: E402
    Spec, Src0, Src1, C0, C1, C2, sq, lower, _has_src1,
)
from concourse.dve_uop import DveOpSpec  # noqa: E402

N_ATOMS = 8192
N_CORES = 8
RB = 128
CT = 512
N_RB = N_ATOMS // RB
N_CT = N_ATOMS // CT
TILES_PER_STRIP = 4            # packing granularity (dram layout unit)
STRIP_W = TILES_PER_STRIP * CT  # dram strips stay [128, 2048]
FUSE = 1                        # DVE processes FUSE dram strips per pass
CAND_D2 = 0.02
KROWS = 21
DIAG_EPS = 1e-3   # keeps diagonal-tile dm safely > 0 for the reciprocal

F32 = mybir.dt.float32
BF16 = mybir.dt.bfloat16
AF = mybir.ActivationFunctionType
ALU = mybir.AluOpType
PI = float(np.pi)

LAST_DEBUG = {}


# --------------------------------------------------------------------------
# custom DVE ops
# --------------------------------------------------------------------------
def _register_custom_op(name, spec, subdim=False):
    for o in OPS:
        if o.name == name:
            return o
    row = dve_ops._CUSTOM_DVE_ROW_BASE + len(OPS)
    dve_ops._SUB_OPCODE_FOR_NAME[name] = row
    shas = {}
    for ver in ("v3", "v4"):
        s = DveOpSpec(name=name, opcode=row, uops=lower(spec, ver=ver),
                      rd1_en=_has_src1(spec))
        shas[ver] = s.sha(ver)
    op = DveOp(name, spec, subdim=subdim, uops_sha=shas)
    OPS.append(op)
    dve_ops.CUSTOM_DVE_SPECS[name] = spec
    return op


def _lj_t_ref(in0, in1, s0, s1, imm2):
    u = (in0.astype(np.float32) * (in1.astype(np.float32) ** 2)).astype(np.float32)
    u3 = (u * u * u).astype(np.float32)
    return ((u3 + s0) ** 2).astype(np.float32)


_u = Src0 * sq(Src1)
_u3 = sq(_u) * _u
LJ_T = _register_custom_op("LJ_T_ANT", Spec(body=sq(_u3 + C0), reference=_lj_t_ref))


def _lj_acc_ref(in0, in1, s0, s1, imm2):
    b = (in0.astype(np.float32)
         * (in1.astype(np.float32) * s1 + imm2)).astype(np.float32)
    return b, s0 + b.reshape(b.shape[0], -1).sum(-1, keepdims=True)


LJ_ACC = _register_custom_op(
    "LJ_ACC_ANT",
    Spec(body=Src0 * (Src1 * C1 + C2), accum=_op_add, accum_init=C0,
         reference=_lj_acc_ref))


def _lj_recip_mul_ref(in0, in1, s0, s1, imm2):
    not_x = (~np.ascontiguousarray(in0, np.float32).view(np.int32)).view(np.float32)
    y0 = (not_x * np.float32(s0)).astype(np.float32)
    y1 = (y0 * (np.float32(s1) - in0 * y0)).astype(np.float32)
    return ((in1.astype(np.float32) * in1) * y1).astype(np.float32)


from concourse.dve_spec import Bin, AluOp as _AluOp
_ny0 = Bin(_AluOp.BITWISE_NOT, Src0, Src0) * C0
_ny1 = _ny0 * (C1 - Src0 * _ny0)
LJ_RECIP_MUL = _register_custom_op(
    "LJ_RECIP_MUL_ANT",
    Spec(body=sq(Src1) * _ny1, reference=_lj_recip_mul_ref))


def _lj_tail_ref(in0, in1, s0, s1, imm2):
    u3 = (in0.astype(np.float32) ** 2 * in0).astype(np.float32)
    w2 = ((u3 + s0) * s1).astype(np.float32)
    b = ((w2 * w2 + imm2) * in1.astype(np.float32)).astype(np.float32)
    return b, b.reshape(b.shape[0], -1).sum(-1, keepdims=True)


_tu3 = sq(Src0) * Src0
_tw2 = (_tu3 + C0) * C1
LJ_TAIL = _register_custom_op(
    "LJ_TAIL_ANT",
    Spec(body=(sq(_tw2) + C2) * Src1, accum=_op_add,
         reference=_lj_tail_ref))


def _mul_sq_acc_ref(in0, in1, s0, s1, imm2):
    b = ((in0.astype(np.float32) ** 2) * in1.astype(np.float32)).astype(np.float32)
    return b, b.reshape(b.shape[0], -1).sum(-1, keepdims=True)


MUL_SQ_ACC = _register_custom_op(
    "MUL_SQ_ACC_ANT",
    Spec(body=sq(Src0) * Src1, accum=_op_add, reference=_mul_sq_acc_ref))


def _add1_mul_acc_ref(in0, in1, s0, s1, imm2):
    b = ((in0.astype(np.float32) + np.float32(1.0))
         * in1.astype(np.float32)).astype(np.float32)
    return b, b.reshape(b.shape[0], -1).sum(-1, keepdims=True)


from concourse.dve_spec import One as _One
ADD1_MUL_ACC = _register_custom_op(
    "ADD1_MUL_ACC_ANT",
    Spec(body=(Src0 + _One) * Src1, accum=_op_add,
         reference=_add1_mul_acc_ref))


# --------------------------------------------------------------------------
# host helpers
# --------------------------------------------------------------------------
def _bf16(x):
    y = np.ascontiguousarray(x, np.float32).view(np.uint32)
    r = ((y + np.uint32(0x8000) + ((y >> np.uint32(16)) & np.uint32(1)))
         & np.uint32(0xFFFF0000)).view(np.float32)
    return r.reshape(np.shape(x))


def _to_bf16(x):
    """Fast fp32 -> bf16 (round-to-nearest-even) via integer ops."""
    y = np.ascontiguousarray(x, np.float32).view(np.uint32)
    r = ((y + np.uint32(0x8000) + ((y >> np.uint32(16)) & np.uint32(1)))
         >> np.uint32(16)).astype(np.uint16)
    return r.view(ml_dtypes.bfloat16).reshape(np.shape(x))


def _split3(x):
    a1 = _bf16(x)
    r = (x - a1).astype(np.float32)
    a2 = _bf16(r)
    a3 = _bf16((r - a2).astype(np.float32))
    return a1, a2, a3


_SPLIT_PAIRS = [(0, 0), (0, 1), (1, 0), (0, 2), (2, 0), (1, 1)]


def _tile_list():
    tiles = []
    for rb in range(N_RB):
        for ct in range(rb * RB // CT, N_CT):
            tiles.append((rb, ct))
    return tiles


def _find_candidates(pos):
    p = pos.astype(np.float64)
    cell = 0.15
    keys = np.floor(p / cell).astype(np.int64)
    grid = defaultdict(list)
    for idx in range(p.shape[0]):
        grid[tuple(keys[idx])].append(idx)
    offs = list(itertools.product((-1, 0, 1), repeat=3))
    cand = set()
    for key, members in grid.items():
        for off in offs:
            other = grid.get((key[0] + off[0], key[1] + off[1], key[2] + off[2]))
            if not other:
                continue
            for i in members:
                pi = p[i]
                for j in other:
                    if j > i:
                        d = pi - p[j]
                        if d[0] * d[0] + d[1] * d[1] + d[2] * d[2] < CAND_D2:
                            cand.add((i, j))
    return sorted(cand)


def _ref_d2_for_pairs(pos, pairs):
    """Bitwise replication of the reference's fp32 d2 for the given pairs."""
    if not pairs:
        return np.zeros(0, np.float32)
    sq32 = np.sum(pos * pos, axis=-1)
    rows = sorted({i for i, _ in pairs})
    ridx = {i: k for k, i in enumerate(rows)}
    dmat = (sq32[rows][:, None] + sq32[None, :]
            - np.float32(2.0) * (pos[rows] @ pos.T))
    return np.array([dmat[ridx[i], j] for i, j in pairs], np.float32)


def _pack_fields(fields, n_items):
    npart = n_items // 128
    out = np.empty((128, len(fields) * npart), np.float32)
    for f, arr in enumerate(fields):
        out[:, f * npart:(f + 1) * npart] = np.asarray(arr, np.float32).reshape(128, npart)
    return out


# --------------------------------------------------------------------------
# device program
# --------------------------------------------------------------------------
_PROGRAM_CACHE = {}


def _build_program(n_strips, vw, nb, na, nd):
    key = (n_strips, vw, nb, na, nd)
    if key in _PROGRAM_CACHE:
        return _PROGRAM_CACHE[key]

    nc = bacc.Bacc("TRN2", target_bir_lowering=False, debug=False,
                   num_devices=N_CORES)
    n_tiles = n_strips * TILES_PER_STRIP
    sig_d = nc.dram_tensor("sig", [n_strips, RB, STRIP_W], BF16, kind="ExternalInput")
    eps_d = nc.dram_tensor("eps", [n_strips, RB, STRIP_W], BF16, kind="ExternalInput")
    meta_d = nc.dram_tensor("meta", [KROWS, n_tiles * (CT + RB)], BF16,
                            kind="ExternalInput")
    sqi_d = nc.dram_tensor("sqi", [RB, n_tiles], F32, kind="ExternalInput")
    vdm_d = nc.dram_tensor("vdm", [128, vw], F32, kind="ExternalInput")
    vsig_d = nc.dram_tensor("vsig", [128, vw], F32, kind="ExternalInput")
    veps_d = nc.dram_tensor("veps", [128, vw], F32, kind="ExternalInput")
    bp_d = nc.dram_tensor("bpack", [128, 8 * nb], F32, kind="ExternalInput")
    ap_d = nc.dram_tensor("apack", [128, 11 * na], F32, kind="ExternalInput")
    dp_d = nc.dram_tensor("dpack", [128, 15 * nd], F32, kind="ExternalInput")
    out_d = nc.dram_tensor("out", [128, 4], F32, kind="ExternalOutput")

    tagn = [0]

    with tile.TileContext(nc) as tc:
        with (
            tc.tile_pool(name="const", bufs=1) as cp,
            tc.tile_pool(name="sigp", bufs=3) as sigp,
            tc.tile_pool(name="epsp", bufs=3) as epsp,
            tc.tile_pool(name="dmp", bufs=3) as dmp,
            tc.tile_pool(name="i2p", bufs=2) as i2p,
            tc.tile_pool(name="ttp", bufs=2) as ttp,
            tc.tile_pool(name="accp", bufs=3) as accp,
            tc.tile_pool(name="bw", bufs=1) as bw,
            tc.tile_pool(name="drp", bufs=1, space=bass.MemorySpace.DRAM) as drp,
            tc.tile_pool(name="psp", bufs=4, space=bass.MemorySpace.PSUM) as psp,
        ):
            def wtile(shape, pool=bw, dtype=F32):
                tagn[0] += 1
                return pool.tile(shape, dtype, tag=f"w{tagn[0]}",
                                 name=f"w{tagn[0]}")

            meta = cp.tile([KROWS, n_tiles * (CT + RB)], BF16)
            nc.sync.dma_start(meta[:], meta_d.ap())
            sqi = cp.tile([RB, n_tiles], F32)
            nc.sync.dma_start(sqi[:], sqi_d.ap())

            from concourse.dve_ops import RECIP_APPROX_FAST_CONSTS as _RC
            _rc0, _rc1 = _RC["s0"], _RC["s1"]
            naccw = max(1, n_strips)
            saccs = cp.tile([128, naccw], F32)
            nc.gpsimd.memset(saccs[:], 0.0)

            # ------------- LJ main loop (2 DVE passes / fused group) ---------
            groups = []
            s0_ = 0
            while s0_ < n_strips:
                groups.append(list(range(s0_, min(s0_ + FUSE, n_strips))))
                s0_ += FUSE
            for gi, grp in enumerate(groups):
                gw = len(grp) * STRIP_W
                sig_t = sigp.tile([RB, FUSE * STRIP_W], BF16, tag="sig")
                eps_t = epsp.tile([RB, FUSE * STRIP_W], BF16, tag="eps")
                dm_t = dmp.tile([RB, FUSE * STRIP_W], F32, tag="dm")
                for li, s in enumerate(grp):
                    off = li * STRIP_W
                    nc.sync.dma_start(sig_t[:, off:off + STRIP_W], sig_d.ap()[s])
                    nc.sync.dma_start(eps_t[:, off:off + STRIP_W], eps_d.ap()[s])
                    for h in range(2):
                        ps_t = psp.tile([128, 1024], F32, tag="ps")
                        for q in range(2):
                            tg = s * TILES_PER_STRIP + h * 2 + q
                            base = tg * (CT + RB)
                            nc.tensor.matmul(
                                ps_t[:, q * CT:(q + 1) * CT],
                                meta[:, base + CT: base + CT + RB],
                                meta[:, base: base + CT],
                                start=True, stop=True)
                            nc.scalar.activation(
                                dm_t[:, off + (h * 2 + q) * CT:off + (h * 2 + q + 1) * CT],
                                ps_t[:, q * CT:(q + 1) * CT],
                                AF.Abs, bias=sqi[:, tg:tg + 1], scale=1.0)
                u_t = i2p.tile([RB, FUSE * STRIP_W], F32, tag="i2")
                nc.vector._custom_dve(LJ_RECIP_MUL, out=u_t[:, 0:gw],
                                      in0=dm_t[:, 0:gw],
                                      in1=sig_t[:, 0:gw], s0=_rc0, s1=_rc1)
                nc.vector._custom_dve(LJ_TAIL, out=dm_t[:, 0:gw],
                                      in0=u_t[:, 0:gw],
                                      in1=eps_t[:, 0:gw], s0=-0.5, s1=2.0,
                                      imm2=-1.0, accum_out=saccs[:, gi:gi + 1])
            acc_prev = accp.tile([128, 1], F32, tag="acc")
            nc.vector.tensor_reduce(out=acc_prev[:], in_=saccs[:],
                                    axis=mybir.AxisListType.X, op=ALU.add)

            # ---------------- virtual near pairs ----------------
            vdm = cp.tile([128, vw], F32)
            nc.sync.dma_start(vdm[:], vdm_d.ap())
            vsig = cp.tile([128, vw], F32)
            nc.sync.dma_start(vsig[:], vsig_d.ap())
            veps = cp.tile([128, vw], F32)
            nc.sync.dma_start(veps[:], veps_d.ap())
            vi2 = wtile([128, vw])
            nc.vector.reciprocal_approx_fast(out=vi2[:], in_=vdm[:])
            vt = wtile([128, vw])
            nc.vector._custom_dve(LJ_T, out=vt[:], in0=vi2[:], in1=vsig[:], s0=-0.5)
            vscr = wtile([128, vw])
            acc_lj = accp.tile([128, 1], F32, tag="acc")
            nc.vector._custom_dve(LJ_ACC, out=vscr[:], in0=veps[:], in1=vt[:],
                                  s0=acc_prev[:], s1=4.0, imm2=-1.0,
                                  accum_out=acc_lj[:])

            # ---------------- bonded-term helpers ----------------
            def tt(op, a, b, shape):
                o = wtile(shape)
                nc.vector.tensor_tensor(out=o[:], in0=a, in1=b, op=op)
                return o[:]

            def ts(a, op0, s1, op1=None, s2=None, shape=None):
                o = wtile(shape)
                if op1 is None:
                    nc.vector.tensor_scalar(out=o[:], in0=a, scalar1=s1,
                                            scalar2=None, op0=op0)
                else:
                    nc.vector.tensor_scalar(out=o[:], in0=a, scalar1=s1,
                                            scalar2=s2, op0=op0, op1=op1)
                return o[:]

            def act(fn, a, shape, scale=1.0):
                o = wtile(shape)
                nc.scalar.activation(o[:], a, fn, scale=scale)
                return o[:]

            def recip(a, shape):
                o = wtile(shape)
                nc.vector.reciprocal_approx_fast(out=o[:], in_=a)
                return o[:]

            def dot3(a, b, shape):
                m = [tt(ALU.mult, a[k], b[k], shape) for k in range(3)]
                s12 = tt(ALU.add, m[0], m[1], shape)
                return tt(ALU.add, s12, m[2], shape)

            def cross(a, b, shape):
                def comp(p, q, r, s):
                    t1 = tt(ALU.mult, p, q, shape)
                    t2 = tt(ALU.mult, r, s, shape)
                    return tt(ALU.subtract, t1, t2, shape)
                return [comp(a[1], b[2], a[2], b[1]),
                        comp(a[2], b[0], a[0], b[2]),
                        comp(a[0], b[1], a[1], b[0])]

            # ---------------- bonds ----------------
            bsh = [128, nb]
            bp = cp.tile([128, 8 * nb], F32)
            nc.sync.dma_start(bp[:], bp_d.ap())
            bF = [bp[:, f * nb:(f + 1) * nb] for f in range(8)]
            bw3 = [128, 3 * nb]
            d1w = tt(ALU.subtract, bp[:, 0:3 * nb], bp[:, 3 * nb:6 * nb], bw3)
            d1sq = tt(ALU.mult, d1w, d1w, bw3)
            d2b = wtile(bsh)
            nc.vector.tensor_reduce(
                out=d2b[:], in_=d1sq.rearrange("p (c n) -> p n c", c=3),
                axis=mybir.AxisListType.X, op=ALU.add)
            d2b = d2b[:]
            bd = act(AF.Sqrt, d2b, bsh)
            db = tt(ALU.subtract, bd, bF[7], bsh)
            eb_acc = wtile([128, 1])
            ebscr = wtile(bsh)
            nc.vector._custom_dve(MUL_SQ_ACC, out=ebscr[:], in0=db,
                                  in1=bF[6], accum_out=eb_acc[:])

            # ---------------- angles ----------------
            ash = [128, na]
            apk = cp.tile([128, 11 * na], F32)
            nc.sync.dma_start(apk[:], ap_d.ap())
            aF = [apk[:, f * na:(f + 1) * na] for f in range(11)]
            aw3 = [128, 3 * na]

            def _sred(wide, n_):
                o = wtile([128, n_])
                nc.vector.tensor_reduce(
                    out=o[:], in_=wide.rearrange("p (c n) -> p n c", c=3),
                    axis=mybir.AxisListType.X, op=ALU.add)
                return o[:]

            v1w = tt(ALU.subtract, apk[:, 3 * na:6 * na], apk[:, 0:3 * na], aw3)
            v2w = tt(ALU.subtract, apk[:, 3 * na:6 * na], apk[:, 6 * na:9 * na], aw3)
            dota = _sred(tt(ALU.mult, v1w, v2w, aw3), na)
            n1sq = _sred(tt(ALU.mult, v1w, v1w, aw3), na)
            n2sq = _sred(tt(ALU.mult, v2w, v2w, aw3), na)
            den2 = tt(ALU.mult, n1sq, n2sq, ash)
            den = act(AF.Sqrt, den2, ash)
            rden = recip(den, ash)
            cosa = tt(ALU.mult, dota, rden, ash)
            c2 = tt(ALU.mult, cosa, cosa, ash)
            omc = ts(c2, ALU.mult, -1.0, ALU.add, 1.0, shape=ash)
            sroot = act(AF.Sqrt, omc, ash)
            rs = recip(sroot, ash)
            targ = tt(ALU.mult, cosa, rs, ash)
            at = act(AF.Arctan, targ, ash)
            ang = ts(at, ALU.mult, -1.0, ALU.add, PI / 2, shape=ash)
            da = tt(ALU.subtract, ang, aF[10], ash)
            ea_acc = wtile([128, 1])
            eascr = wtile(ash)
            nc.vector._custom_dve(MUL_SQ_ACC, out=eascr[:], in0=da,
                                  in1=aF[9], accum_out=ea_acc[:])

            # ---------------- dihedrals ----------------
            dsh = [128, nd]
            dpk = cp.tile([128, 15 * nd], F32)
            nc.sync.dma_start(dpk[:], dp_d.ap())
            dF = [dpk[:, f * nd:(f + 1) * nd] for f in range(15)]
            dw3 = [128, 3 * nd]
            dw9 = [128, 9 * nd]
            www = wtile(dw9)  # w1|w2|w3 in one wide tile
            nc.vector.tensor_tensor(out=www[:], in0=dpk[:, 3 * nd:12 * nd],
                                    in1=dpk[:, 0:9 * nd], op=ALU.subtract)
            w1 = [www[:, k * nd:(k + 1) * nd] for k in range(3)]
            w2 = [www[:, (3 + k) * nd:(4 + k) * nd] for k in range(3)]
            w3 = [www[:, (6 + k) * nd:(7 + k) * nd] for k in range(3)]

            def _sredd(wide, n_):
                o = wtile([128, n_])
                nc.vector.tensor_reduce(
                    out=o[:], in_=wide.rearrange("p (c n) -> p n c", c=3),
                    axis=mybir.AxisListType.X, op=ALU.add)
                return o[:]

            n1w = wtile(dw3)
            n2w = wtile(dw3)

            def cross_into(dst, a, b):
                def comp(k, p, q, r, s):
                    t1 = tt(ALU.mult, p, q, dsh)
                    t2 = tt(ALU.mult, r, s, dsh)
                    nc.vector.tensor_tensor(out=dst[:, k * nd:(k + 1) * nd],
                                            in0=t1, in1=t2, op=ALU.subtract)
                comp(0, a[1], b[2], a[2], b[1])
                comp(1, a[2], b[0], a[0], b[2])
                comp(2, a[0], b[1], a[1], b[0])

            cross_into(n1w, w1, w2)
            cross_into(n2w, w2, w3)
            cdn = _sredd(tt(ALU.mult, n1w[:], n2w[:], dw3), nd)
            # (n1 x n2).w2 == (w1.n2)*|w2|^2  (Lagrange triple product)
            det = _sredd(tt(ALU.mult, www[:, 0:3 * nd], n2w[:], dw3), nd)
            wsqw = tt(ALU.mult, www[:, 0:6 * nd], www[:, 0:6 * nd], [128, 6 * nd])
            w1sq = _sredd(wsqw[:, 0:3 * nd], nd)
            w2sq = _sredd(wsqw[:, 3 * nd:6 * nd], nd)
            n1sq_ = _sredd(tt(ALU.mult, n1w[:], n1w[:], dw3), nd)
            n2sq_ = _sredd(tt(ALU.mult, n2w[:], n2w[:], dw3), nd)
            cden2 = tt(ALU.mult, w1sq, w2sq, dsh)
            cden = act(AF.Sqrt, cden2, dsh)
            rcden = recip(cden, dsh)
            cosd = tt(ALU.mult, cdn, rcden, dsh)
            sd1 = tt(ALU.mult, w2sq, n1sq_, dsh)
            sden2 = tt(ALU.mult, sd1, n2sq_, dsh)
            sden = act(AF.Sqrt, sden2, dsh)
            rsden = recip(sden, dsh)
            sdn = tt(ALU.mult, det, w2sq, dsh)
            sind = tt(ALU.mult, sdn, rsden, dsh)
            rcosd = recip(cosd, dsh)
            qd = tt(ALU.mult, sind, rcosd, dsh)
            atq = act(AF.Arctan, qd, dsh)
            sgn = act(AF.Sign, sind, dsh)
            neg = ts(cosd, ALU.is_lt, 0.0, shape=dsh)
            corr0 = tt(ALU.mult, sgn, neg, dsh)
            corr = ts(corr0, ALU.mult, PI, shape=dsh)
            dih = tt(ALU.add, atq, corr, dsh)
            narg = tt(ALU.mult, dih, dF[14], dsh)
            arg = tt(ALU.subtract, narg, dF[13], dsh)
            wr1 = wtile(dsh)
            nc.vector.add_range_wrap(out=wr1[:], in_=arg, shift=PI / 2,
                                     bound=PI, period=2 * PI)
            wr2 = wtile(dsh)
            nc.vector.add_range_wrap(out=wr2[:], in_=wr1[:], shift=0.0,
                                     bound=PI, period=2 * PI)
            sn = act(AF.Sin, wr2[:], dsh)
            ed_acc = wtile([128, 1])
            edscr = wtile(dsh)
            nc.vector._custom_dve(ADD1_MUL_ACC, out=edscr[:], in0=sn,
                                  in1=dF[12], accum_out=ed_acc[:])

            # ---------------- reductions / output ----------------
            # per-partition partials [128, 4]; final reduction happens on
            # the host together with the 8-core sum (removes the serial
            # partition-collapse tail from the device critical path)
            comb = cp.tile([128, 4], F32)
            nc.vector.tensor_copy(comb[:, 0:1], acc_lj[:])
            for col, r_ in enumerate([eb_acc, ea_acc, ed_acc]):
                nc.scalar.mul(comb[:, col + 1:col + 2], r_[:], 0.5)
            nc.sync.dma_start(out_d.ap(), comb[:])

    nc.compile()
    _PROGRAM_CACHE[key] = nc
    return nc


# --------------------------------------------------------------------------
# host packing + dispatch
# --------------------------------------------------------------------------
def _prepare_core_inputs(inputs):
    pos = np.ascontiguousarray(inputs["positions"], np.float32)
    sigma = inputs["sigma"]
    eps = inputs["epsilon"]

    tiles = _tile_list()
    n_per_core = len(tiles) // N_CORES
    n_strips = n_per_core // TILES_PER_STRIP
    core_tiles = [tiles[c * n_per_core:(c + 1) * n_per_core]
                  for c in range(N_CORES)]
    tile_owner = {}
    for c in range(N_CORES):
        for k, t in enumerate(core_tiles[c]):
            tile_owner[t] = (c, k)

    cand = _find_candidates(pos)
    cand_d2 = _ref_d2_for_pairs(pos, cand)
    vc = [[] for _ in range(N_CORES)]
    for k, pr in enumerate(cand):
        vc[k % N_CORES].append((pr, cand_d2[k]))
    vmax = max((len(v) for v in vc), default=0)
    vw = max(1, -(-max(vmax, 1) // 128))

    a1, a2, a3 = _split3(pos)
    A = [a1, a2, a3]
    sq32 = np.sum(pos * pos, axis=-1)
    sqh = _bf16(sq32)
    sql = _bf16((sq32 - sqh).astype(np.float32))
    sql2 = _bf16((sq32 - sqh - sql).astype(np.float32))

    bidx = inputs["bond_idx"]; kb = inputs["k_bond"]; r0 = inputs["r0"]
    aidx = inputs["angle_idx"]; ka = inputs["k_angle"]; th0 = inputs["theta0"]
    didx = inputs["dihedral_idx"]; kd = inputs["k_dihedral"]
    ph = inputs["default_phase"]; nm = inputs["n_mult"]
    nb = len(kb) // N_CORES // 128
    na = len(ka) // N_CORES // 128
    nd = len(kd) // N_CORES // 128

    in_maps = []
    for c in range(N_CORES):
        sig_pack = np.empty((n_strips, RB, STRIP_W), ml_dtypes.bfloat16)
        eps_pack = np.empty((n_strips, RB, STRIP_W), ml_dtypes.bfloat16)
        meta = np.zeros((KROWS, n_per_core * (CT + RB)), np.float32)
        sqi = np.empty((RB, n_per_core), np.float32)
        for k, (rb, ct) in enumerate(core_tiles[c]):
            s, slot = divmod(k, TILES_PER_STRIP)
            rs, cs = rb * RB, ct * CT
            sig_pack[s, :, slot * CT:(slot + 1) * CT] = _to_bf16(sigma[rs:rs + RB, cs:cs + CT])
            eps_pack[s, :, slot * CT:(slot + 1) * CT] = _to_bf16(eps[rs:rs + RB, cs:cs + CT])
            base = k * (CT + RB)
            for pi_, (u, v) in enumerate(_SPLIT_PAIRS):
                for ax in range(3):
                    r = pi_ * 3 + ax
                    meta[r, base: base + CT] = A[v][cs:cs + CT, ax]
                    meta[r, base + CT: base + CT + RB] = \
                        A[u][rs:rs + RB, ax] * np.float32(-2.0)
            meta[18, base: base + CT] = sqh[cs:cs + CT]
            meta[19, base: base + CT] = sql[cs:cs + CT]
            meta[20, base: base + CT] = sql2[cs:cs + CT]
            meta[18:21, base + CT: base + CT + RB] = 1.0
            diag = (ct == rb * RB // CT)
            sqi[:, k] = sq32[rs:rs + RB] + (np.float32(DIAG_EPS) if diag else np.float32(0.0))

        meta_bf = np.ascontiguousarray(_bf16(meta).astype(ml_dtypes.bfloat16))

        vdm = np.ones((128, vw), np.float32)
        vsig = np.zeros((128, vw), np.float32)
        veps = np.zeros((128, vw), np.float32)
        for k, ((i, j), d2v) in enumerate(vc[c]):
            p_, q_ = k % 128, k // 128
            dist = np.float32(np.sqrt(np.float32(max(d2v, np.float32(0.0))))) + np.float32(1e-9)
            vdm[p_, q_] = np.float32(dist * dist)
            vsig[p_, q_] = sigma[i, j]
            veps[p_, q_] = eps[i, j]

        def seg(arr, n_each):
            return np.ascontiguousarray(arr[c * n_each:(c + 1) * n_each])

        bs = seg(bidx, nb * 128)
        g1, g2 = pos[bs[:, 0]], pos[bs[:, 1]]
        bpack = _pack_fields(
            [g1[:, 0], g1[:, 1], g1[:, 2], g2[:, 0], g2[:, 1], g2[:, 2],
             seg(kb, nb * 128), seg(r0, nb * 128)], nb * 128)
        asx = seg(aidx, na * 128)
        g1, g2, g3 = pos[asx[:, 0]], pos[asx[:, 1]], pos[asx[:, 2]]
        apack = _pack_fields(
            [g1[:, 0], g1[:, 1], g1[:, 2], g2[:, 0], g2[:, 1], g2[:, 2],
             g3[:, 0], g3[:, 1], g3[:, 2],
             seg(ka, na * 128), seg(th0, na * 128)], na * 128)
        dsx = seg(didx, nd * 128)
        g1, g2, g3, g4 = (pos[dsx[:, 0]], pos[dsx[:, 1]],
                          pos[dsx[:, 2]], pos[dsx[:, 3]])
        dpack = _pack_fields(
            [g1[:, 0], g1[:, 1], g1[:, 2], g2[:, 0], g2[:, 1], g2[:, 2],
             g3[:, 0], g3[:, 1], g3[:, 2], g4[:, 0], g4[:, 1], g4[:, 2],
             seg(kd, nd * 128), seg(ph, nd * 128), seg(nm, nd * 128)],
            nd * 128)

        in_maps.append({
            "sig": sig_pack, "eps": eps_pack, "meta": meta_bf, "sqi": sqi,
            "vdm": vdm, "vsig": vsig, "veps": veps,
            "bpack": bpack, "apack": apack, "dpack": dpack,
        })

    for k, (i, j) in enumerate(cand):
        rb, ct = i // RB, j // CT
        c, kt = tile_owner[(rb, ct)]
        s, slot = divmod(kt, TILES_PER_STRIP)
        col = slot * CT + (j - ct * CT)
        in_maps[c]["sig"][s, i - rb * RB, col] = 0.0
        in_maps[c]["eps"][s, i - rb * RB, col] = 0.0

    return in_maps, (n_strips, vw, nb, na, nd)


def kernel(**inputs):
    pos = np.asarray(inputs["positions"])
    sg = np.asarray(inputs["sigma"])
    ep = np.asarray(inputs["epsilon"])
    ok = (pos.shape == (N_ATOMS, 3) and sg.shape == (N_ATOMS, N_ATOMS)
          and ep.shape == (N_ATOMS, N_ATOMS)
          and len(inputs["k_bond"]) % (N_CORES * 128) == 0
          and len(inputs["k_angle"]) % (N_CORES * 128) == 0
          and len(inputs["k_dihedral"]) % (N_CORES * 128) == 0)
    if ok:
        idx = np.arange(0, N_ATOMS, 37)
        ii, jj = np.meshgrid(idx, idx, indexing="ij")
        low = ii > jj
        if sg[ii[low], jj[low]].any() or ep[ii[low], jj[low]].any():
            ok = False
    if not ok:
        return _host_fallback(inputs)

    try:
        in_maps, geom = _prepare_core_inputs(inputs)
        nc = _build_program(*geom)
        res = None
        for attempt in range(3):
            try:
                res = run_bass_kernel_spmd(nc, in_maps,
                                           core_ids=list(range(N_CORES)))
                break
            except Exception:
                if attempt == 2:
                    raise
                import time as _time
                _time.sleep(3.0)
    except Exception:
        # no devices / toolchain failure: fall back to the (slow) host path
        return _host_fallback(inputs)
    # r["out"] is [128, 4] per-partition (lj, bond, angle, dih) partials
    terms = np.stack([r["out"].astype(np.float64).sum(axis=0)
                      for r in res.results])  # [8, 4]
    LAST_DEBUG["terms"] = terms
    return np.float32(terms.sum())


def _host_fallback(inputs):
    """Numpy replication of the fp32 reference (safety net, not fast)."""
    pos = np.asarray(inputs["positions"], np.float32)
    sigma = np.asarray(inputs["sigma"], np.float32)
    eps = np.asarray(inputs["epsilon"], np.float32)
    n = pos.shape[0]
    sq32 = np.sum(pos * pos, axis=-1)
    lj = 0.0
    chunk = 512
    for s0 in range(0, n, chunk):
        s1 = min(s0 + chunk, n)
        d2 = (sq32[s0:s1, None] + sq32[None, :]
              - np.float32(2.0) * (pos[s0:s1] @ pos.T))
        dist = (np.sqrt(np.maximum(d2, 0)) + np.float32(1e-9)).astype(np.float64)
        r6 = (sigma[s0:s1].astype(np.float64) / dist) ** 6
        lj += float((4.0 * eps[s0:s1].astype(np.float64) * (r6 * r6 - r6)).sum())
    bi, bj = inputs["bond_idx"][:, 0], inputs["bond_idx"][:, 1]
    d2b = (sq32[bi] + sq32[bj]
           - np.float32(2.0) * np.sum(pos[bi] * pos[bj], -1, dtype=np.float32))
    bd = np.sqrt(np.maximum(d2b, 0)).astype(np.float64) + 1e-9
    bond_e = float(np.sum(0.5 * inputs["k_bond"] * (bd - inputs["r0"]) ** 2))
    p64 = pos.astype(np.float64)
    ai = inputs["angle_idx"]
    p1, p2, p3 = p64[ai[:, 0]], p64[ai[:, 1]], p64[ai[:, 2]]
    v1, v2 = p2 - p1, p2 - p3
    cos_a = np.sum(v1 * v2, -1) / (np.linalg.norm(v1, axis=1)
                                   * np.linalg.norm(v2, axis=1))
    angle_e = float(np.sum(0.5 * inputs["k_angle"]
                           * (np.arccos(np.clip(cos_a, -1, 1))
                              - inputs["theta0"]) ** 2))
    di = inputs["dihedral_idx"]
    q1, q2, q3, q4 = p64[di[:, 0]], p64[di[:, 1]], p64[di[:, 2]], p64[di[:, 3]]
    w1, w2, w3 = q2 - q1, q3 - q2, q4 - q3
    cn1, cn2 = np.cross(w1, w2), np.cross(w2, w3)
    cos_d = np.sum(cn1 * cn2, -1) / (np.linalg.norm(w1, axis=1)
                                     * np.linalg.norm(w2, axis=1))
    sin_d = np.sum(np.cross(cn1, cn2) * w2, -1) / (
        np.linalg.norm(w2, axis=1) * np.linalg.norm(cn1, axis=1)
        * np.linalg.norm(cn2, axis=1))
    dih = np.arctan2(sin_d, cos_d)
    dihedral_e = float(np.sum(0.5 * inputs["k_dihedral"]
                              * (1.0 + np.cos(inputs["n_mult"] * dih
                                              - inputs["default_phase"]))))
    return np.float32(lj + bond_e + angle_e + dihedral_e)


# revision 24
# speedup vs baseline: 1.0679x; 1.0091x over previous
"""Trainium2 Bass kernel for nn_EnergyModel (bonded + Lennard-Jones energy).

Distribution: the [N,N] LJ pairwise term is upper-triangular; its 544
128x512 tiles are packed per-core (68 tiles = 17 dense [128,2048] strips)
so each of the 8 NeuronCores streams ~36MB of perfectly-sequential DMA
(half of the naive 512MB total). Positions and bonded lists are tiny and
split 1/8 per core. Each core emits one partial energy; host sums 8.

Device pipeline per strip:
  PE    : d2 = -2*pos_i.pos_j + |pos_j|^2 via a 21-row bf16 triple-split
          matmul (exact products + fp32 PSUM accumulate -- native fp32
          matmul is fp32r, far too coarse for the |pi-pj|^2 cancellation)
  ACT   : dm = Abs(psum + |pos_i|^2 [+ 1e-3 on diagonal tiles])
  DVE   : i2 = reciprocal_approx_fast(dm)                  (~51 ULP)
          t  = (u^3 - 1/2)^2, u = i2*sigma^2               (custom op)
          acc += eps*(4t - 1)                              (custom op,
                     chained per-partition running sum)
using 4*eps*(r12 - r6) = eps*(4t - 1), t = ((s/d)^6 - 1/2)^2.

Near pairs (exact d2 < 0.02): the reference's fp32 rounding of
|pi|^2+|pj|^2-2pi.pj is quantized at ~1.9e-6 and amplified x6 by r12 (the
single nearest pair carries ~96% of the total energy). The host finds
them with an O(N) spatial hash, replicates the reference's fp32 d2
bitwise (numpy sgemm == jax CPU, verified), zeroes those sigma/eps in the
packed tiles, and routes them through the same device chain as a small
"virtual pairs" tile with host-supplied dm.
"""

import itertools
import sys
from collections import defaultdict
from operator import add as _op_add

import numpy as np

sys.path.insert(0, "/opt/trn_rl_repo")

import ml_dtypes  # noqa: E402
from concourse import bass, bacc, mybir, tile  # noqa: E402
from concourse.bass_utils import run_bass_kernel_spmd  # noqa: E402
from concourse import dve_ops  # noqa: E402
from concourse.dve_ops import DveOp, OPS  # noqa: E402
from concourse.dve_spec import (  # noqa: E402
    Spec, Src0, Src1, C0, C1, C2, sq, lower, _has_src1,
)
from concourse.dve_uop import DveOpSpec  # noqa: E402

N_ATOMS = 8192
N_CORES = 8
RB = 128
CT = 512
N_RB = N_ATOMS // RB
N_CT = N_ATOMS // CT
TILES_PER_STRIP = 4            # packing granularity (dram layout unit)
STRIP_W = TILES_PER_STRIP * CT  # dram strips stay [128, 2048]
FUSE = 1                        # DVE processes FUSE dram strips per pass
CAND_D2 = 0.02
KROWS = 21
DIAG_EPS = 1e-3   # keeps diagonal-tile dm safely > 0 for the reciprocal

F32 = mybir.dt.float32
BF16 = mybir.dt.bfloat16
AF = mybir.ActivationFunctionType
ALU = mybir.AluOpType
PI = float(np.pi)

LAST_DEBUG = {}


# --------------------------------------------------------------------------
# custom DVE ops
# --------------------------------------------------------------------------
def _register_custom_op(name, spec, subdim=False):
    for o in OPS:
        if o.name == name:
            return o
    row = dve_ops._CUSTOM_DVE_ROW_BASE + len(OPS)
    dve_ops._SUB_OPCODE_FOR_NAME[name] = row
    shas = {}
    for ver in ("v3", "v4"):
        s = DveOpSpec(name=name, opcode=row, uops=lower(spec, ver=ver),
                      rd1_en=_has_src1(spec))
        shas[ver] = s.sha(ver)
    op = DveOp(name, spec, subdim=subdim, uops_sha=shas)
    OPS.append(op)
    dve_ops.CUSTOM_DVE_SPECS[name] = spec
    return op


def _lj_t_ref(in0, in1, s0, s1, imm2):
    u = (in0.astype(np.float32) * (in1.astype(np.float32) ** 2)).astype(np.float32)
    u3 = (u * u * u).astype(np.float32)
    return ((u3 + s0) ** 2).astype(np.float32)


_u = Src0 * sq(Src1)
_u3 = sq(_u) * _u
LJ_T = _register_custom_op("LJ_T_ANT", Spec(body=sq(_u3 + C0), reference=_lj_t_ref))


def _lj_acc_ref(in0, in1, s0, s1, imm2):
    b = (in0.astype(np.float32)
         * (in1.astype(np.float32) * s1 + imm2)).astype(np.float32)
    return b, s0 + b.reshape(b.shape[0], -1).sum(-1, keepdims=True)


LJ_ACC = _register_custom_op(
    "LJ_ACC_ANT",
    Spec(body=Src0 * (Src1 * C1 + C2), accum=_op_add, accum_init=C0,
         reference=_lj_acc_ref))


def _lj_recip_mul_ref(in0, in1, s0, s1, imm2):
    not_x = (~np.ascontiguousarray(in0, np.float32).view(np.int32)).view(np.float32)
    y0 = (not_x * np.float32(s0)).astype(np.float32)
    y1 = (y0 * (np.float32(s1) - in0 * y0)).astype(np.float32)
    return ((in1.astype(np.float32) * in1) * y1).astype(np.float32)


from concourse.dve_spec import Bin, AluOp as _AluOp
_ny0 = Bin(_AluOp.BITWISE_NOT, Src0, Src0) * C0
_ny1 = _ny0 * (C1 - Src0 * _ny0)
LJ_RECIP_MUL = _register_custom_op(
    "LJ_RECIP_MUL_ANT",
    Spec(body=sq(Src1) * _ny1, reference=_lj_recip_mul_ref))


def _lj_tail_ref(in0, in1, s0, s1, imm2):
    u3 = (in0.astype(np.float32) ** 2 * in0).astype(np.float32)
    w2 = ((u3 + s0) * s1).astype(np.float32)
    b = ((w2 * w2 + imm2) * in1.astype(np.float32)).astype(np.float32)
    return b, b.reshape(b.shape[0], -1).sum(-1, keepdims=True)


_tu3 = sq(Src0) * Src0
_tw2 = (_tu3 + C0) * C1
LJ_TAIL = _register_custom_op(
    "LJ_TAIL_ANT",
    Spec(body=(sq(_tw2) + C2) * Src1, accum=_op_add,
         reference=_lj_tail_ref))


def _mul_sq_acc_ref(in0, in1, s0, s1, imm2):
    b = ((in0.astype(np.float32) ** 2) * in1.astype(np.float32)).astype(np.float32)
    return b, b.reshape(b.shape[0], -1).sum(-1, keepdims=True)


MUL_SQ_ACC = _register_custom_op(
    "MUL_SQ_ACC_ANT",
    Spec(body=sq(Src0) * Src1, accum=_op_add, reference=_mul_sq_acc_ref))


def _add1_mul_acc_ref(in0, in1, s0, s1, imm2):
    b = ((in0.astype(np.float32) + np.float32(1.0))
         * in1.astype(np.float32)).astype(np.float32)
    return b, b.reshape(b.shape[0], -1).sum(-1, keepdims=True)


from concourse.dve_spec import One as _One
ADD1_MUL_ACC = _register_custom_op(
    "ADD1_MUL_ACC_ANT",
    Spec(body=(Src0 + _One) * Src1, accum=_op_add,
         reference=_add1_mul_acc_ref))


def _atan2_corr_ref(in0, in1, s0, s1, imm2):
    # in0 = sin-num, in1 = cos-den: +-pi quadrant correction for atan2
    return np.where(in1 < 0,
                    np.where(in0 < 0, np.float32(-s0), np.float32(s0)),
                    np.float32(0.0)).astype(np.float32)


from concourse.dve_spec import Zero as _Zero, select as _select
ATAN2_CORR = _register_custom_op(
    "ATAN2_CORR_ANT",
    Spec(body=_select(Src1 < _Zero,
                      _select(Src0 < _Zero, _Zero - C0, C0), _Zero),
         reference=_atan2_corr_ref))


# --------------------------------------------------------------------------
# host helpers
# --------------------------------------------------------------------------
def _bf16(x):
    y = np.ascontiguousarray(x, np.float32).view(np.uint32)
    r = ((y + np.uint32(0x8000) + ((y >> np.uint32(16)) & np.uint32(1)))
         & np.uint32(0xFFFF0000)).view(np.float32)
    return r.reshape(np.shape(x))


def _to_bf16(x):
    """Fast fp32 -> bf16 (round-to-nearest-even) via integer ops."""
    y = np.ascontiguousarray(x, np.float32).view(np.uint32)
    r = ((y + np.uint32(0x8000) + ((y >> np.uint32(16)) & np.uint32(1)))
         >> np.uint32(16)).astype(np.uint16)
    return r.view(ml_dtypes.bfloat16).reshape(np.shape(x))


def _split3(x):
    a1 = _bf16(x)
    r = (x - a1).astype(np.float32)
    a2 = _bf16(r)
    a3 = _bf16((r - a2).astype(np.float32))
    return a1, a2, a3


_SPLIT_PAIRS = [(0, 0), (0, 1), (1, 0), (0, 2), (2, 0), (1, 1)]


def _tile_list():
    tiles = []
    for rb in range(N_RB):
        for ct in range(rb * RB // CT, N_CT):
            tiles.append((rb, ct))
    return tiles


def _find_candidates(pos):
    p = pos.astype(np.float64)
    cell = 0.15
    keys = np.floor(p / cell).astype(np.int64)
    grid = defaultdict(list)
    for idx in range(p.shape[0]):
        grid[tuple(keys[idx])].append(idx)
    offs = list(itertools.product((-1, 0, 1), repeat=3))
    cand = set()
    for key, members in grid.items():
        for off in offs:
            other = grid.get((key[0] + off[0], key[1] + off[1], key[2] + off[2]))
            if not other:
                continue
            for i in members:
                pi = p[i]
                for j in other:
                    if j > i:
                        d = pi - p[j]
                        if d[0] * d[0] + d[1] * d[1] + d[2] * d[2] < CAND_D2:
                            cand.add((i, j))
    return sorted(cand)


def _ref_d2_for_pairs(pos, pairs):
    """Bitwise replication of the reference's fp32 d2 for the given pairs."""
    if not pairs:
        return np.zeros(0, np.float32)
    sq32 = np.sum(pos * pos, axis=-1)
    rows = sorted({i for i, _ in pairs})
    ridx = {i: k for k, i in enumerate(rows)}
    dmat = (sq32[rows][:, None] + sq32[None, :]
            - np.float32(2.0) * (pos[rows] @ pos.T))
    return np.array([dmat[ridx[i], j] for i, j in pairs], np.float32)


def _pack_fields(fields, n_items):
    npart = n_items // 128
    out = np.empty((128, len(fields) * npart), np.float32)
    for f, arr in enumerate(fields):
        out[:, f * npart:(f + 1) * npart] = np.asarray(arr, np.float32).reshape(128, npart)
    return out


# --------------------------------------------------------------------------
# device program
# --------------------------------------------------------------------------
_PROGRAM_CACHE = {}


def _build_program(n_strips, vw, nb, na, nd):
    key = (n_strips, vw, nb, na, nd)
    if key in _PROGRAM_CACHE:
        return _PROGRAM_CACHE[key]

    nc = bacc.Bacc("TRN2", target_bir_lowering=False, debug=False,
                   num_devices=N_CORES)
    n_tiles = n_strips * TILES_PER_STRIP
    sig_d = nc.dram_tensor("sig", [n_strips, RB, STRIP_W], BF16, kind="ExternalInput")
    eps_d = nc.dram_tensor("eps", [n_strips, RB, STRIP_W], BF16, kind="ExternalInput")
    meta_d = nc.dram_tensor("meta", [KROWS, n_tiles * (CT + RB)], BF16,
                            kind="ExternalInput")
    sqi_d = nc.dram_tensor("sqi", [RB, n_tiles], F32, kind="ExternalInput")
    vdm_d = nc.dram_tensor("vdm", [128, vw], F32, kind="ExternalInput")
    vsig_d = nc.dram_tensor("vsig", [128, vw], F32, kind="ExternalInput")
    veps_d = nc.dram_tensor("veps", [128, vw], F32, kind="ExternalInput")
    bp_d = nc.dram_tensor("bpack", [128, 8 * nb], F32, kind="ExternalInput")
    ap_d = nc.dram_tensor("apack", [128, 11 * na], F32, kind="ExternalInput")
    dp_d = nc.dram_tensor("dpack", [128, 15 * nd], F32, kind="ExternalInput")
    out_d = nc.dram_tensor("out", [128, 4], F32, kind="ExternalOutput")

    tagn = [0]

    with tile.TileContext(nc) as tc:
        with (
            tc.tile_pool(name="const", bufs=1) as cp,
            tc.tile_pool(name="sigp", bufs=3) as sigp,
            tc.tile_pool(name="epsp", bufs=3) as epsp,
            tc.tile_pool(name="dmp", bufs=3) as dmp,
            tc.tile_pool(name="i2p", bufs=2) as i2p,
            tc.tile_pool(name="ttp", bufs=2) as ttp,
            tc.tile_pool(name="accp", bufs=3) as accp,
            tc.tile_pool(name="bw", bufs=1) as bw,
            tc.tile_pool(name="drp", bufs=1, space=bass.MemorySpace.DRAM) as drp,
            tc.tile_pool(name="psp", bufs=4, space=bass.MemorySpace.PSUM) as psp,
        ):
            def wtile(shape, pool=bw, dtype=F32):
                tagn[0] += 1
                return pool.tile(shape, dtype, tag=f"w{tagn[0]}",
                                 name=f"w{tagn[0]}")

            meta = cp.tile([KROWS, n_tiles * (CT + RB)], BF16)
            nc.sync.dma_start(meta[:], meta_d.ap())
            sqi = cp.tile([RB, n_tiles], F32)
            nc.sync.dma_start(sqi[:], sqi_d.ap())

            from concourse.dve_ops import RECIP_APPROX_FAST_CONSTS as _RC
            _rc0, _rc1 = _RC["s0"], _RC["s1"]
            naccw = max(1, n_strips)
            saccs = cp.tile([128, naccw], F32)
            nc.gpsimd.memset(saccs[:], 0.0)

            # ------------- LJ main loop (2 DVE passes / fused group) ---------
            groups = []
            s0_ = 0
            while s0_ < n_strips:
                groups.append(list(range(s0_, min(s0_ + FUSE, n_strips))))
                s0_ += FUSE
            for gi, grp in enumerate(groups):
                gw = len(grp) * STRIP_W
                sig_t = sigp.tile([RB, FUSE * STRIP_W], BF16, tag="sig")
                eps_t = epsp.tile([RB, FUSE * STRIP_W], BF16, tag="eps")
                dm_t = dmp.tile([RB, FUSE * STRIP_W], F32, tag="dm")
                for li, s in enumerate(grp):
                    off = li * STRIP_W
                    nc.sync.dma_start(sig_t[:, off:off + STRIP_W], sig_d.ap()[s])
                    nc.sync.dma_start(eps_t[:, off:off + STRIP_W], eps_d.ap()[s])
                    for h in range(2):
                        ps_t = psp.tile([128, 1024], F32, tag="ps")
                        for q in range(2):
                            tg = s * TILES_PER_STRIP + h * 2 + q
                            base = tg * (CT + RB)
                            nc.tensor.matmul(
                                ps_t[:, q * CT:(q + 1) * CT],
                                meta[:, base + CT: base + CT + RB],
                                meta[:, base: base + CT],
                                start=True, stop=True)
                            nc.scalar.activation(
                                dm_t[:, off + (h * 2 + q) * CT:off + (h * 2 + q + 1) * CT],
                                ps_t[:, q * CT:(q + 1) * CT],
                                AF.Abs, bias=sqi[:, tg:tg + 1], scale=1.0)
                u_t = i2p.tile([RB, FUSE * STRIP_W], F32, tag="i2")
                nc.vector._custom_dve(LJ_RECIP_MUL, out=u_t[:, 0:gw],
                                      in0=dm_t[:, 0:gw],
                                      in1=sig_t[:, 0:gw], s0=_rc0, s1=_rc1)
                nc.vector._custom_dve(LJ_TAIL, out=dm_t[:, 0:gw],
                                      in0=u_t[:, 0:gw],
                                      in1=eps_t[:, 0:gw], s0=-0.5, s1=2.0,
                                      imm2=-1.0, accum_out=saccs[:, gi:gi + 1])
            acc_prev = accp.tile([128, 1], F32, tag="acc")
            nc.vector.tensor_reduce(out=acc_prev[:], in_=saccs[:],
                                    axis=mybir.AxisListType.X, op=ALU.add)

            # ---------------- virtual near pairs ----------------
            vdm = cp.tile([128, vw], F32)
            nc.sync.dma_start(vdm[:], vdm_d.ap())
            vsig = cp.tile([128, vw], F32)
            nc.sync.dma_start(vsig[:], vsig_d.ap())
            veps = cp.tile([128, vw], F32)
            nc.sync.dma_start(veps[:], veps_d.ap())
            vi2 = wtile([128, vw])
            nc.vector.reciprocal_approx_fast(out=vi2[:], in_=vdm[:])
            vt = wtile([128, vw])
            nc.vector._custom_dve(LJ_T, out=vt[:], in0=vi2[:], in1=vsig[:], s0=-0.5)
            vscr = wtile([128, vw])
            acc_lj = accp.tile([128, 1], F32, tag="acc")
            nc.vector._custom_dve(LJ_ACC, out=vscr[:], in0=veps[:], in1=vt[:],
                                  s0=acc_prev[:], s1=4.0, imm2=-1.0,
                                  accum_out=acc_lj[:])

            # ---------------- bonded-term helpers ----------------
            def tt(op, a, b, shape):
                o = wtile(shape)
                nc.vector.tensor_tensor(out=o[:], in0=a, in1=b, op=op)
                return o[:]

            def ts(a, op0, s1, op1=None, s2=None, shape=None):
                o = wtile(shape)
                if op1 is None:
                    nc.vector.tensor_scalar(out=o[:], in0=a, scalar1=s1,
                                            scalar2=None, op0=op0)
                else:
                    nc.vector.tensor_scalar(out=o[:], in0=a, scalar1=s1,
                                            scalar2=s2, op0=op0, op1=op1)
                return o[:]

            def act(fn, a, shape, scale=1.0):
                o = wtile(shape)
                nc.scalar.activation(o[:], a, fn, scale=scale)
                return o[:]

            def recip(a, shape):
                o = wtile(shape)
                nc.vector.reciprocal_approx_fast(out=o[:], in_=a)
                return o[:]

            def dot3(a, b, shape):
                m = [tt(ALU.mult, a[k], b[k], shape) for k in range(3)]
                s12 = tt(ALU.add, m[0], m[1], shape)
                return tt(ALU.add, s12, m[2], shape)

            def cross(a, b, shape):
                def comp(p, q, r, s):
                    t1 = tt(ALU.mult, p, q, shape)
                    t2 = tt(ALU.mult, r, s, shape)
                    return tt(ALU.subtract, t1, t2, shape)
                return [comp(a[1], b[2], a[2], b[1]),
                        comp(a[2], b[0], a[0], b[2]),
                        comp(a[0], b[1], a[1], b[0])]

            # ---------------- bonds ----------------
            bsh = [128, nb]
            bp = cp.tile([128, 8 * nb], F32)
            nc.sync.dma_start(bp[:], bp_d.ap())
            bF = [bp[:, f * nb:(f + 1) * nb] for f in range(8)]
            bw3 = [128, 3 * nb]
            d1w = tt(ALU.subtract, bp[:, 0:3 * nb], bp[:, 3 * nb:6 * nb], bw3)
            d1sq = tt(ALU.mult, d1w, d1w, bw3)
            d2b = wtile(bsh)
            nc.vector.tensor_reduce(
                out=d2b[:], in_=d1sq.rearrange("p (c n) -> p n c", c=3),
                axis=mybir.AxisListType.X, op=ALU.add)
            d2b = d2b[:]
            bd = act(AF.Sqrt, d2b, bsh)
            db = tt(ALU.subtract, bd, bF[7], bsh)
            eb_acc = wtile([128, 1])
            ebscr = wtile(bsh)
            nc.vector._custom_dve(MUL_SQ_ACC, out=ebscr[:], in0=db,
                                  in1=bF[6], accum_out=eb_acc[:])

            # ---------------- angles ----------------
            ash = [128, na]
            apk = cp.tile([128, 11 * na], F32)
            nc.sync.dma_start(apk[:], ap_d.ap())
            aF = [apk[:, f * na:(f + 1) * na] for f in range(11)]
            aw3 = [128, 3 * na]

            def _sred(wide, n_):
                o = wtile([128, n_])
                nc.vector.tensor_reduce(
                    out=o[:], in_=wide.rearrange("p (c n) -> p n c", c=3),
                    axis=mybir.AxisListType.X, op=ALU.add)
                return o[:]

            v1w = tt(ALU.subtract, apk[:, 3 * na:6 * na], apk[:, 0:3 * na], aw3)
            v2w = tt(ALU.subtract, apk[:, 3 * na:6 * na], apk[:, 6 * na:9 * na], aw3)
            dota = _sred(tt(ALU.mult, v1w, v2w, aw3), na)
            n1sq = _sred(tt(ALU.mult, v1w, v1w, aw3), na)
            n2sq = _sred(tt(ALU.mult, v2w, v2w, aw3), na)
            den2 = tt(ALU.mult, n1sq, n2sq, ash)
            den = act(AF.Sqrt, den2, ash)
            rden = recip(den, ash)
            cosa = tt(ALU.mult, dota, rden, ash)
            c2 = tt(ALU.mult, cosa, cosa, ash)
            omc = ts(c2, ALU.mult, -1.0, ALU.add, 1.0, shape=ash)
            sroot = act(AF.Sqrt, omc, ash)
            rs = recip(sroot, ash)
            targ = tt(ALU.mult, cosa, rs, ash)
            at = act(AF.Arctan, targ, ash)
            ang = ts(at, ALU.mult, -1.0, ALU.add, PI / 2, shape=ash)
            da = tt(ALU.subtract, ang, aF[10], ash)
            ea_acc = wtile([128, 1])
            eascr = wtile(ash)
            nc.vector._custom_dve(MUL_SQ_ACC, out=eascr[:], in0=da,
                                  in1=aF[9], accum_out=ea_acc[:])

            # ---------------- dihedrals ----------------
            dsh = [128, nd]
            dpk = cp.tile([128, 15 * nd], F32)
            nc.sync.dma_start(dpk[:], dp_d.ap())
            dF = [dpk[:, f * nd:(f + 1) * nd] for f in range(15)]
            dw3 = [128, 3 * nd]
            dw9 = [128, 9 * nd]
            www = wtile(dw9)  # w1|w2|w3 in one wide tile
            nc.vector.tensor_tensor(out=www[:], in0=dpk[:, 3 * nd:12 * nd],
                                    in1=dpk[:, 0:9 * nd], op=ALU.subtract)
            w1 = [www[:, k * nd:(k + 1) * nd] for k in range(3)]
            w2 = [www[:, (3 + k) * nd:(4 + k) * nd] for k in range(3)]
            w3 = [www[:, (6 + k) * nd:(7 + k) * nd] for k in range(3)]

            def _sredd(wide, n_):
                o = wtile([128, n_])
                nc.vector.tensor_reduce(
                    out=o[:], in_=wide.rearrange("p (c n) -> p n c", c=3),
                    axis=mybir.AxisListType.X, op=ALU.add)
                return o[:]

            n1w = wtile(dw3)
            n2w = wtile(dw3)

            def cross_into(dst, a, b):
                def comp(k, p, q, r, s):
                    t1 = tt(ALU.mult, p, q, dsh)
                    t2 = tt(ALU.mult, r, s, dsh)
                    nc.vector.tensor_tensor(out=dst[:, k * nd:(k + 1) * nd],
                                            in0=t1, in1=t2, op=ALU.subtract)
                comp(0, a[1], b[2], a[2], b[1])
                comp(1, a[2], b[0], a[0], b[2])
                comp(2, a[0], b[1], a[1], b[0])

            cross_into(n1w, w1, w2)
            cross_into(n2w, w2, w3)
            cdn = _sredd(tt(ALU.mult, n1w[:], n2w[:], dw3), nd)
            # (n1 x n2).w2 == (w1.n2)*|w2|^2  (Lagrange triple product)
            det = _sredd(tt(ALU.mult, www[:, 0:3 * nd], n2w[:], dw3), nd)
            wsqw = tt(ALU.mult, www[:, 0:6 * nd], www[:, 0:6 * nd], [128, 6 * nd])
            w1sq = _sredd(wsqw[:, 0:3 * nd], nd)
            w2sq = _sredd(wsqw[:, 3 * nd:6 * nd], nd)
            n1sq_ = _sredd(tt(ALU.mult, n1w[:], n1w[:], dw3), nd)
            n2sq_ = _sredd(tt(ALU.mult, n2w[:], n2w[:], dw3), nd)
            cden2 = tt(ALU.mult, w1sq, w2sq, dsh)
            cden = act(AF.Sqrt, cden2, dsh)
            rcden = recip(cden, dsh)
            cosd = tt(ALU.mult, cdn, rcden, dsh)
            sd1 = tt(ALU.mult, w2sq, n1sq_, dsh)
            sden2 = tt(ALU.mult, sd1, n2sq_, dsh)
            sden = act(AF.Sqrt, sden2, dsh)
            rsden = recip(sden, dsh)
            sdn = tt(ALU.mult, det, w2sq, dsh)
            sind = tt(ALU.mult, sdn, rsden, dsh)
            rcosd = recip(cosd, dsh)
            qd = tt(ALU.mult, sind, rcosd, dsh)
            atq = act(AF.Arctan, qd, dsh)
            corr_t = wtile(dsh)
            nc.vector._custom_dve(ATAN2_CORR, out=corr_t[:], in0=sind,
                                  in1=cosd, s0=PI)
            dih = tt(ALU.add, atq, corr_t[:], dsh)
            narg = tt(ALU.mult, dih, dF[14], dsh)
            arg = tt(ALU.subtract, narg, dF[13], dsh)
            wr1 = wtile(dsh)
            nc.vector.add_range_wrap(out=wr1[:], in_=arg, shift=PI / 2,
                                     bound=PI, period=2 * PI)
            wr2 = wtile(dsh)
            nc.vector.add_range_wrap(out=wr2[:], in_=wr1[:], shift=0.0,
                                     bound=PI, period=2 * PI)
            sn = act(AF.Sin, wr2[:], dsh)
            ed_acc = wtile([128, 1])
            edscr = wtile(dsh)
            nc.vector._custom_dve(ADD1_MUL_ACC, out=edscr[:], in0=sn,
                                  in1=dF[12], accum_out=ed_acc[:])

            # ---------------- reductions / output ----------------
            # per-partition partials [128, 4]; final reduction happens on
            # the host together with the 8-core sum (removes the serial
            # partition-collapse tail from the device critical path)
            comb = cp.tile([128, 4], F32)
            nc.vector.tensor_copy(comb[:, 0:1], acc_lj[:])
            for col, r_ in enumerate([eb_acc, ea_acc, ed_acc]):
                nc.scalar.mul(comb[:, col + 1:col + 2], r_[:], 0.5)
            nc.sync.dma_start(out_d.ap(), comb[:])

    nc.compile()
    _PROGRAM_CACHE[key] = nc
    return nc


# --------------------------------------------------------------------------
# host packing + dispatch
# --------------------------------------------------------------------------
def _prepare_core_inputs(inputs):
    pos = np.ascontiguousarray(inputs["positions"], np.float32)
    sigma = inputs["sigma"]
    eps = inputs["epsilon"]

    tiles = _tile_list()
    n_per_core = len(tiles) // N_CORES
    n_strips = n_per_core // TILES_PER_STRIP
    core_tiles = [tiles[c * n_per_core:(c + 1) * n_per_core]
                  for c in range(N_CORES)]
    tile_owner = {}
    for c in range(N_CORES):
        for k, t in enumerate(core_tiles[c]):
            tile_owner[t] = (c, k)

    cand = _find_candidates(pos)
    cand_d2 = _ref_d2_for_pairs(pos, cand)
    vc = [[] for _ in range(N_CORES)]
    for k, pr in enumerate(cand):
        vc[k % N_CORES].append((pr, cand_d2[k]))
    vmax = max((len(v) for v in vc), default=0)
    vw = max(1, -(-max(vmax, 1) // 128))

    a1, a2, a3 = _split3(pos)
    A = [a1, a2, a3]
    sq32 = np.sum(pos * pos, axis=-1)
    sqh = _bf16(sq32)
    sql = _bf16((sq32 - sqh).astype(np.float32))
    sql2 = _bf16((sq32 - sqh - sql).astype(np.float32))

    bidx = inputs["bond_idx"]; kb = inputs["k_bond"]; r0 = inputs["r0"]
    aidx = inputs["angle_idx"]; ka = inputs["k_angle"]; th0 = inputs["theta0"]
    didx = inputs["dihedral_idx"]; kd = inputs["k_dihedral"]
    ph = inputs["default_phase"]; nm = inputs["n_mult"]
    nb = len(kb) // N_CORES // 128
    na = len(ka) // N_CORES // 128
    nd = len(kd) // N_CORES // 128

    in_maps = []
    for c in range(N_CORES):
        sig_pack = np.empty((n_strips, RB, STRIP_W), ml_dtypes.bfloat16)
        eps_pack = np.empty((n_strips, RB, STRIP_W), ml_dtypes.bfloat16)
        meta = np.zeros((KROWS, n_per_core * (CT + RB)), np.float32)
        sqi = np.empty((RB, n_per_core), np.float32)
        for k, (rb, ct) in enumerate(core_tiles[c]):
            s, slot = divmod(k, TILES_PER_STRIP)
            rs, cs = rb * RB, ct * CT
            sig_pack[s, :, slot * CT:(slot + 1) * CT] = _to_bf16(sigma[rs:rs + RB, cs:cs + CT])
            eps_pack[s, :, slot * CT:(slot + 1) * CT] = _to_bf16(eps[rs:rs + RB, cs:cs + CT])
            base = k * (CT + RB)
            for pi_, (u, v) in enumerate(_SPLIT_PAIRS):
                for ax in range(3):
                    r = pi_ * 3 + ax
                    meta[r, base: base + CT] = A[v][cs:cs + CT, ax]
                    meta[r, base + CT: base + CT + RB] = \
                        A[u][rs:rs + RB, ax] * np.float32(-2.0)
            meta[18, base: base + CT] = sqh[cs:cs + CT]
            meta[19, base: base + CT] = sql[cs:cs + CT]
            meta[20, base: base + CT] = sql2[cs:cs + CT]
            meta[18:21, base + CT: base + CT + RB] = 1.0
            diag = (ct == rb * RB // CT)
            sqi[:, k] = sq32[rs:rs + RB] + (np.float32(DIAG_EPS) if diag else np.float32(0.0))

        meta_bf = np.ascontiguousarray(_bf16(meta).astype(ml_dtypes.bfloat16))

        vdm = np.ones((128, vw), np.float32)
        vsig = np.zeros((128, vw), np.float32)
        veps = np.zeros((128, vw), np.float32)
        for k, ((i, j), d2v) in enumerate(vc[c]):
            p_, q_ = k % 128, k // 128
            dist = np.float32(np.sqrt(np.float32(max(d2v, np.float32(0.0))))) + np.float32(1e-9)
            vdm[p_, q_] = np.float32(dist * dist)
            vsig[p_, q_] = sigma[i, j]
            veps[p_, q_] = eps[i, j]

        def seg(arr, n_each):
            return np.ascontiguousarray(arr[c * n_each:(c + 1) * n_each])

        bs = seg(bidx, nb * 128)
        g1, g2 = pos[bs[:, 0]], pos[bs[:, 1]]
        bpack = _pack_fields(
            [g1[:, 0], g1[:, 1], g1[:, 2], g2[:, 0], g2[:, 1], g2[:, 2],
             seg(kb, nb * 128), seg(r0, nb * 128)], nb * 128)
        asx = seg(aidx, na * 128)
        g1, g2, g3 = pos[asx[:, 0]], pos[asx[:, 1]], pos[asx[:, 2]]
        apack = _pack_fields(
            [g1[:, 0], g1[:, 1], g1[:, 2], g2[:, 0], g2[:, 1], g2[:, 2],
             g3[:, 0], g3[:, 1], g3[:, 2],
             seg(ka, na * 128), seg(th0, na * 128)], na * 128)
        dsx = seg(didx, nd * 128)
        g1, g2, g3, g4 = (pos[dsx[:, 0]], pos[dsx[:, 1]],
                          pos[dsx[:, 2]], pos[dsx[:, 3]])
        dpack = _pack_fields(
            [g1[:, 0], g1[:, 1], g1[:, 2], g2[:, 0], g2[:, 1], g2[:, 2],
             g3[:, 0], g3[:, 1], g3[:, 2], g4[:, 0], g4[:, 1], g4[:, 2],
             seg(kd, nd * 128), seg(ph, nd * 128), seg(nm, nd * 128)],
            nd * 128)

        in_maps.append({
            "sig": sig_pack, "eps": eps_pack, "meta": meta_bf, "sqi": sqi,
            "vdm": vdm, "vsig": vsig, "veps": veps,
            "bpack": bpack, "apack": apack, "dpack": dpack,
        })

    for k, (i, j) in enumerate(cand):
        rb, ct = i // RB, j // CT
        c, kt = tile_owner[(rb, ct)]
        s, slot = divmod(kt, TILES_PER_STRIP)
        col = slot * CT + (j - ct * CT)
        in_maps[c]["sig"][s, i - rb * RB, col] = 0.0
        in_maps[c]["eps"][s, i - rb * RB, col] = 0.0

    return in_maps, (n_strips, vw, nb, na, nd)


def kernel(**inputs):
    pos = np.asarray(inputs["positions"])
    sg = np.asarray(inputs["sigma"])
    ep = np.asarray(inputs["epsilon"])
    ok = (pos.shape == (N_ATOMS, 3) and sg.shape == (N_ATOMS, N_ATOMS)
          and ep.shape == (N_ATOMS, N_ATOMS)
          and len(inputs["k_bond"]) % (N_CORES * 128) == 0
          and len(inputs["k_angle"]) % (N_CORES * 128) == 0
          and len(inputs["k_dihedral"]) % (N_CORES * 128) == 0)
    if ok:
        idx = np.arange(0, N_ATOMS, 37)
        ii, jj = np.meshgrid(idx, idx, indexing="ij")
        low = ii > jj
        if sg[ii[low], jj[low]].any() or ep[ii[low], jj[low]].any():
            ok = False
    if not ok:
        return _host_fallback(inputs)

    try:
        in_maps, geom = _prepare_core_inputs(inputs)
        nc = _build_program(*geom)
        res = None
        for attempt in range(3):
            try:
                res = run_bass_kernel_spmd(nc, in_maps,
                                           core_ids=list(range(N_CORES)))
                break
            except Exception:
                if attempt == 2:
                    raise
                import time as _time
                _time.sleep(3.0)
    except Exception:
        # no devices / toolchain failure: fall back to the (slow) host path
        return _host_fallback(inputs)
    # r["out"] is [128, 4] per-partition (lj, bond, angle, dih) partials
    terms = np.stack([r["out"].astype(np.float64).sum(axis=0)
                      for r in res.results])  # [8, 4]
    LAST_DEBUG["terms"] = terms
    return np.float32(terms.sum())


def _host_fallback(inputs):
    """Numpy replication of the fp32 reference (safety net, not fast)."""
    pos = np.asarray(inputs["positions"], np.float32)
    sigma = np.asarray(inputs["sigma"], np.float32)
    eps = np.asarray(inputs["epsilon"], np.float32)
    n = pos.shape[0]
    sq32 = np.sum(pos * pos, axis=-1)
    lj = 0.0
    chunk = 512
    for s0 in range(0, n, chunk):
        s1 = min(s0 + chunk, n)
        d2 = (sq32[s0:s1, None] + sq32[None, :]
              - np.float32(2.0) * (pos[s0:s1] @ pos.T))
        dist = (np.sqrt(np.maximum(d2, 0)) + np.float32(1e-9)).astype(np.float64)
        r6 = (sigma[s0:s1].astype(np.float64) / dist) ** 6
        lj += float((4.0 * eps[s0:s1].astype(np.float64) * (r6 * r6 - r6)).sum())
    bi, bj = inputs["bond_idx"][:, 0], inputs["bond_idx"][:, 1]
    d2b = (sq32[bi] + sq32[bj]
           - np.float32(2.0) * np.sum(pos[bi] * pos[bj], -1, dtype=np.float32))
    bd = np.sqrt(np.maximum(d2b, 0)).astype(np.float64) + 1e-9
    bond_e = float(np.sum(0.5 * inputs["k_bond"] * (bd - inputs["r0"]) ** 2))
    p64 = pos.astype(np.float64)
    ai = inputs["angle_idx"]
    p1, p2, p3 = p64[ai[:, 0]], p64[ai[:, 1]], p64[ai[:, 2]]
    v1, v2 = p2 - p1, p2 - p3
    cos_a = np.sum(v1 * v2, -1) / (np.linalg.norm(v1, axis=1)
                                   * np.linalg.norm(v2, axis=1))
    angle_e = float(np.sum(0.5 * inputs["k_angle"]
                           * (np.arccos(np.clip(cos_a, -1, 1))
                              - inputs["theta0"]) ** 2))
    di = inputs["dihedral_idx"]
    q1, q2, q3, q4 = p64[di[:, 0]], p64[di[:, 1]], p64[di[:, 2]], p64[di[:, 3]]
    w1, w2, w3 = q2 - q1, q3 - q2, q4 - q3
    cn1, cn2 = np.cross(w1, w2), np.cross(w2, w3)
    cos_d = np.sum(cn1 * cn2, -1) / (np.linalg.norm(w1, axis=1)
                                     * np.linalg.norm(w2, axis=1))
    sin_d = np.sum(np.cross(cn1, cn2) * w2, -1) / (
        np.linalg.norm(w2, axis=1) * np.linalg.norm(cn1, axis=1)
        * np.linalg.norm(cn2, axis=1))
    dih = np.arctan2(sin_d, cos_d)
    dihedral_e = float(np.sum(0.5 * inputs["k_dihedral"]
                              * (1.0 + np.cos(inputs["n_mult"] * dih
                                              - inputs["default_phase"]))))
    return np.float32(lj + bond_e + angle_e + dihedral_e)


# revision 26
# speedup vs baseline: 1.0749x; 1.0066x over previous
"""Trainium2 Bass kernel for nn_EnergyModel (bonded + Lennard-Jones energy).

Distribution: the [N,N] LJ pairwise term is upper-triangular; its 544
128x512 tiles are packed per-core (68 tiles = 17 dense [128,2048] strips)
so each of the 8 NeuronCores streams ~36MB of perfectly-sequential DMA
(half of the naive 512MB total). Positions and bonded lists are tiny and
split 1/8 per core. Each core emits one partial energy; host sums 8.

Device pipeline per strip:
  PE    : d2 = -2*pos_i.pos_j + |pos_j|^2 via a 21-row bf16 triple-split
          matmul (exact products + fp32 PSUM accumulate -- native fp32
          matmul is fp32r, far too coarse for the |pi-pj|^2 cancellation)
  ACT   : dm = Abs(psum + |pos_i|^2 [+ 1e-3 on diagonal tiles])
  DVE   : i2 = reciprocal_approx_fast(dm)                  (~51 ULP)
          t  = (u^3 - 1/2)^2, u = i2*sigma^2               (custom op)
          acc += eps*(4t - 1)                              (custom op,
                     chained per-partition running sum)
using 4*eps*(r12 - r6) = eps*(4t - 1), t = ((s/d)^6 - 1/2)^2.

Near pairs (exact d2 < 0.02): the reference's fp32 rounding of
|pi|^2+|pj|^2-2pi.pj is quantized at ~1.9e-6 and amplified x6 by r12 (the
single nearest pair carries ~96% of the total energy). The host finds
them with an O(N) spatial hash, replicates the reference's fp32 d2
bitwise (numpy sgemm == jax CPU, verified), zeroes those sigma/eps in the
packed tiles, and routes them through the same device chain as a small
"virtual pairs" tile with host-supplied dm.
"""

import itertools
import sys
from collections import defaultdict
from operator import add as _op_add

import numpy as np

sys.path.insert(0, "/opt/trn_rl_repo")

import ml_dtypes  # noqa: E402
from concourse import bass, bacc, mybir, tile  # noqa: E402
from concourse.bass_utils import run_bass_kernel_spmd  # noqa: E402
from concourse import dve_ops  # noqa: E402
from concourse.dve_ops import DveOp, OPS  # noqa: E402
from concourse.dve_spec import (  # noqa: E402
    Spec, Src0, Src1, C0, C1, C2, sq, lower, _has_src1,
)
from concourse.dve_uop import DveOpSpec  # noqa: E402

N_ATOMS = 8192
N_CORES = 8
RB = 128
CT = 512
N_RB = N_ATOMS // RB
N_CT = N_ATOMS // CT
TILES_PER_STRIP = 4            # packing granularity (dram layout unit)
STRIP_W = TILES_PER_STRIP * CT  # dram strips stay [128, 2048]
FUSE = 1                        # DVE processes FUSE dram strips per pass
CAND_D2 = 0.02
KROWS = 21
DIAG_EPS = 1e-3   # keeps diagonal-tile dm safely > 0 for the reciprocal

F32 = mybir.dt.float32
BF16 = mybir.dt.bfloat16
AF = mybir.ActivationFunctionType
ALU = mybir.AluOpType
PI = float(np.pi)

LAST_DEBUG = {}


# --------------------------------------------------------------------------
# custom DVE ops
# --------------------------------------------------------------------------
def _register_custom_op(name, spec, subdim=False):
    for o in OPS:
        if o.name == name:
            return o
    row = dve_ops._CUSTOM_DVE_ROW_BASE + len(OPS)
    dve_ops._SUB_OPCODE_FOR_NAME[name] = row
    shas = {}
    for ver in ("v3", "v4"):
        s = DveOpSpec(name=name, opcode=row, uops=lower(spec, ver=ver),
                      rd1_en=_has_src1(spec))
        shas[ver] = s.sha(ver)
    op = DveOp(name, spec, subdim=subdim, uops_sha=shas)
    OPS.append(op)
    dve_ops.CUSTOM_DVE_SPECS[name] = spec
    return op


def _lj_t_ref(in0, in1, s0, s1, imm2):
    u = (in0.astype(np.float32) * (in1.astype(np.float32) ** 2)).astype(np.float32)
    u3 = (u * u * u).astype(np.float32)
    return ((u3 + s0) ** 2).astype(np.float32)


_u = Src0 * sq(Src1)
_u3 = sq(_u) * _u
LJ_T = _register_custom_op("LJ_T_ANT", Spec(body=sq(_u3 + C0), reference=_lj_t_ref))


def _lj_acc_ref(in0, in1, s0, s1, imm2):
    b = (in0.astype(np.float32)
         * (in1.astype(np.float32) * s1 + imm2)).astype(np.float32)
    return b, s0 + b.reshape(b.shape[0], -1).sum(-1, keepdims=True)


LJ_ACC = _register_custom_op(
    "LJ_ACC_ANT",
    Spec(body=Src0 * (Src1 * C1 + C2), accum=_op_add, accum_init=C0,
         reference=_lj_acc_ref))


def _lj_recip_mul_ref(in0, in1, s0, s1, imm2):
    not_x = (~np.ascontiguousarray(in0, np.float32).view(np.int32)).view(np.float32)
    y0 = (not_x * np.float32(s0)).astype(np.float32)
    y1 = (y0 * (np.float32(s1) - in0 * y0)).astype(np.float32)
    return ((in1.astype(np.float32) * in1) * y1).astype(np.float32)


from concourse.dve_spec import Bin, AluOp as _AluOp
_ny0 = Bin(_AluOp.BITWISE_NOT, Src0, Src0) * C0
_ny1 = _ny0 * (C1 - Src0 * _ny0)
LJ_RECIP_MUL = _register_custom_op(
    "LJ_RECIP_MUL_ANT",
    Spec(body=sq(Src1) * _ny1, reference=_lj_recip_mul_ref))


def _lj_tail_ref(in0, in1, s0, s1, imm2):
    u3 = (in0.astype(np.float32) ** 2 * in0).astype(np.float32)
    w2 = ((u3 + s0) * s1).astype(np.float32)
    b = ((w2 * w2 + imm2) * in1.astype(np.float32)).astype(np.float32)
    return b, b.reshape(b.shape[0], -1).sum(-1, keepdims=True)


_tu3 = sq(Src0) * Src0
_tw2 = (_tu3 + C0) * C1
LJ_TAIL = _register_custom_op(
    "LJ_TAIL_ANT",
    Spec(body=(sq(_tw2) + C2) * Src1, accum=_op_add,
         reference=_lj_tail_ref))


def _mul_sq_acc_ref(in0, in1, s0, s1, imm2):
    b = ((in0.astype(np.float32) ** 2) * in1.astype(np.float32)).astype(np.float32)
    return b, b.reshape(b.shape[0], -1).sum(-1, keepdims=True)


MUL_SQ_ACC = _register_custom_op(
    "MUL_SQ_ACC_ANT",
    Spec(body=sq(Src0) * Src1, accum=_op_add, reference=_mul_sq_acc_ref))


def _add1_mul_acc_ref(in0, in1, s0, s1, imm2):
    b = ((in0.astype(np.float32) + np.float32(1.0))
         * in1.astype(np.float32)).astype(np.float32)
    return b, b.reshape(b.shape[0], -1).sum(-1, keepdims=True)


from concourse.dve_spec import One as _One
ADD1_MUL_ACC = _register_custom_op(
    "ADD1_MUL_ACC_ANT",
    Spec(body=(Src0 + _One) * Src1, accum=_op_add,
         reference=_add1_mul_acc_ref))


def _atan2_corr_ref(in0, in1, s0, s1, imm2):
    # in0 = sin-num, in1 = cos-den: +-pi quadrant correction for atan2
    return np.where(in1 < 0,
                    np.where(in0 < 0, np.float32(-s0), np.float32(s0)),
                    np.float32(0.0)).astype(np.float32)


from concourse.dve_spec import Zero as _Zero, select as _select
ATAN2_CORR = _register_custom_op(
    "ATAN2_CORR_ANT",
    Spec(body=_select(Src1 < _Zero,
                      _select(Src0 < _Zero, _Zero - C0, C0), _Zero),
         reference=_atan2_corr_ref))


# --------------------------------------------------------------------------
# host helpers
# --------------------------------------------------------------------------
def _bf16(x):
    y = np.ascontiguousarray(x, np.float32).view(np.uint32)
    r = ((y + np.uint32(0x8000) + ((y >> np.uint32(16)) & np.uint32(1)))
         & np.uint32(0xFFFF0000)).view(np.float32)
    return r.reshape(np.shape(x))


def _to_bf16(x):
    """Fast fp32 -> bf16 (round-to-nearest-even) via integer ops."""
    y = np.ascontiguousarray(x, np.float32).view(np.uint32)
    r = ((y + np.uint32(0x8000) + ((y >> np.uint32(16)) & np.uint32(1)))
         >> np.uint32(16)).astype(np.uint16)
    return r.view(ml_dtypes.bfloat16).reshape(np.shape(x))


def _split3(x):
    a1 = _bf16(x)
    r = (x - a1).astype(np.float32)
    a2 = _bf16(r)
    a3 = _bf16((r - a2).astype(np.float32))
    return a1, a2, a3


_SPLIT_PAIRS = [(0, 0), (0, 1), (1, 0), (0, 2), (2, 0), (1, 1)]


def _tile_list():
    tiles = []
    for rb in range(N_RB):
        for ct in range(rb * RB // CT, N_CT):
            tiles.append((rb, ct))
    return tiles


def _find_candidates(pos):
    p = pos.astype(np.float64)
    cell = 0.15
    keys = np.floor(p / cell).astype(np.int64)
    grid = defaultdict(list)
    for idx in range(p.shape[0]):
        grid[tuple(keys[idx])].append(idx)
    offs = list(itertools.product((-1, 0, 1), repeat=3))
    cand = set()
    for key, members in grid.items():
        for off in offs:
            other = grid.get((key[0] + off[0], key[1] + off[1], key[2] + off[2]))
            if not other:
                continue
            for i in members:
                pi = p[i]
                for j in other:
                    if j > i:
                        d = pi - p[j]
                        if d[0] * d[0] + d[1] * d[1] + d[2] * d[2] < CAND_D2:
                            cand.add((i, j))
    return sorted(cand)


def _ref_d2_for_pairs(pos, pairs):
    """Bitwise replication of the reference's fp32 d2 for the given pairs."""
    if not pairs:
        return np.zeros(0, np.float32)
    sq32 = np.sum(pos * pos, axis=-1)
    rows = sorted({i for i, _ in pairs})
    ridx = {i: k for k, i in enumerate(rows)}
    dmat = (sq32[rows][:, None] + sq32[None, :]
            - np.float32(2.0) * (pos[rows] @ pos.T))
    return np.array([dmat[ridx[i], j] for i, j in pairs], np.float32)


def _pack_fields(fields, n_items):
    npart = n_items // 128
    out = np.empty((128, len(fields) * npart), np.float32)
    for f, arr in enumerate(fields):
        out[:, f * npart:(f + 1) * npart] = np.asarray(arr, np.float32).reshape(128, npart)
    return out


# --------------------------------------------------------------------------
# device program
# --------------------------------------------------------------------------
_PROGRAM_CACHE = {}


def _build_program(n_strips, vw, nb, na, nd):
    key = (n_strips, vw, nb, na, nd)
    if key in _PROGRAM_CACHE:
        return _PROGRAM_CACHE[key]

    nc = bacc.Bacc("TRN2", target_bir_lowering=False, debug=False,
                   num_devices=N_CORES)
    n_tiles = n_strips * TILES_PER_STRIP
    sig_d = nc.dram_tensor("sig", [n_strips, RB, STRIP_W], BF16, kind="ExternalInput")
    eps_d = nc.dram_tensor("eps", [n_strips, RB, STRIP_W], BF16, kind="ExternalInput")
    meta_d = nc.dram_tensor("meta", [KROWS, n_tiles * (CT + RB)], BF16,
                            kind="ExternalInput")
    sqi_d = nc.dram_tensor("sqi", [RB, n_tiles], F32, kind="ExternalInput")
    vdm_d = nc.dram_tensor("vdm", [128, vw], F32, kind="ExternalInput")
    vsig_d = nc.dram_tensor("vsig", [128, vw], F32, kind="ExternalInput")
    veps_d = nc.dram_tensor("veps", [128, vw], F32, kind="ExternalInput")
    bp_d = nc.dram_tensor("bpack", [128, 8 * nb], F32, kind="ExternalInput")
    ap_d = nc.dram_tensor("apack", [128, 11 * na], F32, kind="ExternalInput")
    dp_d = nc.dram_tensor("dpack", [128, 15 * nd], F32, kind="ExternalInput")
    out_d = nc.dram_tensor("out", [128, 4], F32, kind="ExternalOutput")

    tagn = [0]

    with tile.TileContext(nc) as tc:
        with (
            tc.tile_pool(name="const", bufs=1) as cp,
            tc.tile_pool(name="sigp", bufs=3) as sigp,
            tc.tile_pool(name="epsp", bufs=3) as epsp,
            tc.tile_pool(name="dmp", bufs=3) as dmp,
            tc.tile_pool(name="i2p", bufs=2) as i2p,
            tc.tile_pool(name="ttp", bufs=2) as ttp,
            tc.tile_pool(name="accp", bufs=3) as accp,
            tc.tile_pool(name="bw", bufs=1) as bw,
            tc.tile_pool(name="drp", bufs=1, space=bass.MemorySpace.DRAM) as drp,
            tc.tile_pool(name="psp", bufs=4, space=bass.MemorySpace.PSUM) as psp,
        ):
            def wtile(shape, pool=bw, dtype=F32):
                tagn[0] += 1
                return pool.tile(shape, dtype, tag=f"w{tagn[0]}",
                                 name=f"w{tagn[0]}")

            meta = cp.tile([KROWS, n_tiles * (CT + RB)], BF16)
            nc.sync.dma_start(meta[:], meta_d.ap())
            sqi = cp.tile([RB, n_tiles], F32)
            nc.sync.dma_start(sqi[:], sqi_d.ap())

            from concourse.dve_ops import RECIP_APPROX_FAST_CONSTS as _RC
            _rc0, _rc1 = _RC["s0"], _RC["s1"]
            naccw = max(1, n_strips)
            saccs = cp.tile([128, naccw], F32)
            nc.gpsimd.memset(saccs[:], 0.0)

            # ------------- LJ main loop (2 DVE passes / fused group) ---------
            groups = []
            s0_ = 0
            while s0_ < n_strips:
                groups.append(list(range(s0_, min(s0_ + FUSE, n_strips))))
                s0_ += FUSE
            for gi, grp in enumerate(groups):
                gw = len(grp) * STRIP_W
                sig_t = sigp.tile([RB, FUSE * STRIP_W], BF16, tag="sig")
                eps_t = epsp.tile([RB, FUSE * STRIP_W], BF16, tag="eps")
                dm_t = dmp.tile([RB, FUSE * STRIP_W], F32, tag="dm")
                for li, s in enumerate(grp):
                    off = li * STRIP_W
                    nc.sync.dma_start(sig_t[:, off:off + STRIP_W], sig_d.ap()[s])
                    nc.sync.dma_start(eps_t[:, off:off + STRIP_W], eps_d.ap()[s])
                    for h in range(2):
                        ps_t = psp.tile([128, 1024], F32, tag="ps")
                        for q in range(2):
                            tg = s * TILES_PER_STRIP + h * 2 + q
                            base = tg * (CT + RB)
                            nc.tensor.matmul(
                                ps_t[:, q * CT:(q + 1) * CT],
                                meta[:, base + CT: base + CT + RB],
                                meta[:, base: base + CT],
                                start=True, stop=True)
                            nc.scalar.activation(
                                dm_t[:, off + (h * 2 + q) * CT:off + (h * 2 + q + 1) * CT],
                                ps_t[:, q * CT:(q + 1) * CT],
                                AF.Abs, bias=sqi[:, tg:tg + 1], scale=1.0)
                u_t = i2p.tile([RB, FUSE * STRIP_W], F32, tag="i2")
                nc.vector._custom_dve(LJ_RECIP_MUL, out=u_t[:, 0:gw],
                                      in0=dm_t[:, 0:gw],
                                      in1=sig_t[:, 0:gw], s0=_rc0, s1=_rc1)
                nc.vector._custom_dve(LJ_TAIL, out=dm_t[:, 0:gw],
                                      in0=u_t[:, 0:gw],
                                      in1=eps_t[:, 0:gw], s0=-0.5, s1=2.0,
                                      imm2=-1.0, accum_out=saccs[:, gi:gi + 1])
            acc_prev = accp.tile([128, 1], F32, tag="acc")
            nc.vector.tensor_reduce(out=acc_prev[:], in_=saccs[:],
                                    axis=mybir.AxisListType.X, op=ALU.add)

            # ---------------- virtual near pairs ----------------
            vdm = cp.tile([128, vw], F32)
            nc.sync.dma_start(vdm[:], vdm_d.ap())
            vsig = cp.tile([128, vw], F32)
            nc.sync.dma_start(vsig[:], vsig_d.ap())
            veps = cp.tile([128, vw], F32)
            nc.sync.dma_start(veps[:], veps_d.ap())
            vi2 = wtile([128, vw])
            nc.vector.reciprocal_approx_fast(out=vi2[:], in_=vdm[:])
            vt = wtile([128, vw])
            nc.vector._custom_dve(LJ_T, out=vt[:], in0=vi2[:], in1=vsig[:], s0=-0.5)
            vscr = wtile([128, vw])
            acc_lj = accp.tile([128, 1], F32, tag="acc")
            nc.vector._custom_dve(LJ_ACC, out=vscr[:], in0=veps[:], in1=vt[:],
                                  s0=acc_prev[:], s1=4.0, imm2=-1.0,
                                  accum_out=acc_lj[:])

            # ---------------- bonded-term helpers ----------------
            def tt(op, a, b, shape):
                o = wtile(shape)
                nc.vector.tensor_tensor(out=o[:], in0=a, in1=b, op=op)
                return o[:]

            def ts(a, op0, s1, op1=None, s2=None, shape=None):
                o = wtile(shape)
                if op1 is None:
                    nc.vector.tensor_scalar(out=o[:], in0=a, scalar1=s1,
                                            scalar2=None, op0=op0)
                else:
                    nc.vector.tensor_scalar(out=o[:], in0=a, scalar1=s1,
                                            scalar2=s2, op0=op0, op1=op1)
                return o[:]

            def act(fn, a, shape, scale=1.0):
                o = wtile(shape)
                nc.scalar.activation(o[:], a, fn, scale=scale)
                return o[:]

            def recip(a, shape):
                o = wtile(shape)
                nc.vector.reciprocal_approx_fast(out=o[:], in_=a)
                return o[:]

            def dot3(a, b, shape):
                m = [tt(ALU.mult, a[k], b[k], shape) for k in range(3)]
                s12 = tt(ALU.add, m[0], m[1], shape)
                return tt(ALU.add, s12, m[2], shape)

            def cross(a, b, shape):
                def comp(p, q, r, s):
                    t1 = tt(ALU.mult, p, q, shape)
                    t2 = tt(ALU.mult, r, s, shape)
                    return tt(ALU.subtract, t1, t2, shape)
                return [comp(a[1], b[2], a[2], b[1]),
                        comp(a[2], b[0], a[0], b[2]),
                        comp(a[0], b[1], a[1], b[0])]

            # ---------------- bonds ----------------
            bsh = [128, nb]
            bp = cp.tile([128, 8 * nb], F32)
            nc.sync.dma_start(bp[:], bp_d.ap())
            bF = [bp[:, f * nb:(f + 1) * nb] for f in range(8)]
            bw3 = [128, 3 * nb]
            d1w = tt(ALU.subtract, bp[:, 0:3 * nb], bp[:, 3 * nb:6 * nb], bw3)
            d1sq = tt(ALU.mult, d1w, d1w, bw3)
            d2b = wtile(bsh)
            nc.vector.tensor_reduce(
                out=d2b[:], in_=d1sq.rearrange("p (c n) -> p n c", c=3),
                axis=mybir.AxisListType.X, op=ALU.add)
            d2b = d2b[:]
            bd = act(AF.Sqrt, d2b, bsh)
            db = tt(ALU.subtract, bd, bF[7], bsh)
            eb_acc = wtile([128, 1])
            ebscr = wtile(bsh)
            nc.vector._custom_dve(MUL_SQ_ACC, out=ebscr[:], in0=db,
                                  in1=bF[6], accum_out=eb_acc[:])

            # ---------------- angles ----------------
            ash = [128, na]
            apk = cp.tile([128, 11 * na], F32)
            nc.sync.dma_start(apk[:], ap_d.ap())
            aF = [apk[:, f * na:(f + 1) * na] for f in range(11)]
            aw3 = [128, 3 * na]

            def _sred(wide, n_):
                o = wtile([128, n_])
                nc.vector.tensor_reduce(
                    out=o[:], in_=wide.rearrange("p (c n) -> p n c", c=3),
                    axis=mybir.AxisListType.X, op=ALU.add)
                return o[:]

            v1w = tt(ALU.subtract, apk[:, 3 * na:6 * na], apk[:, 0:3 * na], aw3)
            v2w = tt(ALU.subtract, apk[:, 3 * na:6 * na], apk[:, 6 * na:9 * na], aw3)
            dota = _sred(tt(ALU.mult, v1w, v2w, aw3), na)
            n1sq = _sred(tt(ALU.mult, v1w, v1w, aw3), na)
            n2sq = _sred(tt(ALU.mult, v2w, v2w, aw3), na)
            den2 = tt(ALU.mult, n1sq, n2sq, ash)
            # stage-A recip batch: [den | cden, sden]; stage-B: [sroot | cosd]
            rbA_in = wtile([128, na + 2 * nd])
            rbA_out = wtile([128, na + 2 * nd])
            rbB_in = wtile([128, na + nd])
            rbB_out = wtile([128, na + nd])
            nc.scalar.activation(rbA_in[:, 0:na], den2, AF.Sqrt)
            rden = rbA_out[:, 0:na]
            cosa = tt(ALU.mult, dota, rden, ash)
            c2 = tt(ALU.mult, cosa, cosa, ash)
            omc = ts(c2, ALU.mult, -1.0, ALU.add, 1.0, shape=ash)
            nc.scalar.activation(rbB_in[:, 0:na], omc, AF.Sqrt)
            rs = rbB_out[:, 0:na]
            targ = tt(ALU.mult, cosa, rs, ash)
            at = act(AF.Arctan, targ, ash)
            ang = ts(at, ALU.mult, -1.0, ALU.add, PI / 2, shape=ash)
            da = tt(ALU.subtract, ang, aF[10], ash)
            ea_acc = wtile([128, 1])
            eascr = wtile(ash)
            nc.vector._custom_dve(MUL_SQ_ACC, out=eascr[:], in0=da,
                                  in1=aF[9], accum_out=ea_acc[:])

            # ---------------- dihedrals ----------------
            dsh = [128, nd]
            dpk = cp.tile([128, 15 * nd], F32)
            nc.sync.dma_start(dpk[:], dp_d.ap())
            dF = [dpk[:, f * nd:(f + 1) * nd] for f in range(15)]
            dw3 = [128, 3 * nd]
            dw9 = [128, 9 * nd]
            www = wtile(dw9)  # w1|w2|w3 in one wide tile
            nc.vector.tensor_tensor(out=www[:], in0=dpk[:, 3 * nd:12 * nd],
                                    in1=dpk[:, 0:9 * nd], op=ALU.subtract)
            w1 = [www[:, k * nd:(k + 1) * nd] for k in range(3)]
            w2 = [www[:, (3 + k) * nd:(4 + k) * nd] for k in range(3)]
            w3 = [www[:, (6 + k) * nd:(7 + k) * nd] for k in range(3)]

            def _sredd(wide, n_):
                o = wtile([128, n_])
                nc.vector.tensor_reduce(
                    out=o[:], in_=wide.rearrange("p (c n) -> p n c", c=3),
                    axis=mybir.AxisListType.X, op=ALU.add)
                return o[:]

            n1w = wtile(dw3)
            n2w = wtile(dw3)

            def cross_into(dst, a, b):
                def comp(k, p, q, r, s):
                    t1 = tt(ALU.mult, p, q, dsh)
                    t2 = tt(ALU.mult, r, s, dsh)
                    nc.vector.tensor_tensor(out=dst[:, k * nd:(k + 1) * nd],
                                            in0=t1, in1=t2, op=ALU.subtract)
                comp(0, a[1], b[2], a[2], b[1])
                comp(1, a[2], b[0], a[0], b[2])
                comp(2, a[0], b[1], a[1], b[0])

            cross_into(n1w, w1, w2)
            cross_into(n2w, w2, w3)
            cdn = _sredd(tt(ALU.mult, n1w[:], n2w[:], dw3), nd)
            # (n1 x n2).w2 == (w1.n2)*|w2|^2  (Lagrange triple product)
            det = _sredd(tt(ALU.mult, www[:, 0:3 * nd], n2w[:], dw3), nd)
            wsqw = tt(ALU.mult, www[:, 0:6 * nd], www[:, 0:6 * nd], [128, 6 * nd])
            w1sq = _sredd(wsqw[:, 0:3 * nd], nd)
            w2sq = _sredd(wsqw[:, 3 * nd:6 * nd], nd)
            n1sq_ = _sredd(tt(ALU.mult, n1w[:], n1w[:], dw3), nd)
            n2sq_ = _sredd(tt(ALU.mult, n2w[:], n2w[:], dw3), nd)
            cden2 = tt(ALU.mult, w1sq, w2sq, dsh)
            nc.scalar.activation(rbA_in[:, na:na + nd], cden2, AF.Sqrt)
            rcden = rbA_out[:, na:na + nd]
            sd1 = tt(ALU.mult, w2sq, n1sq_, dsh)
            sden2 = tt(ALU.mult, sd1, n2sq_, dsh)
            nc.scalar.activation(rbA_in[:, na + nd:na + 2 * nd], sden2, AF.Sqrt)
            rsden = rbA_out[:, na + nd:na + 2 * nd]
            nc.vector.reciprocal_approx_fast(out=rbA_out[:], in_=rbA_in[:])
            nc.vector.tensor_tensor(out=rbB_in[:, na:na + nd],
                                    in0=cdn, in1=rcden, op=ALU.mult)
            cosd = rbB_in[:, na:na + nd]
            rcosd = rbB_out[:, na:na + nd]
            nc.vector.reciprocal_approx_fast(out=rbB_out[:], in_=rbB_in[:])
            sdn = tt(ALU.mult, det, w2sq, dsh)
            sind = tt(ALU.mult, sdn, rsden, dsh)
            qd = tt(ALU.mult, sind, rcosd, dsh)
            atq = act(AF.Arctan, qd, dsh)
            corr_t = wtile(dsh)
            nc.vector._custom_dve(ATAN2_CORR, out=corr_t[:], in0=sind,
                                  in1=cosd, s0=PI)
            dih = tt(ALU.add, atq, corr_t[:], dsh)
            narg = tt(ALU.mult, dih, dF[14], dsh)
            arg = tt(ALU.subtract, narg, dF[13], dsh)
            wr1 = wtile(dsh)
            nc.vector.add_range_wrap(out=wr1[:], in_=arg, shift=PI / 2,
                                     bound=PI, period=2 * PI)
            wr2 = wtile(dsh)
            nc.vector.add_range_wrap(out=wr2[:], in_=wr1[:], shift=0.0,
                                     bound=PI, period=2 * PI)
            sn = act(AF.Sin, wr2[:], dsh)
            ed_acc = wtile([128, 1])
            edscr = wtile(dsh)
            nc.vector._custom_dve(ADD1_MUL_ACC, out=edscr[:], in0=sn,
                                  in1=dF[12], accum_out=ed_acc[:])

            # ---------------- reductions / output ----------------
            # per-partition partials [128, 4]; final reduction happens on
            # the host together with the 8-core sum (removes the serial
            # partition-collapse tail from the device critical path)
            comb = cp.tile([128, 4], F32)
            nc.vector.tensor_copy(comb[:, 0:1], acc_lj[:])
            for col, r_ in enumerate([eb_acc, ea_acc, ed_acc]):
                nc.scalar.mul(comb[:, col + 1:col + 2], r_[:], 0.5)
            nc.sync.dma_start(out_d.ap(), comb[:])

    nc.compile()
    _PROGRAM_CACHE[key] = nc
    return nc


# --------------------------------------------------------------------------
# host packing + dispatch
# --------------------------------------------------------------------------
def _prepare_core_inputs(inputs):
    pos = np.ascontiguousarray(inputs["positions"], np.float32)
    sigma = inputs["sigma"]
    eps = inputs["epsilon"]

    tiles = _tile_list()
    n_per_core = len(tiles) // N_CORES
    n_strips = n_per_core // TILES_PER_STRIP
    core_tiles = [tiles[c * n_per_core:(c + 1) * n_per_core]
                  for c in range(N_CORES)]
    tile_owner = {}
    for c in range(N_CORES):
        for k, t in enumerate(core_tiles[c]):
            tile_owner[t] = (c, k)

    cand = _find_candidates(pos)
    cand_d2 = _ref_d2_for_pairs(pos, cand)
    vc = [[] for _ in range(N_CORES)]
    for k, pr in enumerate(cand):
        vc[k % N_CORES].append((pr, cand_d2[k]))
    vmax = max((len(v) for v in vc), default=0)
    vw = max(1, -(-max(vmax, 1) // 128))

    a1, a2, a3 = _split3(pos)
    A = [a1, a2, a3]
    sq32 = np.sum(pos * pos, axis=-1)
    sqh = _bf16(sq32)
    sql = _bf16((sq32 - sqh).astype(np.float32))
    sql2 = _bf16((sq32 - sqh - sql).astype(np.float32))

    bidx = inputs["bond_idx"]; kb = inputs["k_bond"]; r0 = inputs["r0"]
    aidx = inputs["angle_idx"]; ka = inputs["k_angle"]; th0 = inputs["theta0"]
    didx = inputs["dihedral_idx"]; kd = inputs["k_dihedral"]
    ph = inputs["default_phase"]; nm = inputs["n_mult"]
    nb = len(kb) // N_CORES // 128
    na = len(ka) // N_CORES // 128
    nd = len(kd) // N_CORES // 128

    in_maps = []
    for c in range(N_CORES):
        sig_pack = np.empty((n_strips, RB, STRIP_W), ml_dtypes.bfloat16)
        eps_pack = np.empty((n_strips, RB, STRIP_W), ml_dtypes.bfloat16)
        meta = np.zeros((KROWS, n_per_core * (CT + RB)), np.float32)
        sqi = np.empty((RB, n_per_core), np.float32)
        for k, (rb, ct) in enumerate(core_tiles[c]):
            s, slot = divmod(k, TILES_PER_STRIP)
            rs, cs = rb * RB, ct * CT
            sig_pack[s, :, slot * CT:(slot + 1) * CT] = _to_bf16(sigma[rs:rs + RB, cs:cs + CT])
            eps_pack[s, :, slot * CT:(slot + 1) * CT] = _to_bf16(eps[rs:rs + RB, cs:cs + CT])
            base = k * (CT + RB)
            for pi_, (u, v) in enumerate(_SPLIT_PAIRS):
                for ax in range(3):
                    r = pi_ * 3 + ax
                    meta[r, base: base + CT] = A[v][cs:cs + CT, ax]
                    meta[r, base + CT: base + CT + RB] = \
                        A[u][rs:rs + RB, ax] * np.float32(-2.0)
            meta[18, base: base + CT] = sqh[cs:cs + CT]
            meta[19, base: base + CT] = sql[cs:cs + CT]
            meta[20, base: base + CT] = sql2[cs:cs + CT]
            meta[18:21, base + CT: base + CT + RB] = 1.0
            diag = (ct == rb * RB // CT)
            sqi[:, k] = sq32[rs:rs + RB] + (np.float32(DIAG_EPS) if diag else np.float32(0.0))

        meta_bf = np.ascontiguousarray(_bf16(meta).astype(ml_dtypes.bfloat16))

        vdm = np.ones((128, vw), np.float32)
        vsig = np.zeros((128, vw), np.float32)
        veps = np.zeros((128, vw), np.float32)
        for k, ((i, j), d2v) in enumerate(vc[c]):
            p_, q_ = k % 128, k // 128
            dist = np.float32(np.sqrt(np.float32(max(d2v, np.float32(0.0))))) + np.float32(1e-9)
            vdm[p_, q_] = np.float32(dist * dist)
            vsig[p_, q_] = sigma[i, j]
            veps[p_, q_] = eps[i, j]

        def seg(arr, n_each):
            return np.ascontiguousarray(arr[c * n_each:(c + 1) * n_each])

        bs = seg(bidx, nb * 128)
        g1, g2 = pos[bs[:, 0]], pos[bs[:, 1]]
        bpack = _pack_fields(
            [g1[:, 0], g1[:, 1], g1[:, 2], g2[:, 0], g2[:, 1], g2[:, 2],
             seg(kb, nb * 128), seg(r0, nb * 128)], nb * 128)
        asx = seg(aidx, na * 128)
        g1, g2, g3 = pos[asx[:, 0]], pos[asx[:, 1]], pos[asx[:, 2]]
        apack = _pack_fields(
            [g1[:, 0], g1[:, 1], g1[:, 2], g2[:, 0], g2[:, 1], g2[:, 2],
             g3[:, 0], g3[:, 1], g3[:, 2],
             seg(ka, na * 128), seg(th0, na * 128)], na * 128)
        dsx = seg(didx, nd * 128)
        g1, g2, g3, g4 = (pos[dsx[:, 0]], pos[dsx[:, 1]],
                          pos[dsx[:, 2]], pos[dsx[:, 3]])
        dpack = _pack_fields(
            [g1[:, 0], g1[:, 1], g1[:, 2], g2[:, 0], g2[:, 1], g2[:, 2],
             g3[:, 0], g3[:, 1], g3[:, 2], g4[:, 0], g4[:, 1], g4[:, 2],
             seg(kd, nd * 128), seg(ph, nd * 128), seg(nm, nd * 128)],
            nd * 128)

        in_maps.append({
            "sig": sig_pack, "eps": eps_pack, "meta": meta_bf, "sqi": sqi,
            "vdm": vdm, "vsig": vsig, "veps": veps,
            "bpack": bpack, "apack": apack, "dpack": dpack,
        })

    for k, (i, j) in enumerate(cand):
        rb, ct = i // RB, j // CT
        c, kt = tile_owner[(rb, ct)]
        s, slot = divmod(kt, TILES_PER_STRIP)
        col = slot * CT + (j - ct * CT)
        in_maps[c]["sig"][s, i - rb * RB, col] = 0.0
        in_maps[c]["eps"][s, i - rb * RB, col] = 0.0

    return in_maps, (n_strips, vw, nb, na, nd)


def kernel(**inputs):
    pos = np.asarray(inputs["positions"])
    sg = np.asarray(inputs["sigma"])
    ep = np.asarray(inputs["epsilon"])
    ok = (pos.shape == (N_ATOMS, 3) and sg.shape == (N_ATOMS, N_ATOMS)
          and ep.shape == (N_ATOMS, N_ATOMS)
          and len(inputs["k_bond"]) % (N_CORES * 128) == 0
          and len(inputs["k_angle"]) % (N_CORES * 128) == 0
          and len(inputs["k_dihedral"]) % (N_CORES * 128) == 0)
    if ok:
        idx = np.arange(0, N_ATOMS, 37)
        ii, jj = np.meshgrid(idx, idx, indexing="ij")
        low = ii > jj
        if sg[ii[low], jj[low]].any() or ep[ii[low], jj[low]].any():
            ok = False
    if not ok:
        return _host_fallback(inputs)

    try:
        in_maps, geom = _prepare_core_inputs(inputs)
        nc = _build_program(*geom)
        res = None
        for attempt in range(3):
            try:
                res = run_bass_kernel_spmd(nc, in_maps,
                                           core_ids=list(range(N_CORES)))
                break
            except Exception:
                if attempt == 2:
                    raise
                import time as _time
                _time.sleep(3.0)
    except Exception:
        # no devices / toolchain failure: fall back to the (slow) host path
        return _host_fallback(inputs)
    # r["out"] is [128, 4] per-partition (lj, bond, angle, dih) partials
    terms = np.stack([r["out"].astype(np.float64).sum(axis=0)
                      for r in res.results])  # [8, 4]
    LAST_DEBUG["terms"] = terms
    return np.float32(terms.sum())


def _host_fallback(inputs):
    """Numpy replication of the fp32 reference (safety net, not fast)."""
    pos = np.asarray(inputs["positions"], np.float32)
    sigma = np.asarray(inputs["sigma"], np.float32)
    eps = np.asarray(inputs["epsilon"], np.float32)
    n = pos.shape[0]
    sq32 = np.sum(pos * pos, axis=-1)
    lj = 0.0
    chunk = 512
    for s0 in range(0, n, chunk):
        s1 = min(s0 + chunk, n)
        d2 = (sq32[s0:s1, None] + sq32[None, :]
              - np.float32(2.0) * (pos[s0:s1] @ pos.T))
        dist = (np.sqrt(np.maximum(d2, 0)) + np.float32(1e-9)).astype(np.float64)
        r6 = (sigma[s0:s1].astype(np.float64) / dist) ** 6
        lj += float((4.0 * eps[s0:s1].astype(np.float64) * (r6 * r6 - r6)).sum())
    bi, bj = inputs["bond_idx"][:, 0], inputs["bond_idx"][:, 1]
    d2b = (sq32[bi] + sq32[bj]
           - np.float32(2.0) * np.sum(pos[bi] * pos[bj], -1, dtype=np.float32))
    bd = np.sqrt(np.maximum(d2b, 0)).astype(np.float64) + 1e-9
    bond_e = float(np.sum(0.5 * inputs["k_bond"] * (bd - inputs["r0"]) ** 2))
    p64 = pos.astype(np.float64)
    ai = inputs["angle_idx"]
    p1, p2, p3 = p64[ai[:, 0]], p64[ai[:, 1]], p64[ai[:, 2]]
    v1, v2 = p2 - p1, p2 - p3
    cos_a = np.sum(v1 * v2, -1) / (np.linalg.norm(v1, axis=1)
                                   * np.linalg.norm(v2, axis=1))
    angle_e = float(np.sum(0.5 * inputs["k_angle"]
                           * (np.arccos(np.clip(cos_a, -1, 1))
                              - inputs["theta0"]) ** 2))
    di = inputs["dihedral_idx"]
    q1, q2, q3, q4 = p64[di[:, 0]], p64[di[:, 1]], p64[di[:, 2]], p64[di[:, 3]]
    w1, w2, w3 = q2 - q1, q3 - q2, q4 - q3
    cn1, cn2 = np.cross(w1, w2), np.cross(w2, w3)
    cos_d = np.sum(cn1 * cn2, -1) / (np.linalg.norm(w1, axis=1)
                                     * np.linalg.norm(w2, axis=1))
    sin_d = np.sum(np.cross(cn1, cn2) * w2, -1) / (
        np.linalg.norm(w2, axis=1) * np.linalg.norm(cn1, axis=1)
        * np.linalg.norm(cn2, axis=1))
    dih = np.arctan2(sin_d, cos_d)
    dihedral_e = float(np.sum(0.5 * inputs["k_dihedral"]
                              * (1.0 + np.cos(inputs["n_mult"] * dih
                                              - inputs["default_phase"]))))
    return np.float32(lj + bond_e + angle_e + dihedral_e)
